# revision 1
# baseline (speedup 1.0000x reference)
"""Trainium2 Bass kernel for a LISTA layer (nn_ListaLayer).

Reference computation (jax, fp32):
    th = relu(Theta) + 1e-7
    xW = (y @ W) / th
    repeat 16: z = xW + (unit_threshold(z) * th @ S) / th
    out = (unit_threshold(z) * th) @ Dx
where unit_threshold(v) = sign(v) * relu(|v| - 1).

Algebraic restructure (exact): track v = z * th.  Then
    v0 = y @ W
    repeat 16:  u = soft_threshold(v, th) = sign(v) * relu(|v| - th)
                v = v0 + u @ S
    out = soft_threshold(v, th) @ Dx
This removes every divide/multiply by th (soft_threshold(v,th) = relu(v-th) - relu(-v-th)).

Distribution: data-parallel over batch rows, 8 NeuronCores, 2048 rows each.
W/Theta/S/Dx replicated; no collectives.

On-chip layout is "transposed space": the dict dimension lives on SBUF
partitions and batch on the free axis, so the per-step matmul is
    vT = v0T + S^T-contract:  matmul(lhsT=S[jtile, itile], rhs=uT[jtile, :])
with S as the stationary operand and no per-step transposes.

Precision: matmuls run as split-fp16 (hi/lo) 3-pass accumulation in fp32 PSUM,
which is end-to-end indistinguishable from fp32 (~4e-6 max abs err; the fp32
reference itself wobbles ~1e-6 vs fp64).  PE fp16 subnormals are kept (measured
on HW), so no scaling of the low halves is needed.  The first K0_FP16 steps may
run as single-pass fp16 (early-step errors wash out through the contraction;
measured 3e-5 max abs at K0=8).  All splitting/transposition of inputs is done
host-side in numpy.
"""

import numpy as np
from contextlib import ExitStack

import concourse.bass as bass
import concourse.bacc as bacc
import concourse.tile as tile
import concourse.mybir as mybir
from concourse.bass import ts, ds

P = 128
NCORES = 8
B_FULL, DIN, DD = 16384, 1024, 2048
BSH = B_FULL // NCORES      # 2048 batch rows per core
CH = 256                    # batch columns per chunk (free dim of step matmuls)
NCH = BSH // CH             # 8 chunks
IT = DD // P                # 16 dict tiles
KW = DIN // P               # 8 d_in tiles
CN = 512                    # free dim of phase-C matmuls
K0_FP16 = 8                 # leading steps in single-pass fp16 (rest split-fp16)

F16 = mybir.dt.float16
F32 = mybir.dt.float32
ADD = mybir.AluOpType.add
SUB = mybir.AluOpType.subtract
RELU = mybir.ActivationFunctionType.Relu

_built = {}


def _build(steps: int):
    """Trace + schedule + compile the SPMD kernel for `steps` unroll steps."""
    nc = bacc.Bacc("TRN2", target_bir_lowering=False, debug=False, num_devices=NCORES)

    def inp(name, shape, dt):
        return nc.dram_tensor(name, shape, dt, kind="ExternalInput").ap()

    yTh = inp("yTh", (DIN, BSH), F16)
    yTl = inp("yTl", (DIN, BSH), F16)
    Wh_d = inp("Wh", (DIN, DD), F16)
    Wl_d = inp("Wl", (DIN, DD), F16)
    Sh_d = inp("Sh", (DD, DD), F16)
    Sl_d = inp("Sl", (DD, DD), F16)
    Dxh_d = inp("Dxh", (DD, DIN), F16)
    Dxl_d = inp("Dxl", (DD, DIN), F16)
    nth_d = inp("nth", (DD,), F32)   # -(relu(Theta) + eps)
    out_d = nc.dram_tensor("out", (BSH, DIN), F32, kind="ExternalOutput").ap()

    # step t (1-based) mode: 'h' = single-pass fp16, 's' = 3-pass split-fp16
    mode = ["h" if t <= K0_FP16 else "s" for t in range(1, steps + 1)]

    with tile.TileContext(nc) as tc, ExitStack() as top:
        dram = top.enter_context(tc.tile_pool(name="dram", bufs=1, space="DRAM"))
        v0_spill = dram.tile([IT, P, BSH], F32)
        ah_spill = dram.tile([IT, P, BSH], F16)
        al_spill = dram.tile([IT, P, BSH], F16)

        thp = top.enter_context(tc.tile_pool(name="thp", bufs=1))
        nth_t = thp.tile([P, IT], F32)
        nc.sync.dma_start(nth_t[:], nth_d.rearrange("(io p) -> p io", p=P))

        # ---------------- Phase A: v0T = W^T @ y^T -> DRAM spill ----------------
        with ExitStack() as ctx:
            wpool = ctx.enter_context(tc.tile_pool(name="wpool", bufs=1))
            ypool = ctx.enter_context(tc.tile_pool(name="ypool", bufs=2))
            psA = ctx.enter_context(tc.tile_pool(name="psA", bufs=4, space="PSUM"))
            stA = ctx.enter_context(tc.tile_pool(name="stA", bufs=3))

            Wh_t = wpool.tile([P, KW, DD], F16, name="Wh_t")
            Wl_t = wpool.tile([P, KW, DD], F16, name="Wl_t")
            for ko in range(KW):
                nc.sync.dma_start(Wh_t[:, ko, :], Wh_d[ts(ko, P), :])
                nc.sync.dma_start(Wl_t[:, ko, :], Wl_d[ts(ko, P), :])

            for c in range(NCH):
                cs = ds(c * CH, CH)
                yh_t = ypool.tile([P, KW, CH], F16, tag="yh")
                yl_t = ypool.tile([P, KW, CH], F16, tag="yl")
                for ko in range(KW):
                    nc.sync.dma_start(yh_t[:, ko, :], yTh[ts(ko, P), cs])
                    nc.sync.dma_start(yl_t[:, ko, :], yTl[ts(ko, P), cs])
                for i in range(IT):
                    ps = psA.tile([P, CH], F32, tag="psA")
                    n_mm = 3 * KW
                    k = 0
                    for ko in range(KW):
                        nc.tensor.matmul(ps[:], Wh_t[:, ko, ts(i, P)], yh_t[:, ko, :],
                                         start=(k == 0), stop=(k == n_mm - 1)); k += 1
                        nc.tensor.matmul(ps[:], Wh_t[:, ko, ts(i, P)], yl_t[:, ko, :],
                                         start=False, stop=(k == n_mm - 1)); k += 1
                    for ko in range(KW):
                        nc.tensor.matmul(ps[:], Wl_t[:, ko, ts(i, P)], yh_t[:, ko, :],
                                         start=False, stop=(k == n_mm - 1)); k += 1
                    st = stA.tile([P, CH], F32, tag="stA")
                    nc.vector.tensor_copy(st[:], ps[:])
                    nc.sync.dma_start(v0_spill[i, :, cs], st[:])

        # ---------------- Phase B: 16 soft-threshold + u@S steps ----------------
        with ExitStack() as ctx:
            spool = ctx.enter_context(tc.tile_pool(name="spool", bufs=1))
            v0pool = ctx.enter_context(tc.tile_pool(name="v0pool", bufs=2))
            upool = ctx.enter_context(tc.tile_pool(name="upool", bufs=2))
            psB = ctx.enter_context(tc.tile_pool(name="psB", bufs=4, space="PSUM"))
            vp = ctx.enter_context(tc.tile_pool(name="vp", bufs=3))
            pp = ctx.enter_context(tc.tile_pool(name="pp", bufs=3))
            qp = ctx.enter_context(tc.tile_pool(name="qp", bufs=3))
            u32p = ctx.enter_context(tc.tile_pool(name="u32p", bufs=3))

            Sh_t = spool.tile([P, IT, DD], F16, name="Sh_t")
            Sl_t = spool.tile([P, IT, DD], F16, name="Sl_t")
            for jo in range(IT):
                nc.sync.dma_start(Sh_t[:, jo, :], Sh_d[ts(jo, P), :])
                nc.sync.dma_start(Sl_t[:, jo, :], Sl_d[ts(jo, P), :])

            def shrink(v_ap, i, uh_n, ul_n):
                """u = relu(v - th) - relu(-v - th); write fp16 hi (and lo if ul_n)."""
                bias = nth_t[:, i:i + 1]
                p_t = pp.tile([P, CH], F32, tag="p")
                q_t = qp.tile([P, CH], F32, tag="q")
                nc.scalar.activation(p_t[:], v_ap, RELU, bias=bias)
                nc.scalar.activation(q_t[:], v_ap, RELU, bias=bias, scale=-1.0)
                if ul_n is None:
                    nc.vector.tensor_tensor(uh_n[:, i, :], p_t[:], q_t[:], SUB)
                else:
                    u32 = u32p.tile([P, CH], F32, tag="u32")
                    nc.vector.tensor_tensor(u32[:], p_t[:], q_t[:], SUB)
                    nc.vector.tensor_copy(uh_n[:, i, :], u32[:])
                    nc.vector.tensor_tensor(ul_n[:, i, :], u32[:], uh_n[:, i, :], SUB)

            for c in range(NCH):
                cs = ds(c * CH, CH)
                v0_t = v0pool.tile([P, IT, CH], F32, tag="v0")
                nc.sync.dma_start(v0_t[:], v0_spill[:, :, cs].rearrange("io p b -> p io b"))

                # u_1 = shrink(v0)
                need_l = mode[0] == "s"
                uh_c = upool.tile([P, IT, CH], F16, tag="uh", name="uh_c")
                ul_c = upool.tile([P, IT, CH], F16, tag="ul", name="ul_c") if need_l else None
                for i in range(IT):
                    shrink(v0_t[:, i, :], i, uh_c, ul_c)

                for t in range(steps):
                    m = mode[t]
                    # u consumed this step: uh_c (+ ul_c if split)
                    nxt_need_l = True if t == steps - 1 else (mode[t + 1] == "s")
                    uh_n = upool.tile([P, IT, CH], F16, tag="uh", name="uh_n")
                    ul_n = upool.tile([P, IT, CH], F16, tag="ul", name="ul_n") if nxt_need_l else None
                    for i in range(IT):
                        ps = psB.tile([P, CH], F32, tag="psB")
                        n_mm = IT * (3 if m == "s" else 1)
                        k = 0
                        if m == "s":
                            for jo in range(IT):
                                nc.tensor.matmul(ps[:], Sh_t[:, jo, ts(i, P)], uh_c[:, jo, :],
                                                 start=(k == 0), stop=(k == n_mm - 1)); k += 1
                                nc.tensor.matmul(ps[:], Sh_t[:, jo, ts(i, P)], ul_c[:, jo, :],
                                                 start=False, stop=(k == n_mm - 1)); k += 1
                            for jo in range(IT):
                                nc.tensor.matmul(ps[:], Sl_t[:, jo, ts(i, P)], uh_c[:, jo, :],
                                                 start=False, stop=(k == n_mm - 1)); k += 1
                        else:
                            for jo in range(IT):
                                nc.tensor.matmul(ps[:], Sh_t[:, jo, ts(i, P)], uh_c[:, jo, :],
                                                 start=(k == 0), stop=(k == n_mm - 1)); k += 1
                        v_t = vp.tile([P, CH], F32, tag="v")
                        nc.vector.tensor_tensor(v_t[:], ps[:], v0_t[:, i, :], ADD)
                        shrink(v_t[:], i, uh_n, ul_n)
                    uh_c, ul_c = uh_n, ul_n

                # after `steps` steps, (uh_c, ul_c) hold a = shrink(v_final)
                nc.sync.dma_start(ah_spill[:, :, cs].rearrange("io p b -> p io b"), uh_c[:])
                nc.sync.dma_start(al_spill[:, :, cs].rearrange("io p b -> p io b"), ul_c[:])

        # ---------------- Phase C: out = a @ Dx (normal orientation) ----------------
        with ExitStack() as ctx:
            dxpool = ctx.enter_context(tc.tile_pool(name="dxpool", bufs=1))
            apool = ctx.enter_context(tc.tile_pool(name="apool", bufs=2))
            psC = ctx.enter_context(tc.tile_pool(name="psC", bufs=3, space="PSUM"))
            stC = ctx.enter_context(tc.tile_pool(name="stC", bufs=3))

            Dxh_t = dxpool.tile([P, IT, DIN], F16, name="Dxh_t")
            Dxl_t = dxpool.tile([P, IT, DIN], F16, name="Dxl_t")
            for io in range(IT):
                nc.sync.dma_start(Dxh_t[:, io, :], Dxh_d[ts(io, P), :])
                nc.sync.dma_start(Dxl_t[:, io, :], Dxl_d[ts(io, P), :])

            for c in range(NCH):
                cs = ds(c * CH, CH)
                ah_c = apool.tile([P, IT, CH], F16, tag="ah")
                al_c = apool.tile([P, IT, CH], F16, tag="al")
                nc.sync.dma_start(ah_c[:], ah_spill[:, :, cs].rearrange("io p b -> p io b"))
                nc.sync.dma_start(al_c[:], al_spill[:, :, cs].rearrange("io p b -> p io b"))
                for bt in range(CH // P):
                    for dn in range(DIN // CN):
                        ps = psC.tile([P, CN], F32, tag="psC")
                        n_mm = 3 * IT
                        k = 0
                        for io in range(IT):
                            nc.tensor.matmul(ps[:], ah_c[:, io, ts(bt, P)],
                                             Dxh_t[:, io, ts(dn, CN)],
                                             start=(k == 0), stop=(k == n_mm - 1)); k += 1
                            nc.tensor.matmul(ps[:], ah_c[:, io, ts(bt, P)],
                                             Dxl_t[:, io, ts(dn, CN)],
                                             start=False, stop=(k == n_mm - 1)); k += 1
                        for io in range(IT):
                            nc.tensor.matmul(ps[:], al_c[:, io, ts(bt, P)],
                                             Dxh_t[:, io, ts(dn, CN)],
                                             start=False, stop=(k == n_mm - 1)); k += 1
                        st = stC.tile([P, CN], F32, tag="stC")
                        nc.vector.tensor_copy(st[:], ps[:])
                        nc.sync.dma_start(out_d[ds(c * CH + bt * P, P), ts(dn, CN)], st[:])

    nc.compile()
    return nc


def _split16(x):
    hi = x.astype(np.float16)
    lo = (x - hi.astype(np.float32)).astype(np.float16)
    return hi, lo


def _prep_in_maps(y, W, Theta, S, Dx):
    y = np.ascontiguousarray(np.asarray(y, dtype=np.float32))
    W = np.asarray(W, dtype=np.float32)
    Theta = np.asarray(Theta, dtype=np.float32)
    S = np.asarray(S, dtype=np.float32)
    Dx = np.asarray(Dx, dtype=np.float32)
    assert y.shape == (B_FULL, DIN) and W.shape == (DIN, DD)
    assert S.shape == (DD, DD) and Dx.shape == (DD, DIN)

    Wh, Wl = _split16(W)
    Sh, Sl = _split16(S)
    Dxh, Dxl = _split16(Dx)
    nth = -(np.maximum(Theta, 0.0) + np.float32(1e-7))
    yT = np.ascontiguousarray(y.T)          # [DIN, B]
    yTh_f, yTl_f = _split16(yT)

    shared = dict(Wh=Wh, Wl=Wl, Sh=Sh, Sl=Sl, Dxh=Dxh, Dxl=Dxl, nth=nth)
    in_maps = []
    for c in range(NCORES):
        sl = slice(c * BSH, (c + 1) * BSH)
        in_maps.append(dict(shared, yTh=np.ascontiguousarray(yTh_f[:, sl]),
                            yTl=np.ascontiguousarray(yTl_f[:, sl])))
    return in_maps


_sharded_cache = {}


def _get_sharded(steps: int):
    """Build (once) the jitted shard_map executable for the compiled NEFF.

    Mirrors concourse.bass2jax.run_bass_via_pjrt's multi-core path, but caches
    the jit so repeated kernel() calls don't re-trace/re-compile."""
    if steps in _sharded_cache:
        return _sharded_cache[steps]
    import jax
    from jax.experimental.shard_map import shard_map
    from jax.sharding import Mesh, PartitionSpec
    from concourse import bass2jax

    if steps not in _built:
        _built[steps] = _build(steps)
    nc = _built[steps]
    bass2jax.install_neuronx_cc_hook()
    assert nc.dbg_addr is None
    partition_name = nc.partition_id_tensor.name if nc.partition_id_tensor else None

    in_names, out_names, out_avals, zero_shapes = [], [], [], []
    for alloc in nc.m.functions[0].allocations:
        if not isinstance(alloc, mybir.MemoryLocationSet):
            continue
        name = alloc.memorylocations[0].name
        if alloc.kind == "ExternalInput":
            if name != partition_name:
                in_names.append(name)
        elif alloc.kind == "ExternalOutput":
            out_names.append(name)
            shape = tuple(alloc.tensor_shape)
            dtype = mybir.dt.np(alloc.dtype)
            out_avals.append(jax.core.ShapedArray(shape, dtype))
            zero_shapes.append((shape, dtype))
    n_params = len(in_names)
    n_outs = len(out_names)
    all_in_names = in_names + out_names
    if partition_name is not None:
        all_in_names.append(partition_name)

    def _body(*args):
        operands = list(args)
        if partition_name is not None:
            operands.append(bass2jax.partition_id_tensor())
        outs = bass2jax._bass_exec_p.bind(
            *operands,
            out_avals=tuple(out_avals),
            in_names=tuple(all_in_names),
            out_names=tuple(out_names),
            lowering_input_output_aliases=(),
            sim_require_finite=True,
            sim_require_nnan=True,
            nc=nc,
        )
        return tuple(outs)

    devices = jax.devices()[:NCORES]
    mesh = Mesh(np.asarray(devices), ("core",))
    donate = tuple(range(n_params, n_params + n_outs))
    sharded = jax.jit(
        shard_map(_body, mesh=mesh,
                  in_specs=(PartitionSpec("core"),) * (n_params + n_outs),
                  out_specs=(PartitionSpec("core"),) * n_outs,
                  check_rep=False),
        donate_argnums=donate, keep_unused=True)
    entry = dict(sharded=sharded, in_names=in_names, out_names=out_names,
                 zero_shapes=zero_shapes, mesh=mesh, n_params=n_params)
    _sharded_cache[steps] = entry
    return entry


def _concat_inputs(entry, in_maps):
    return [np.concatenate([np.asarray(in_maps[c][n]) for c in range(NCORES)], axis=0)
            for n in entry["in_names"]]


def _run(entry, concat_in):
    import jax.numpy as jnp
    zeros = [np.zeros((NCORES * s[0], *s[1:]), d) for s, d in entry["zero_shapes"]]
    out_arrs = entry["sharded"](*concat_in, *zeros)
    return out_arrs


def kernel(y, W, Theta, S, Dx, unroll_steps):
    steps = int(unroll_steps)
    entry = _get_sharded(steps)
    in_maps = _prep_in_maps(y, W, Theta, S, Dx)
    out_arrs = _run(entry, _concat_inputs(entry, in_maps))
    idx = entry["out_names"].index("out")
    return np.ascontiguousarray(np.asarray(out_arrs[idx]))  # [NCORES*BSH, DIN]


def time_kernel(np_inputs, iters=6):
    """Steady-state wall time per NEFF execution (ns), device-resident inputs."""
    import jax
    from jax.sharding import NamedSharding, PartitionSpec
    steps = int(np_inputs["unroll_steps"])
    entry = _get_sharded(steps)
    in_maps = _prep_in_maps(np_inputs["y"], np_inputs["W"], np_inputs["Theta"],
                            np_inputs["S"], np_inputs["Dx"])
    concat_in = _concat_inputs(entry, in_maps)
    sh = NamedSharding(entry["mesh"], PartitionSpec("core"))
    dev_in = [jax.device_put(a, sh) for a in concat_in]
    import time as _time
    times = []
    for it in range(iters):
        zeros = [jax.device_put(np.zeros((NCORES * s[0], *s[1:]), d), sh)
                 for s, d in entry["zero_shapes"]]
        for z in zeros:
            z.block_until_ready()
        t0 = _time.perf_counter()
        outs = entry["sharded"](*dev_in, *zeros)
        for o in outs:
            o.block_until_ready()
        times.append(_time.perf_counter() - t0)
    best = min(times[1:]) if len(times) > 1 else times[0]
    print("  per-iter times (ms):", [f"{t*1e3:.1f}" for t in times])
    return best * 1e9


if __name__ == "__main__":
    rng = np.random.default_rng(0)
    inputs = dict(
        y=rng.standard_normal((B_FULL, DIN), dtype=np.float32),
        W=(rng.standard_normal((DIN, DD)) * 0.02).astype(np.float32),
        Theta=rng.random(DD, dtype=np.float32),
        S=(rng.standard_normal((DD, DD)) * 0.02).astype(np.float32),
        Dx=(rng.standard_normal((DD, DIN)) * 0.02).astype(np.float32),
        unroll_steps=16,
    )
    out = kernel(**inputs)
    print("out", out.shape, out.dtype, np.abs(out).max())



# revision 8
# speedup vs baseline: 3.5274x; 3.5274x over previous
"""Trainium2 Bass kernel for a LISTA layer (nn_ListaLayer).

Reference computation (jax, fp32):
    th = relu(Theta) + 1e-7
    xW = (y @ W) / th
    repeat 16: z = xW + (unit_threshold(z) * th @ S) / th
    out = (unit_threshold(z) * th) @ Dx
where unit_threshold(v) = sign(v) * relu(|v| - 1).

Algebraic restructure (exact): track v = z * th:
    v0 = y @ W
    repeat 16:  u = soft_threshold(v, th) = sign(v) * relu(|v| - th)
                v = v0 + u @ S
    out = soft_threshold(v, th) @ Dx

Distribution: data-parallel over batch rows, 8 NeuronCores, 2048 rows each.
W/th/S/Dx replicated; no collectives.

Numerics / performance scheme (v-space carried SCALED by 32 in fp16):
  - A:  v~0 = y16 @ f16(32*W)   (fp16 matmul, fp32 PSUM)
  - B:  16 soft-threshold + u@S steps. First NF8 steps run single-pass
        fp8-e4m3 with perf_mode=DoubleRow (2 dict-tiles contracted per pass);
        the last steps run "split-fp8" (u = uh8+ul8, S = S8H+S8L, 3 logical
        passes folded into 24 DoubleRow matmuls using (S8L[j],S8H[j]) x
        (uh[j],ul[j]) pairing for the cross terms).
        S8H = e4m3(32*S), S8L = e4m3(32*S - S8H); u is consumed UNSCALED
        (ACT applies the 2^-5 descale when emitting fp8), so psum is scaled
        32 and adds directly onto v~0.
        Shrink pipeline per tile: GPSIMD add (psum+v~0), DVE clamp
        (tensor_scalar min/max with per-partition +-32*th), GPSIMD/DVE sub,
        ACT copy->fp8 (scale 2^-5).
  - C:  a16 @ f16(Dx) (fp16), PSUM DMA'd straight to DRAM.
All phases fused per 256-column batch chunk; two chunks interleaved so the
tensor engine never waits on the shrink chain.
"""

import numpy as np
from contextlib import ExitStack

import concourse.bass as bass
import concourse.bacc as bacc
import concourse.tile as tile
import concourse.mybir as mybir
from concourse.bass import ts, ds

P = 128
NCORES = 8
B_FULL, DIN, DD = 16384, 1024, 2048
BSH = B_FULL // NCORES      # 2048 batch rows per core
CH = 256                    # batch columns per chunk
NCH = BSH // CH             # 8 chunks
IT = DD // P                # 16 dict tiles
JP = IT // 2                # 8 DoubleRow pairs
KW = DIN // P               # 8 d_in tiles
CN = 512                    # free dim of phase-C matmuls
SC = 32.0                   # global scale 2^5
NSF8 = 4                    # trailing split-fp8 steps (rest single fp8)

F8 = mybir.dt.float8e4
F16 = mybir.dt.float16
F32 = mybir.dt.float32
ADD = mybir.AluOpType.add
SUB = mybir.AluOpType.subtract
MIN = mybir.AluOpType.min
MAX = mybir.AluOpType.max
RELU = mybir.ActivationFunctionType.Relu
COPY = mybir.ActivationFunctionType.Copy
DR = mybir.MatmulPerfMode.DoubleRow

_built = {}


def _build(steps: int):
    nc = bacc.Bacc("TRN2", target_bir_lowering=False, debug=False, num_devices=NCORES)

    def inp(name, shape, dt):
        return nc.dram_tensor(name, shape, dt, kind="ExternalInput").ap()

    yT16 = inp("yT16", (DIN, BSH), F16)
    W16_d = inp("W16", (DIN, DD), F16)        # f16(32*W)
    S8_d = inp("S8", (DD, 2, DD), F8)         # [j, (lo,hi), :] interleaved
    Dx16_d = inp("Dx16", (DD, DIN), F16)
    pth_d = inp("pth", (DD,), F32)            # +32*th
    nth_d = inp("nth", (DD,), F32)            # -32*th
    nthu_d = inp("nthu", (DD,), F32)          # -th (unscaled)
    out_d = nc.dram_tensor("out", (BSH, DIN), F32, kind="ExternalOutput").ap()

    n_sf8 = min(NSF8, steps)
    n_f8 = steps - n_sf8
    mode = ["f8"] * n_f8 + ["sf8"] * n_sf8    # mode[t] for step t (0-based)

    with tile.TileContext(nc) as tc, ExitStack() as top:
        thp = top.enter_context(tc.tile_pool(name="thp", bufs=1))
        pth_t = thp.tile([P, IT], F32)
        nth_t = thp.tile([P, IT], F32)
        nthu_t = thp.tile([P, IT], F32)
        nc.sync.dma_start(pth_t[:], pth_d.rearrange("(io p) -> p io", p=P))
        nc.sync.dma_start(nth_t[:], nth_d.rearrange("(io p) -> p io", p=P))
        nc.sync.dma_start(nthu_t[:], nthu_d.rearrange("(io p) -> p io", p=P))

        wp = top.enter_context(tc.tile_pool(name="wp", bufs=1))
        W_t = wp.tile([P, KW, DD], F16, name="W_t")
        for k in range(KW):
            nc.sync.dma_start(W_t[:, k, :], W16_d[ts(k, P), :])

        sp = top.enter_context(tc.tile_pool(name="sp", bufs=1))
        S_t = sp.tile([P, IT, 2, DD], F8, name="S_t")   # slot0=lo, slot1=hi
        for j in range(IT):
            nc.sync.dma_start(S_t[:, j, :, :], S8_d[ts(j, P), :, :])

        dxp = top.enter_context(tc.tile_pool(name="dxp", bufs=1))
        Dx_t = dxp.tile([P, IT, DIN], F16, name="Dx_t")
        for io in range(IT):
            nc.sync.dma_start(Dx_t[:, io, :], Dx16_d[ts(io, P), :])

        yp = top.enter_context(tc.tile_pool(name="yp", bufs=2))
        v0p = top.enter_context(tc.tile_pool(name="v0p", bufs=2))
        up = top.enter_context(tc.tile_pool(name="up", bufs=3))
        app = top.enter_context(tc.tile_pool(name="app", bufs=1))
        psp = top.enter_context(tc.tile_pool(name="psp", bufs=4, space="PSUM"))
        vp = top.enter_context(tc.tile_pool(name="vp", bufs=3))
        cp = top.enter_context(tc.tile_pool(name="cp", bufs=2))
        u16p = top.enter_context(tc.tile_pool(name="u16p", bufs=2))
        pqp = top.enter_context(tc.tile_pool(name="pqp", bufs=4))
        u32p = top.enter_context(tc.tile_pool(name="u32p", bufs=3))
        stp = top.enter_context(tc.tile_pool(name="stp", bufs=2))

        def shrink_f8(vsrc_pair, ip, u8_t, dve_u16):
            """vsrc_pair: [P,2,CH] f16 AP (scaled-32 v). Writes uh into slot 0."""
            c_t = cp.tile([P, 2, CH], F16, tag="c")
            for s in range(2):
                i = 2 * ip + s
                nc.gpsimd.tensor_scalar(
                    c_t[:, s, :], vsrc_pair[:, s, :],
                    pth_t[:, i:i + 1], nth_t[:, i:i + 1], MIN, op1=MAX)
            u16_t = u16p.tile([P, 2, CH], F16, tag="u16")
            eng = nc.vector if dve_u16 else nc.gpsimd
            eng.tensor_tensor(u16_t[:], vsrc_pair, c_t[:], SUB)
            nc.scalar.activation(u8_t[:, 2 * ip:2 * ip + 2, 0, :], u16_t[:],
                                 COPY, scale=1.0 / SC)

        def shrink_sf8(vsrc_pair, ip, u8_t):
            """Split-fp8 shrink: uh -> slot0, ul -> slot1 (u8 slots REVERSED vs S8)."""
            for s in range(2):
                i = 2 * ip + s
                bias = nthu_t[:, i:i + 1]
                p_t = pqp.tile([P, CH], F32, tag="p")
                q_t = pqp.tile([P, CH], F32, tag="q")
                nc.scalar.activation(p_t[:], vsrc_pair[:, s, :], RELU,
                                     bias=bias, scale=1.0 / SC)
                nc.scalar.activation(q_t[:], vsrc_pair[:, s, :], RELU,
                                     bias=bias, scale=-1.0 / SC)
                u32 = u32p.tile([P, CH], F32, tag="u32")
                nc.gpsimd.tensor_tensor(u32[:], p_t[:], q_t[:], SUB)
                nc.vector.tensor_copy(u8_t[:, i, 0, :], u32[:])
                nc.gpsimd.tensor_tensor(u8_t[:, i, 1, :], u32[:], u8_t[:, i, 0, :], SUB)

        def shrink_a(vsrc_pair, ip, a_t):
            """Final shrink -> unscaled fp16 a for phase C."""
            c_t = cp.tile([P, 2, CH], F16, tag="c")
            for s in range(2):
                i = 2 * ip + s
                nc.gpsimd.tensor_scalar(
                    c_t[:, s, :], vsrc_pair[:, s, :],
                    pth_t[:, i:i + 1], nth_t[:, i:i + 1], MIN, op1=MAX)
            u16_t = u16p.tile([P, 2, CH], F16, tag="u16")
            nc.gpsimd.tensor_tensor(u16_t[:], vsrc_pair, c_t[:], SUB)
            nc.scalar.activation(a_t[:, 2 * ip:2 * ip + 2, :], u16_t[:],
                                 COPY, scale=1.0 / SC)

        def phase_a(c):
            cs = ds(c * CH, CH)
            y_t = yp.tile([P, KW, CH], F16, tag="y")
            for k in range(KW):
                nc.sync.dma_start(y_t[:, k, :], yT16[ts(k, P), cs])
            v0_t = v0p.tile([P, IT, CH], F16, tag="v0")
            for ip in range(JP):
                ps = psp.tile([P, 2, CH], F32, tag="ps")
                for s in range(2):
                    i = 2 * ip + s
                    for k in range(KW):
                        nc.tensor.matmul(ps[:, s, :], W_t[:, k, ts(i, P)],
                                         y_t[:, k, :],
                                         start=(k == 0), stop=(k == KW - 1))
                nc.scalar.activation(v0_t[:, 2 * ip:2 * ip + 2, :], ps[:],
                                     COPY, scale=1.0)
            return v0_t

        def first_shrink(v0_t, u8_t, fmt):
            for ip in range(JP):
                if fmt == "f8":
                    shrink_f8(v0_t[:, 2 * ip:2 * ip + 2, :], ip, u8_t,
                              dve_u16=(ip % 2 == 0))
                else:
                    shrink_sf8(v0_t[:, 2 * ip:2 * ip + 2, :], ip, u8_t)

        def step(t, v0_t, u8_t, u8_n, a_t):
            """One B step: psum = u@S (fp8), v~ = ps + v~0, shrink -> u8_n/a_t."""
            m = mode[t]
            last = t == steps - 1
            nxt = None if last else mode[t + 1]
            for ip in range(JP):
                ps = psp.tile([P, 2, CH], F32, tag="ps")
                for s in range(2):
                    i = 2 * ip + s
                    if m == "f8":
                        n_mm = JP
                        for jp in range(JP):
                            nc.tensor.matmul(
                                ps[:, s, :],
                                S_t[:, 2 * jp:2 * jp + 2, 1, ts(i, P)],
                                u8_t[:, 2 * jp:2 * jp + 2, 0, :],
                                start=(jp == 0), stop=(jp == n_mm - 1),
                                perf_mode=DR)
                    else:
                        # pass1: uh @ S8H (pairs), pass2: uh@S8L + ul@S8H (per j)
                        n_mm = JP + IT
                        k = 0
                        for jp in range(JP):
                            nc.tensor.matmul(
                                ps[:, s, :],
                                S_t[:, 2 * jp:2 * jp + 2, 1, ts(i, P)],
                                u8_t[:, 2 * jp:2 * jp + 2, 0, :],
                                start=(k == 0), stop=(k == n_mm - 1),
                                perf_mode=DR)
                            k += 1
                        for j in range(IT):
                            nc.tensor.matmul(
                                ps[:, s, :],
                                S_t[:, j, :, ts(i, P)],
                                u8_t[:, j, :, :],
                                start=False, stop=(k == n_mm - 1),
                                perf_mode=DR)
                            k += 1
                v_t = vp.tile([P, 2, CH], F16, tag="v")
                nc.vector.tensor_tensor(v_t[:], ps[:], v0_t[:, 2 * ip:2 * ip + 2, :], ADD)
                if last:
                    shrink_a(v_t[:], ip, a_t)
                elif nxt == "f8":
                    shrink_f8(v_t[:], ip, u8_n, dve_u16=(ip % 2 == 0))
                else:
                    shrink_sf8(v_t[:], ip, u8_n)

        def phase_c(c, a_t):
            cs_base = c * CH
            for bt in range(CH // P):
                for dn in range(DIN // CN):
                    ps = psp.tile([P, CN], F32, tag="psC")
                    for io in range(IT):
                        nc.tensor.matmul(ps[:], a_t[:, io, ts(bt, P)],
                                         Dx_t[:, io, ts(dn, CN)],
                                         start=(io == 0), stop=(io == IT - 1))
                    st = stp.tile([P, CN], F32, tag="st")
                    nc.vector.tensor_copy(st[:], ps[:])
                    nc.sync.dma_start(out_d[ds(cs_base + bt * P, P), ts(dn, CN)],
                                      st[:])

        # ---- main schedule: chunks in interleaved pairs ----
        for cp0 in range(0, NCH, 2):
            pair = (cp0, cp0 + 1)
            v0s, u8s, ats = {}, {}, {}
            for c in pair:
                v0s[c] = phase_a(c)
                u8s[c] = up.tile([P, IT, 2, CH], F8, tag="u8", name="u8_t")
                first_shrink(v0s[c], u8s[c], mode[0])
                ats[c] = None
            for t in range(steps):
                for c in pair:
                    last = t == steps - 1
                    if last:
                        u8_n = None
                        ats[c] = app.tile([P, IT, CH], F16, tag="a", name="a_t")
                    else:
                        u8_n = up.tile([P, IT, 2, CH], F8, tag="u8", name="u8_n")
                    step(t, v0s[c], u8s[c], u8_n, ats[c])
                    u8s[c] = u8_n
            for c in pair:
                phase_c(c, ats[c])

    nc.compile()
    return nc


def _prep_in_maps(y, W, Theta, S, Dx):
    import ml_dtypes
    E4 = ml_dtypes.float8_e4m3  # TRN flavor (max normal 240)

    y = np.asarray(y, dtype=np.float32)
    W = np.asarray(W, dtype=np.float32)
    Theta = np.asarray(Theta, dtype=np.float32)
    S = np.asarray(S, dtype=np.float32)
    Dx = np.asarray(Dx, dtype=np.float32)
    assert y.shape == (B_FULL, DIN) and W.shape == (DIN, DD)
    assert S.shape == (DD, DD) and Dx.shape == (DD, DIN)

    th = np.maximum(Theta, 0.0) + np.float32(1e-7)
    W16 = (SC * W).astype(np.float16)
    Ss = np.float32(SC) * S
    S8H = np.clip(Ss, -240, 240).astype(E4)
    S8L = np.clip(Ss - S8H.astype(np.float32), -240, 240).astype(E4)
    S8 = np.stack([S8L, S8H], axis=1)         # [DD, 2, DD], slot0=lo slot1=hi
    Dx16 = Dx.astype(np.float16)
    yT16 = np.ascontiguousarray(y.T).astype(np.float16)

    shared = dict(
        W16=W16, S8=np.ascontiguousarray(S8), Dx16=Dx16,
        pth=(SC * th).astype(np.float32),
        nth=(-SC * th).astype(np.float32),
        nthu=(-th).astype(np.float32),
    )
    in_maps = []
    for c in range(NCORES):
        sl = slice(c * BSH, (c + 1) * BSH)
        in_maps.append(dict(shared, yT16=np.ascontiguousarray(yT16[:, sl])))
    return in_maps


_sharded_cache = {}


def _get_sharded(steps: int):
    """Build (once) the jitted shard_map executable for the compiled NEFF."""
    if steps in _sharded_cache:
        return _sharded_cache[steps]
    import jax
    from jax.experimental.shard_map import shard_map
    from jax.sharding import Mesh, PartitionSpec
    from concourse import bass2jax

    if steps not in _built:
        _built[steps] = _build(steps)
    nc = _built[steps]
    bass2jax.install_neuronx_cc_hook()
    assert nc.dbg_addr is None
    partition_name = nc.partition_id_tensor.name if nc.partition_id_tensor else None

    in_names, out_names, out_avals, zero_shapes = [], [], [], []
    for alloc in nc.m.functions[0].allocations:
        if not isinstance(alloc, mybir.MemoryLocationSet):
            continue
        name = alloc.memorylocations[0].name
        if alloc.kind == "ExternalInput":
            if name != partition_name:
                in_names.append(name)
        elif alloc.kind == "ExternalOutput":
            out_names.append(name)
            shape = tuple(alloc.tensor_shape)
            dtype = mybir.dt.np(alloc.dtype)
            out_avals.append(jax.core.ShapedArray(shape, dtype))
            zero_shapes.append((shape, dtype))
    n_params = len(in_names)
    n_outs = len(out_names)
    all_in_names = in_names + out_names
    if partition_name is not None:
        all_in_names.append(partition_name)

    def _body(*args):
        operands = list(args)
        if partition_name is not None:
            operands.append(bass2jax.partition_id_tensor())
        outs = bass2jax._bass_exec_p.bind(
            *operands,
            out_avals=tuple(out_avals),
            in_names=tuple(all_in_names),
            out_names=tuple(out_names),
            lowering_input_output_aliases=(),
            sim_require_finite=True,
            sim_require_nnan=True,
            nc=nc,
        )
        return tuple(outs)

    devices = jax.devices()[:NCORES]
    mesh = Mesh(np.asarray(devices), ("core",))
    donate = tuple(range(n_params, n_params + n_outs))
    sharded = jax.jit(
        shard_map(_body, mesh=mesh,
                  in_specs=(PartitionSpec("core"),) * (n_params + n_outs),
                  out_specs=(PartitionSpec("core"),) * n_outs,
                  check_rep=False),
        donate_argnums=donate, keep_unused=True)
    entry = dict(sharded=sharded, in_names=in_names, out_names=out_names,
                 zero_shapes=zero_shapes, mesh=mesh, n_params=n_params)
    _sharded_cache[steps] = entry
    return entry


def _concat_inputs(entry, in_maps):
    return [np.concatenate([np.asarray(in_maps[c][n]) for c in range(NCORES)], axis=0)
            for n in entry["in_names"]]


def _run(entry, concat_in):
    zeros = [np.zeros((NCORES * s[0], *s[1:]), d) for s, d in entry["zero_shapes"]]
    out_arrs = entry["sharded"](*concat_in, *zeros)
    return out_arrs


def kernel(y, W, Theta, S, Dx, unroll_steps):
    steps = int(unroll_steps)
    entry = _get_sharded(steps)
    in_maps = _prep_in_maps(y, W, Theta, S, Dx)
    out_arrs = _run(entry, _concat_inputs(entry, in_maps))
    idx = entry["out_names"].index("out")
    return np.ascontiguousarray(np.asarray(out_arrs[idx]))  # [NCORES*BSH, DIN]


def time_kernel(np_inputs, iters=6):
    """Steady-state wall time per NEFF execution (ns), device-resident inputs."""
    import jax
    from jax.sharding import NamedSharding, PartitionSpec
    steps = int(np_inputs["unroll_steps"])
    entry = _get_sharded(steps)
    in_maps = _prep_in_maps(np_inputs["y"], np_inputs["W"], np_inputs["Theta"],
                            np_inputs["S"], np_inputs["Dx"])
    concat_in = _concat_inputs(entry, in_maps)
    sh = NamedSharding(entry["mesh"], PartitionSpec("core"))
    dev_in = [jax.device_put(a, sh) for a in concat_in]
    import time as _time
    times = []
    for it in range(iters):
        zeros = [jax.device_put(np.zeros((NCORES * s[0], *s[1:]), d), sh)
                 for s, d in entry["zero_shapes"]]
        for z in zeros:
            z.block_until_ready()
        t0 = _time.perf_counter()
        outs = entry["sharded"](*dev_in, *zeros)
        for o in outs:
            o.block_until_ready()
        times.append(_time.perf_counter() - t0)
    best = min(times[1:]) if len(times) > 1 else times[0]
    print("  per-iter times (ms):", [f"{t*1e3:.1f}" for t in times])
    return best * 1e9


if __name__ == "__main__":
    rng = np.random.default_rng(0)
    inputs = dict(
        y=rng.standard_normal((B_FULL, DIN), dtype=np.float32),
        W=(rng.standard_normal((DIN, DD)) * 0.02).astype(np.float32),
        Theta=rng.random(DD, dtype=np.float32),
        S=(rng.standard_normal((DD, DD)) * 0.02).astype(np.float32),
        Dx=(rng.standard_normal((DD, DIN)) * 0.02).astype(np.float32),
        unroll_steps=16,
    )
    out = kernel(**inputs)
    print("out", out.shape, out.dtype, np.abs(out).max())


# revision 23
# speedup vs baseline: 4.5764x; 1.2974x over previous
"""Trainium2 Bass kernel for a LISTA layer (nn_ListaLayer).

Reference computation (jax, fp32):
    th = relu(Theta) + 1e-7
    xW = (y @ W) / th
    repeat 16: z = xW + (unit_threshold(z) * th @ S) / th
    out = (unit_threshold(z) * th) @ Dx
where unit_threshold(v) = sign(v) * relu(|v| - 1).

Algebraic restructure (exact): track v = z * th:
    v0 = y @ W
    repeat 16:  u = soft_threshold(v, th) = sign(v) * relu(|v| - th)
                v = v0 + u @ S
    out = soft_threshold(v, th) @ Dx

Distribution: data-parallel over batch rows, 8 NeuronCores, 2048 rows each.
W/th/S/Dx replicated; no collectives.

Numerics / performance scheme (v-space carried SCALED by 32 in fp16):
  - A:  v~0 = y16 @ f16(32*W)   (fp16 matmul, fp32 PSUM)
  - B:  16 soft-threshold + u@S steps. First NF8 steps run single-pass
        fp8-e4m3 with perf_mode=DoubleRow (2 dict-tiles contracted per pass);
        the last steps run "split-fp8" (u = uh8+ul8, S = S8H+S8L, 3 logical
        passes folded into 24 DoubleRow matmuls using (S8L[j],S8H[j]) x
        (uh[j],ul[j]) pairing for the cross terms).
        S8H = e4m3(32*S), S8L = e4m3(32*S - S8H); u is consumed UNSCALED
        (ACT applies the 2^-5 descale when emitting fp8), so psum is scaled
        32 and adds directly onto v~0.
        Shrink pipeline per tile: GPSIMD add (psum+v~0), DVE clamp
        (tensor_scalar min/max with per-partition +-32*th), GPSIMD/DVE sub,
        ACT copy->fp8 (scale 2^-5).
  - C:  a16 @ f16(Dx) (fp16), PSUM DMA'd straight to DRAM.
All phases fused per 256-column batch chunk; two chunks interleaved so the
tensor engine never waits on the shrink chain.
"""

import numpy as np
from contextlib import ExitStack

import concourse.bass as bass
import concourse.bacc as bacc
import concourse.tile as tile
import concourse.mybir as mybir
from concourse.bass import ts, ds

P = 128
NCORES = 8
B_FULL, DIN, DD = 16384, 1024, 2048
BSH = B_FULL // NCORES      # 2048 batch rows per core
CH = 256                    # batch columns per chunk
NCH = BSH // CH             # 8 chunks
IT = DD // P                # 16 dict tiles
JP = IT // 2                # 8 DoubleRow pairs
KW = DIN // P               # 8 d_in tiles
CN = 512                    # free dim of phase-C matmuls
SC = 32.0                   # global scale 2^5
NSF8 = 4                    # trailing split-fp8 steps (rest single fp8)
GR = 2                      # dict tiles per shrink group
GRP = 4                     # dict tiles per psum/add group

F8 = mybir.dt.float8e4
F16 = mybir.dt.float16
F32 = mybir.dt.float32
ADD = mybir.AluOpType.add
SUB = mybir.AluOpType.subtract
MIN = mybir.AluOpType.min
MAX = mybir.AluOpType.max
RELU = mybir.ActivationFunctionType.Relu
COPY = mybir.ActivationFunctionType.Copy
DR = mybir.MatmulPerfMode.DoubleRow

_built = {}


def _build(steps: int):
    nc = bacc.Bacc("TRN2", target_bir_lowering=False, debug=False, num_devices=NCORES)

    def inp(name, shape, dt):
        return nc.dram_tensor(name, shape, dt, kind="ExternalInput").ap()

    yT16 = inp("yT16", (DIN, BSH), F16)
    W16_d = inp("W16", (DIN, DD), F16)        # f16(32*W)
    S8_d = inp("S8", (DD, 2, DD), F8)         # [j, (lo,hi), :] interleaved
    Dx16_d = inp("Dx16", (DD, DIN), F16)
    pth_d = inp("pth", (DD,), F32)            # +32*th
    nth_d = inp("nth", (DD,), F32)            # -32*th
    nthu_d = inp("nthu", (DD,), F32)          # -th (unscaled)
    out_d = nc.dram_tensor("out", (BSH, DIN), F32, kind="ExternalOutput").ap()

    n_sf8 = min(NSF8, steps)
    n_f8 = steps - n_sf8
    mode = ["f8"] * n_f8 + ["sf8"] * n_sf8    # mode[t] for step t (0-based)

    with tile.TileContext(nc) as tc, ExitStack() as top:
        thp = top.enter_context(tc.tile_pool(name="thp", bufs=1))
        pth_t = thp.tile([P, IT], F32)
        nth_t = thp.tile([P, IT], F32)
        nthu_t = thp.tile([P, IT], F32)
        nc.sync.dma_start(pth_t[:], pth_d.rearrange("(io p) -> p io", p=P))
        nc.sync.dma_start(nth_t[:], nth_d.rearrange("(io p) -> p io", p=P))
        nc.sync.dma_start(nthu_t[:], nthu_d.rearrange("(io p) -> p io", p=P))

        wp = top.enter_context(tc.tile_pool(name="wp", bufs=1))
        W_t = wp.tile([P, KW, DD], F16, name="W_t")
        for k in range(KW):
            nc.sync.dma_start(W_t[:, k, :], W16_d[ts(k, P), :])

        yp = top.enter_context(tc.tile_pool(name="yp", bufs=2))
        y_pre = {}
        for c in (0, 1):
            y_t = yp.tile([P, KW, CH], F16, tag="y", name="y_pre")
            for k in range(KW):
                nc.sync.dma_start(y_t[:, k, :], yT16[ts(k, P), ds(c * CH, CH)])
            y_pre[c] = y_t

        sp = top.enter_context(tc.tile_pool(name="sp", bufs=1))
        S_t = sp.tile([P, IT, 2, DD], F8, name="S_t")   # slot0=lo, slot1=hi
        for j in range(IT):
            nc.sync.dma_start(S_t[:, j, :, :], S8_d[ts(j, P), :, :])

        dxp = top.enter_context(tc.tile_pool(name="dxp", bufs=1))
        Dx_t = dxp.tile([P, IT, DIN], F16, name="Dx_t")
        for io in range(IT):
            nc.sync.dma_start(Dx_t[:, io, :], Dx16_d[ts(io, P), :])
        v0p = top.enter_context(tc.tile_pool(name="v0p", bufs=2))
        up = top.enter_context(tc.tile_pool(name="up", bufs=3))
        app = top.enter_context(tc.tile_pool(name="app", bufs=1))
        psp = top.enter_context(tc.tile_pool(name="psp", bufs=3, space="PSUM"))
        pscp = top.enter_context(tc.tile_pool(name="pscp", bufs=2, space="PSUM"))
        vp = top.enter_context(tc.tile_pool(name="vp", bufs=2))
        cp = top.enter_context(tc.tile_pool(name="cp", bufs=2))
        u16p = top.enter_context(tc.tile_pool(name="u16p", bufs=2))
        # NOTE: SBUF is within ~1KB of full; keep pool sizes in sync with budget
        pqp = top.enter_context(tc.tile_pool(name="pqp", bufs=2))
        u32p = top.enter_context(tc.tile_pool(name="u32p", bufs=2))
        stp = top.enter_context(tc.tile_pool(name="stp", bufs=2))

        def shrink_f8(vsrc_quad, q, u8_t):
            """vsrc_quad: [P,4,CH] f16 AP (scaled-32 v). Writes uh into slot 0."""
            c_t = cp.tile([P, GR, CH], F16, tag="c")
            for s in range(GR):
                i = GR * q + s
                nc.gpsimd.tensor_scalar(
                    c_t[:, s, :], vsrc_quad[:, s, :],
                    pth_t[:, i:i + 1], nth_t[:, i:i + 1], MIN, op1=MAX)
            u16_t = u16p.tile([P, GR, CH], F16, tag="u16")
            nc.vector.tensor_tensor(u16_t[:], vsrc_quad, c_t[:], SUB)
            nc.scalar.activation(u8_t[:, GR * q:GR * q + GR, 0, :], u16_t[:],
                                 COPY, scale=1.0 / SC)

        def shrink_sf8(vsrc_quad, q, u8_t):
            """Split-fp8 shrink: uh -> slot0, ul -> slot1 (u8 slots REVERSED vs S8)."""
            for s in range(GR):
                i = GR * q + s
                bias = nthu_t[:, i:i + 1]
                p_t = pqp.tile([P, CH], F32, tag="p")
                q_t = pqp.tile([P, CH], F32, tag="q")
                nc.scalar.activation(p_t[:], vsrc_quad[:, s, :], RELU,
                                     bias=bias, scale=1.0 / SC)
                nc.scalar.activation(q_t[:], vsrc_quad[:, s, :], RELU,
                                     bias=bias, scale=-1.0 / SC)
                u32 = u32p.tile([P, CH], F32, tag="u32")
                nc.vector.tensor_tensor(u32[:], p_t[:], q_t[:], SUB)
                nc.vector.tensor_copy(u8_t[:, i, 0, :], u32[:])
                nc.gpsimd.tensor_tensor(u8_t[:, i, 1, :], u32[:], u8_t[:, i, 0, :], SUB)

        def shrink_a(vsrc_quad, q, a_t):
            """Final shrink -> unscaled fp16 a for phase C."""
            c_t = cp.tile([P, GR, CH], F16, tag="c")
            for s in range(GR):
                i = GR * q + s
                nc.gpsimd.tensor_scalar(
                    c_t[:, s, :], vsrc_quad[:, s, :],
                    pth_t[:, i:i + 1], nth_t[:, i:i + 1], MIN, op1=MAX)
            u16_t = u16p.tile([P, GR, CH], F16, tag="u16")
            nc.vector.tensor_tensor(u16_t[:], vsrc_quad, c_t[:], SUB)
            nc.scalar.activation(a_t[:, GR * q:GR * q + GR, :], u16_t[:],
                                 COPY, scale=1.0 / SC)

        NQ = IT // GR   # shrink groups per step

        def phase_a(c):
            cs = ds(c * CH, CH)
            if c in y_pre:
                y_t = y_pre.pop(c)
            else:
                y_t = yp.tile([P, KW, CH], F16, tag="y")
                for k in range(KW):
                    nc.sync.dma_start(y_t[:, k, :], yT16[ts(k, P), cs])
            v0_t = v0p.tile([P, IT, CH], F16, tag="v0")
            for q in range(IT // GRP):
                ps = psp.tile([P, GRP, CH], F32, tag="ps")
                for s in range(GRP):
                    i = GRP * q + s
                    for k in range(KW):
                        nc.tensor.matmul(ps[:, s, :], W_t[:, k, ts(i, P)],
                                         y_t[:, k, :],
                                         start=(k == 0), stop=(k == KW - 1))
                nc.scalar.activation(v0_t[:, GRP * q:GRP * q + GRP, :], ps[:],
                                     COPY, scale=1.0)
            return v0_t

        def first_shrink(v0_t, u8_t, fmt):
            for q in range(NQ):
                if fmt == "f8":
                    shrink_f8(v0_t[:, GR * q:GR * q + GR, :], q, u8_t)
                else:
                    shrink_sf8(v0_t[:, GR * q:GR * q + GR, :], q, u8_t)

        def step(t, v0_t, u8_t, u8_n, a_t):
            """One B step: psum = u@S (fp8 DR), v~ = ps + v~0, shrink -> u8_n/a_t."""
            m = mode[t]
            last = t == steps - 1
            nxt = None if last else mode[t + 1]
            for qq in range(IT // GRP):
                ps = psp.tile([P, GRP, CH], F32, tag="ps")
                for s in range(GRP):
                    i = GRP * qq + s
                    if m == "f8":
                        n_mm = JP
                        for jp in range(JP):
                            nc.tensor.matmul(
                                ps[:, s, :],
                                S_t[:, 2 * jp:2 * jp + 2, 1, ts(i, P)],
                                u8_t[:, 2 * jp:2 * jp + 2, 0, :],
                                start=(jp == 0), stop=(jp == n_mm - 1),
                                perf_mode=DR)
                    else:
                        n_mm = JP + IT
                        k = 0
                        for jp in range(JP):
                            nc.tensor.matmul(
                                ps[:, s, :],
                                S_t[:, 2 * jp:2 * jp + 2, 1, ts(i, P)],
                                u8_t[:, 2 * jp:2 * jp + 2, 0, :],
                                start=(k == 0), stop=(k == n_mm - 1),
                                perf_mode=DR)
                            k += 1
                        for j in range(IT):
                            nc.tensor.matmul(
                                ps[:, s, :],
                                S_t[:, j, :, ts(i, P)],
                                u8_t[:, j, :, :],
                                start=False, stop=(k == n_mm - 1),
                                perf_mode=DR)
                            k += 1
                v_t = vp.tile([P, GRP, CH], F16, tag="v")
                nc.vector.tensor_tensor(v_t[:], ps[:],
                                        v0_t[:, GRP * qq:GRP * qq + GRP, :], ADD)
                for h in range(GRP // GR):
                    q = (GRP // GR) * qq + h
                    v_pair = v_t[:, GR * h:GR * h + GR, :]
                    if last:
                        shrink_a(v_pair, q, a_t)
                    elif nxt == "f8":
                        shrink_f8(v_pair, q, u8_n)
                    else:
                        shrink_sf8(v_pair, q, u8_n)

        def phase_c(c, a_t):
            cs_base = c * CH
            for bt in range(CH // P):
                for dn in range(DIN // CN):
                    ps = pscp.tile([P, CN], F32, tag="psC")
                    for io in range(IT):
                        nc.tensor.matmul(ps[:], a_t[:, io, ts(bt, P)],
                                         Dx_t[:, io, ts(dn, CN)],
                                         start=(io == 0), stop=(io == IT - 1))
                    st = stp.tile([P, CN], F32, tag="st")
                    nc.scalar.activation(st[:], ps[:], COPY, scale=1.0)
                    nc.sync.dma_start(out_d[ds(cs_base + bt * P, P), ts(dn, CN)],
                                      st[:])

        # ---- main schedule: chunk pairs; previous pair's C is emitted after
        # the next pair's A so its matmuls fill the shrink-chain latency ----
        pending_c = []
        for cp0 in range(0, NCH, 2):
            pair = (cp0, cp0 + 1)
            v0s, u8s, ats = {}, {}, {}
            for c in pair:
                v0s[c] = phase_a(c)
                u8s[c] = up.tile([P, IT, 2, CH], F8, tag="u8", name="u8_t")
                first_shrink(v0s[c], u8s[c], mode[0])
                ats[c] = None
            for nxt_c in (cp0 + 2, cp0 + 3):
                if nxt_c < NCH and nxt_c not in y_pre:
                    y_t = yp.tile([P, KW, CH], F16, tag="y", name="y_nxt")
                    for k in range(KW):
                        nc.sync.dma_start(y_t[:, k, :],
                                          yT16[ts(k, P), ds(nxt_c * CH, CH)])
                    y_pre[nxt_c] = y_t
            for c, a_t in pending_c:
                phase_c(c, a_t)
            pending_c = []
            for t in range(steps):
                for c in pair:
                    last = t == steps - 1
                    if last:
                        u8_n = None
                        ats[c] = app.tile([P, IT, CH], F16, tag="a", name="a_t")
                    else:
                        u8_n = up.tile([P, IT, 2, CH], F8, tag="u8", name="u8_n")
                    step(t, v0s[c], u8s[c], u8_n, ats[c])
                    u8s[c] = u8_n
            pending_c = [(c, ats[c]) for c in pair]
        for c, a_t in pending_c:
            phase_c(c, a_t)

    nc.compile()
    return nc


def _prep_in_maps(y, W, Theta, S, Dx):
    import ml_dtypes
    E4 = ml_dtypes.float8_e4m3  # TRN flavor (max normal 240)

    y = np.asarray(y, dtype=np.float32)
    W = np.asarray(W, dtype=np.float32)
    Theta = np.asarray(Theta, dtype=np.float32)
    S = np.asarray(S, dtype=np.float32)
    Dx = np.asarray(Dx, dtype=np.float32)
    assert y.shape == (B_FULL, DIN) and W.shape == (DIN, DD)
    assert S.shape == (DD, DD) and Dx.shape == (DD, DIN)

    th = np.maximum(Theta, 0.0) + np.float32(1e-7)
    W16 = (SC * W).astype(np.float16)
    Ss = np.float32(SC) * S
    S8H = np.clip(Ss, -240, 240).astype(E4)
    S8L = np.clip(Ss - S8H.astype(np.float32), -240, 240).astype(E4)
    S8 = np.stack([S8L, S8H], axis=1)         # [DD, 2, DD], slot0=lo slot1=hi
    Dx16 = Dx.astype(np.float16)
    yT16 = np.ascontiguousarray(y.T).astype(np.float16)

    shared = dict(
        W16=W16, S8=np.ascontiguousarray(S8), Dx16=Dx16,
        pth=(SC * th).astype(np.float32),
        nth=(-SC * th).astype(np.float32),
        nthu=(-th).astype(np.float32),
    )
    in_maps = []
    for c in range(NCORES):
        sl = slice(c * BSH, (c + 1) * BSH)
        in_maps.append(dict(shared, yT16=np.ascontiguousarray(yT16[:, sl])))
    return in_maps


_sharded_cache = {}


def _get_sharded(steps: int):
    """Build (once) the jitted shard_map executable for the compiled NEFF."""
    if steps in _sharded_cache:
        return _sharded_cache[steps]
    import jax
    from jax.experimental.shard_map import shard_map
    from jax.sharding import Mesh, PartitionSpec
    from concourse import bass2jax

    if steps not in _built:
        _built[steps] = _build(steps)
    nc = _built[steps]
    bass2jax.install_neuronx_cc_hook()
    assert nc.dbg_addr is None
    partition_name = nc.partition_id_tensor.name if nc.partition_id_tensor else None

    in_names, out_names, out_avals, zero_shapes = [], [], [], []
    for alloc in nc.m.functions[0].allocations:
        if not isinstance(alloc, mybir.MemoryLocationSet):
            continue
        name = alloc.memorylocations[0].name
        if alloc.kind == "ExternalInput":
            if name != partition_name:
                in_names.append(name)
        elif alloc.kind == "ExternalOutput":
            out_names.append(name)
            shape = tuple(alloc.tensor_shape)
            dtype = mybir.dt.np(alloc.dtype)
            out_avals.append(jax.core.ShapedArray(shape, dtype))
            zero_shapes.append((shape, dtype))
    n_params = len(in_names)
    n_outs = len(out_names)
    all_in_names = in_names + out_names
    if partition_name is not None:
        all_in_names.append(partition_name)

    def _body(*args):
        operands = list(args)
        if partition_name is not None:
            operands.append(bass2jax.partition_id_tensor())
        outs = bass2jax._bass_exec_p.bind(
            *operands,
            out_avals=tuple(out_avals),
            in_names=tuple(all_in_names),
            out_names=tuple(out_names),
            lowering_input_output_aliases=(),
            sim_require_finite=True,
            sim_require_nnan=True,
            nc=nc,
        )
        return tuple(outs)

    devices = jax.devices()[:NCORES]
    mesh = Mesh(np.asarray(devices), ("core",))
    donate = tuple(range(n_params, n_params + n_outs))
    sharded = jax.jit(
        shard_map(_body, mesh=mesh,
                  in_specs=(PartitionSpec("core"),) * (n_params + n_outs),
                  out_specs=(PartitionSpec("core"),) * n_outs,
                  check_rep=False),
        donate_argnums=donate, keep_unused=True)
    entry = dict(sharded=sharded, in_names=in_names, out_names=out_names,
                 zero_shapes=zero_shapes, mesh=mesh, n_params=n_params)
    _sharded_cache[steps] = entry
    return entry


def _concat_inputs(entry, in_maps):
    return [np.concatenate([np.asarray(in_maps[c][n]) for c in range(NCORES)], axis=0)
            for n in entry["in_names"]]


def _run(entry, concat_in):
    zeros = [np.zeros((NCORES * s[0], *s[1:]), d) for s, d in entry["zero_shapes"]]
    out_arrs = entry["sharded"](*concat_in, *zeros)
    return out_arrs


def kernel(y, W, Theta, S, Dx, unroll_steps):
    steps = int(unroll_steps)
    entry = _get_sharded(steps)
    in_maps = _prep_in_maps(y, W, Theta, S, Dx)
    out_arrs = _run(entry, _concat_inputs(entry, in_maps))
    idx = entry["out_names"].index("out")
    return np.ascontiguousarray(np.asarray(out_arrs[idx]))  # [NCORES*BSH, DIN]


def time_kernel(np_inputs, iters=6):
    """Steady-state wall time per NEFF execution (ns), device-resident inputs."""
    import jax
    from jax.sharding import NamedSharding, PartitionSpec
    steps = int(np_inputs["unroll_steps"])
    entry = _get_sharded(steps)
    in_maps = _prep_in_maps(np_inputs["y"], np_inputs["W"], np_inputs["Theta"],
                            np_inputs["S"], np_inputs["Dx"])
    concat_in = _concat_inputs(entry, in_maps)
    sh = NamedSharding(entry["mesh"], PartitionSpec("core"))
    dev_in = [jax.device_put(a, sh) for a in concat_in]
    import time as _time
    times = []
    for it in range(iters):
        zeros = [jax.device_put(np.zeros((NCORES * s[0], *s[1:]), d), sh)
                 for s, d in entry["zero_shapes"]]
        for z in zeros:
            z.block_until_ready()
        t0 = _time.perf_counter()
        outs = entry["sharded"](*dev_in, *zeros)
        for o in outs:
            o.block_until_ready()
        times.append(_time.perf_counter() - t0)
    best = min(times[1:]) if len(times) > 1 else times[0]
    print("  per-iter times (ms):", [f"{t*1e3:.1f}" for t in times])
    return best * 1e9


if __name__ == "__main__":
    rng = np.random.default_rng(0)
    inputs = dict(
        y=rng.standard_normal((B_FULL, DIN), dtype=np.float32),
        W=(rng.standard_normal((DIN, DD)) * 0.02).astype(np.float32),
        Theta=rng.random(DD, dtype=np.float32),
        S=(rng.standard_normal((DD, DD)) * 0.02).astype(np.float32),
        Dx=(rng.standard_normal((DD, DIN)) * 0.02).astype(np.float32),
        unroll_steps=16,
    )
    out = kernel(**inputs)
    print("out", out.shape, out.dtype, np.abs(out).max())


# revision 32
# speedup vs baseline: 4.6911x; 1.0251x over previous
"""Trainium2 Bass kernel for a LISTA layer (nn_ListaLayer).

Reference computation (jax, fp32):
    th = relu(Theta) + 1e-7
    xW = (y @ W) / th
    repeat 16: z = xW + (unit_threshold(z) * th @ S) / th
    out = (unit_threshold(z) * th) @ Dx
where unit_threshold(v) = sign(v) * relu(|v| - 1).

Algebraic restructure (exact): track v = z * th:
    v0 = y @ W
    repeat 16:  u = soft_threshold(v, th) = sign(v) * relu(|v| - th)
                v = v0 + u @ S
    out = soft_threshold(v, th) @ Dx

Distribution: data-parallel over batch rows, 8 NeuronCores, 2048 rows each.
W/th/S/Dx replicated; no collectives.

Numerics / performance scheme (v-space carried SCALED by 32 in fp16):
  - A:  v~0 = y16 @ f16(32*W)   (fp16 matmul, fp32 PSUM)
  - B:  16 soft-threshold + u@S steps. First NF8 steps run single-pass
        fp8-e4m3 with perf_mode=DoubleRow (2 dict-tiles contracted per pass);
        the last steps run "split-fp8" (u = uh8+ul8, S = S8H+S8L, 3 logical
        passes folded into 24 DoubleRow matmuls using (S8L[j],S8H[j]) x
        (uh[j],ul[j]) pairing for the cross terms).
        S8H = e4m3(32*S), S8L = e4m3(32*S - S8H); u is consumed UNSCALED
        (ACT applies the 2^-5 descale when emitting fp8), so psum is scaled
        32 and adds directly onto v~0.
        Shrink pipeline per tile: GPSIMD add (psum+v~0), DVE clamp
        (tensor_scalar min/max with per-partition +-32*th), GPSIMD/DVE sub,
        ACT copy->fp8 (scale 2^-5).
  - C:  a16 @ f16(Dx) (fp16), PSUM DMA'd straight to DRAM.
All phases fused per 256-column batch chunk; two chunks interleaved so the
tensor engine never waits on the shrink chain.
"""

import numpy as np
from contextlib import ExitStack

import concourse.bass as bass
import concourse.bacc as bacc
import concourse.tile as tile
import concourse.mybir as mybir
from concourse.bass import ts, ds

P = 128
NCORES = 8
B_FULL, DIN, DD = 16384, 1024, 2048
BSH = B_FULL // NCORES      # 2048 batch rows per core
CH = 256                    # batch columns per chunk
NCH = BSH // CH             # 8 chunks
IT = DD // P                # 16 dict tiles
JP = IT // 2                # 8 DoubleRow pairs
KW = DIN // P               # 8 d_in tiles
CN = 512                    # free dim of phase-C matmuls
SC = 32.0                   # global scale 2^5
NSF8 = 4                    # trailing split-fp8 steps (rest single fp8)
GR = 2                      # dict tiles per shrink group
GRP = 4                     # dict tiles per psum/add group

F8 = mybir.dt.float8e4
F16 = mybir.dt.float16
F32 = mybir.dt.float32
ADD = mybir.AluOpType.add
SUB = mybir.AluOpType.subtract
MIN = mybir.AluOpType.min
MAX = mybir.AluOpType.max
RELU = mybir.ActivationFunctionType.Relu
COPY = mybir.ActivationFunctionType.Copy
DR = mybir.MatmulPerfMode.DoubleRow

_built = {}


def _build(steps: int):
    nc = bacc.Bacc("TRN2", target_bir_lowering=False, debug=False, num_devices=NCORES)

    def inp(name, shape, dt):
        return nc.dram_tensor(name, shape, dt, kind="ExternalInput").ap()

    yT16 = inp("yT16", (DIN, BSH), F16)
    W16_d = inp("W16", (DIN, DD), F16)        # f16(32*W)
    S8_d = inp("S8", (DD, 2, DD), F8)         # [j, (lo,hi), :] interleaved
    Dx8_d = inp("Dx8", (DD, 2, DIN), F8)        # [j, (lo,hi), :] * 32
    pth_d = inp("pth", (DD,), F32)            # +32*th
    nth_d = inp("nth", (DD,), F32)            # -32*th
    nthu_d = inp("nthu", (DD,), F32)          # -th (unscaled)
    out_d = nc.dram_tensor("out", (BSH, DIN), F32, kind="ExternalOutput").ap()

    n_sf8 = min(NSF8, steps)
    n_f8 = steps - n_sf8
    mode = ["f8"] * n_f8 + ["sf8"] * n_sf8    # mode[t] for step t (0-based)

    with tile.TileContext(nc) as tc, ExitStack() as top:
        thp = top.enter_context(tc.tile_pool(name="thp", bufs=1))
        pth_t = thp.tile([P, IT], F32)
        nth_t = thp.tile([P, IT], F32)
        nthu_t = thp.tile([P, IT], F32)
        nc.sync.dma_start(pth_t[:], pth_d.rearrange("(io p) -> p io", p=P))
        nc.sync.dma_start(nth_t[:], nth_d.rearrange("(io p) -> p io", p=P))
        nc.sync.dma_start(nthu_t[:], nthu_d.rearrange("(io p) -> p io", p=P))

        wp = top.enter_context(tc.tile_pool(name="wp", bufs=1))
        W_t = wp.tile([P, KW, DD], F16, name="W_t")
        for k in range(KW):
            nc.sync.dma_start(W_t[:, k, :], W16_d[ts(k, P), :])

        yp = top.enter_context(tc.tile_pool(name="yp", bufs=2))
        y_pre = {}
        for c in (0, 1):
            y_t = yp.tile([P, KW, CH], F16, tag="y", name="y_pre")
            for k in range(KW):
                nc.sync.dma_start(y_t[:, k, :], yT16[ts(k, P), ds(c * CH, CH)])
            y_pre[c] = y_t

        sp = top.enter_context(tc.tile_pool(name="sp", bufs=1))
        S_t = sp.tile([P, IT, 2, DD], F8, name="S_t")   # slot0=lo, slot1=hi
        for j in range(IT):
            nc.sync.dma_start(S_t[:, j, :, :], S8_d[ts(j, P), :, :])

        dxp = top.enter_context(tc.tile_pool(name="dxp", bufs=1))
        Dx_t = dxp.tile([P, IT, 2, DIN], F8, name="Dx_t")   # slot0=lo, slot1=hi
        for io in range(IT):
            nc.sync.dma_start(Dx_t[:, io, :, :], Dx8_d[ts(io, P), :, :])
        v0p = top.enter_context(tc.tile_pool(name="v0p", bufs=2))
        up = top.enter_context(tc.tile_pool(name="up", bufs=4))
        psp = top.enter_context(tc.tile_pool(name="psp", bufs=2, space="PSUM"))
        psap = top.enter_context(tc.tile_pool(name="psap", bufs=2, space="PSUM"))
        pscp = top.enter_context(tc.tile_pool(name="pscp", bufs=2, space="PSUM"))
        vp = top.enter_context(tc.tile_pool(name="vp", bufs=2))
        cp = top.enter_context(tc.tile_pool(name="cp", bufs=2))
        u16p = top.enter_context(tc.tile_pool(name="u16p", bufs=2))
        # NOTE: SBUF is within ~1KB of full; keep pool sizes in sync with budget
        pqp = top.enter_context(tc.tile_pool(name="pqp", bufs=2))
        u32p = top.enter_context(tc.tile_pool(name="u32p", bufs=2))
        stp = top.enter_context(tc.tile_pool(name="stp", bufs=2))

        def shrink_f8(vsrc_quad, q, u8_t):
            """vsrc_quad: [P,4,CH] f16 AP (scaled-32 v). Writes uh into slot 0."""
            c_t = cp.tile([P, GR, CH], F16, tag="c")
            for s in range(GR):
                i = GR * q + s
                nc.gpsimd.tensor_scalar(
                    c_t[:, s, :], vsrc_quad[:, s, :],
                    pth_t[:, i:i + 1], nth_t[:, i:i + 1], MIN, op1=MAX)
            u16_t = u16p.tile([P, GR, CH], F16, tag="u16")
            nc.vector.tensor_tensor(u16_t[:], vsrc_quad, c_t[:], SUB)
            nc.scalar.activation(u8_t[:, GR * q:GR * q + GR, 0, :], u16_t[:],
                                 COPY, scale=1.0 / SC)

        def shrink_sf8(vsrc_quad, q, u8_t):
            """Split-fp8 shrink: uh -> slot0, ul -> slot1 (u8 slots REVERSED vs S8)."""
            for s in range(GR):
                i = GR * q + s
                bias = nthu_t[:, i:i + 1]
                p_t = pqp.tile([P, CH], F32, tag="p")
                q_t = pqp.tile([P, CH], F32, tag="q")
                nc.scalar.activation(p_t[:], vsrc_quad[:, s, :], RELU,
                                     bias=bias, scale=1.0 / SC)
                nc.scalar.activation(q_t[:], vsrc_quad[:, s, :], RELU,
                                     bias=bias, scale=-1.0 / SC)
                u32 = u32p.tile([P, CH], F32, tag="u32")
                nc.vector.tensor_tensor(u32[:], p_t[:], q_t[:], SUB)
                nc.vector.tensor_copy(u8_t[:, i, 0, :], u32[:])
                nc.gpsimd.tensor_tensor(u8_t[:, i, 1, :], u32[:], u8_t[:, i, 0, :], SUB)

        def shrink_af8(vsrc_pair, q, a_t):
            """Final shrink -> SCALED-32 split-fp8 a (ah8 slot0, al8 slot1).
            e4m3(32*a) == 32*e4m3(a) exactly, so phase C just descales by 2^-10."""
            c_t = cp.tile([P, GR, CH], F16, tag="c")
            for s in range(GR):
                i = GR * q + s
                nc.gpsimd.tensor_scalar(
                    c_t[:, s, :], vsrc_pair[:, s, :],
                    pth_t[:, i:i + 1], nth_t[:, i:i + 1], MIN, op1=MAX)
            u16_t = u16p.tile([P, GR, CH], F16, tag="u16")
            nc.vector.tensor_tensor(u16_t[:], vsrc_pair, c_t[:], SUB)
            nc.scalar.activation(a_t[:, GR * q:GR * q + GR, 0, :], u16_t[:],
                                 COPY, scale=1.0)
            nc.gpsimd.tensor_tensor(a_t[:, GR * q:GR * q + GR, 1, :], u16_t[:],
                                    a_t[:, GR * q:GR * q + GR, 0, :], SUB)

        NQ = IT // GR   # shrink groups per step

        def phase_a(c):
            cs = ds(c * CH, CH)
            if c in y_pre:
                y_t = y_pre.pop(c)
            else:
                y_t = yp.tile([P, KW, CH], F16, tag="y")
                for k in range(KW):
                    nc.sync.dma_start(y_t[:, k, :], yT16[ts(k, P), cs])
            v0_t = v0p.tile([P, IT, CH], F16, tag="v0")
            for q in range(NQ):
                ps = psap.tile([P, GR, CH], F32, tag="psA")
                for s in range(GR):
                    i = GR * q + s
                    for k in range(KW):
                        nc.tensor.matmul(ps[:, s, :], W_t[:, k, ts(i, P)],
                                         y_t[:, k, :],
                                         start=(k == 0), stop=(k == KW - 1))
                nc.scalar.activation(v0_t[:, GR * q:GR * q + GR, :], ps[:],
                                     COPY, scale=1.0)
            return v0_t

        def first_shrink(v0_t, u8_t, fmt):
            for q in range(NQ):
                if fmt == "f8":
                    shrink_f8(v0_t[:, GR * q:GR * q + GR, :], q, u8_t)
                else:
                    shrink_sf8(v0_t[:, GR * q:GR * q + GR, :], q, u8_t)

        def step(t, v0_t, u8_t, u8_n, a_t):
            """One B step: psum = u@S (fp8 DR), v~ = ps + v~0, shrink -> u8_n/a_t."""
            m = mode[t]
            last = t == steps - 1
            nxt = None if last else mode[t + 1]
            for qq in range(IT // GRP):
                ps = psp.tile([P, GRP, CH], F32, tag="ps")
                for s in range(GRP):
                    i = GRP * qq + s
                    if m == "f8":
                        n_mm = JP
                        for jp in range(JP):
                            nc.tensor.matmul(
                                ps[:, s, :],
                                S_t[:, 2 * jp:2 * jp + 2, 1, ts(i, P)],
                                u8_t[:, 2 * jp:2 * jp + 2, 0, :],
                                start=(jp == 0), stop=(jp == n_mm - 1),
                                perf_mode=DR)
                    else:
                        n_mm = JP + IT
                        k = 0
                        for jp in range(JP):
                            nc.tensor.matmul(
                                ps[:, s, :],
                                S_t[:, 2 * jp:2 * jp + 2, 1, ts(i, P)],
                                u8_t[:, 2 * jp:2 * jp + 2, 0, :],
                                start=(k == 0), stop=(k == n_mm - 1),
                                perf_mode=DR)
                            k += 1
                        for j in range(IT):
                            nc.tensor.matmul(
                                ps[:, s, :],
                                S_t[:, j, :, ts(i, P)],
                                u8_t[:, j, :, :],
                                start=False, stop=(k == n_mm - 1),
                                perf_mode=DR)
                            k += 1
                v_t = vp.tile([P, GRP, CH], F16, tag="v")
                nc.vector.tensor_tensor(v_t[:], ps[:],
                                        v0_t[:, GRP * qq:GRP * qq + GRP, :], ADD)
                for h in range(GRP // GR):
                    q = (GRP // GR) * qq + h
                    v_pair = v_t[:, GR * h:GR * h + GR, :]
                    if last:
                        shrink_af8(v_pair, q, a_t)
                    elif nxt == "f8":
                        shrink_f8(v_pair, q, u8_n)
                    else:
                        shrink_sf8(v_pair, q, u8_n)

        def phase_c(c, a_t):
            # out = (ah @ (DxH + DxL) + al @ DxH) / 32, all DoubleRow fp8
            cs_base = c * CH
            for bt in range(CH // P):
                for dn in range(DIN // CN):
                    ps = pscp.tile([P, CN], F32, tag="psC")
                    n_mm = JP + IT
                    k = 0
                    for jp in range(JP):
                        nc.tensor.matmul(
                            ps[:], a_t[:, 2 * jp:2 * jp + 2, 0, ts(bt, P)],
                            Dx_t[:, 2 * jp:2 * jp + 2, 1, ts(dn, CN)],
                            start=(k == 0), stop=(k == n_mm - 1), perf_mode=DR)
                        k += 1
                    for j in range(IT):
                        nc.tensor.matmul(
                            ps[:], a_t[:, j, :, ts(bt, P)],
                            Dx_t[:, j, :, ts(dn, CN)],
                            start=False, stop=(k == n_mm - 1), perf_mode=DR)
                        k += 1
                    st = stp.tile([P, CN], F32, tag="st")
                    nc.scalar.activation(st[:], ps[:], COPY, scale=1.0 / (SC * SC))
                    nc.sync.dma_start(out_d[ds(cs_base + bt * P, P), ts(dn, CN)],
                                      st[:])

        # ---- main schedule: chunk pairs; previous pair's C is emitted after
        # the next pair's A so its matmuls fill the shrink-chain latency ----
        pending_c = []
        for cp0 in range(0, NCH, 2):
            pair = (cp0, cp0 + 1)
            v0s, u8s, ats = {}, {}, {}
            for c in pair:
                v0s[c] = phase_a(c)
                u8s[c] = up.tile([P, IT, 2, CH], F8, tag="u8", name="u8_t")
                first_shrink(v0s[c], u8s[c], mode[0])
                ats[c] = None
            for nxt_c in (cp0 + 2, cp0 + 3):
                if nxt_c < NCH and nxt_c not in y_pre:
                    y_t = yp.tile([P, KW, CH], F16, tag="y", name="y_nxt")
                    for k in range(KW):
                        nc.sync.dma_start(y_t[:, k, :],
                                          yT16[ts(k, P), ds(nxt_c * CH, CH)])
                    y_pre[nxt_c] = y_t
            for c, a_t in pending_c:
                phase_c(c, a_t)
            pending_c = []
            for t in range(steps):
                for c in pair:
                    last = t == steps - 1
                    if last:
                        u8_n = None
                        ats[c] = up.tile([P, IT, 2, CH], F8, tag="u8", name="a_t")
                    else:
                        u8_n = up.tile([P, IT, 2, CH], F8, tag="u8", name="u8_n")
                    step(t, v0s[c], u8s[c], u8_n, ats[c])
                    u8s[c] = u8_n
            pending_c = [(c, ats[c]) for c in pair]
        for c, a_t in pending_c:
            phase_c(c, a_t)

    nc.compile()
    return nc


def _prep_in_maps(y, W, Theta, S, Dx):
    import ml_dtypes
    E4 = ml_dtypes.float8_e4m3  # TRN flavor (max normal 240)

    y = np.asarray(y, dtype=np.float32)
    W = np.asarray(W, dtype=np.float32)
    Theta = np.asarray(Theta, dtype=np.float32)
    S = np.asarray(S, dtype=np.float32)
    Dx = np.asarray(Dx, dtype=np.float32)
    assert y.shape == (B_FULL, DIN) and W.shape == (DIN, DD)
    assert S.shape == (DD, DD) and Dx.shape == (DD, DIN)

    th = np.maximum(Theta, 0.0) + np.float32(1e-7)
    W16 = (SC * W).astype(np.float16)
    Ss = np.float32(SC) * S
    S8H = np.clip(Ss, -240, 240).astype(E4)
    S8L = np.clip(Ss - S8H.astype(np.float32), -240, 240).astype(E4)
    S8 = np.stack([S8L, S8H], axis=1)         # [DD, 2, DD], slot0=lo slot1=hi
    Dxs = np.float32(SC) * Dx
    Dx8H = np.clip(Dxs, -240, 240).astype(E4)
    Dx8L = np.clip(Dxs - Dx8H.astype(np.float32), -240, 240).astype(E4)
    Dx8 = np.ascontiguousarray(np.stack([Dx8L, Dx8H], axis=1))
    yT16 = np.ascontiguousarray(y.T).astype(np.float16)

    shared = dict(
        W16=W16, S8=np.ascontiguousarray(S8), Dx8=Dx8,
        pth=(SC * th).astype(np.float32),
        nth=(-SC * th).astype(np.float32),
        nthu=(-th).astype(np.float32),
    )
    in_maps = []
    for c in range(NCORES):
        sl = slice(c * BSH, (c + 1) * BSH)
        in_maps.append(dict(shared, yT16=np.ascontiguousarray(yT16[:, sl])))
    return in_maps


_sharded_cache = {}


def _get_sharded(steps: int):
    """Build (once) the jitted shard_map executable for the compiled NEFF."""
    if steps in _sharded_cache:
        return _sharded_cache[steps]
    import jax
    from jax.experimental.shard_map import shard_map
    from jax.sharding import Mesh, PartitionSpec
    from concourse import bass2jax

    if steps not in _built:
        _built[steps] = _build(steps)
    nc = _built[steps]
    bass2jax.install_neuronx_cc_hook()
    assert nc.dbg_addr is None
    partition_name = nc.partition_id_tensor.name if nc.partition_id_tensor else None

    in_names, out_names, out_avals, zero_shapes = [], [], [], []
    for alloc in nc.m.functions[0].allocations:
        if not isinstance(alloc, mybir.MemoryLocationSet):
            continue
        name = alloc.memorylocations[0].name
        if alloc.kind == "ExternalInput":
            if name != partition_name:
                in_names.append(name)
        elif alloc.kind == "ExternalOutput":
            out_names.append(name)
            shape = tuple(alloc.tensor_shape)
            dtype = mybir.dt.np(alloc.dtype)
            out_avals.append(jax.core.ShapedArray(shape, dtype))
            zero_shapes.append((shape, dtype))
    n_params = len(in_names)
    n_outs = len(out_names)
    all_in_names = in_names + out_names
    if partition_name is not None:
        all_in_names.append(partition_name)

    def _body(*args):
        operands = list(args)
        if partition_name is not None:
            operands.append(bass2jax.partition_id_tensor())
        outs = bass2jax._bass_exec_p.bind(
            *operands,
            out_avals=tuple(out_avals),
            in_names=tuple(all_in_names),
            out_names=tuple(out_names),
            lowering_input_output_aliases=(),
            sim_require_finite=True,
            sim_require_nnan=True,
            nc=nc,
        )
        return tuple(outs)

    devices = jax.devices()[:NCORES]
    mesh = Mesh(np.asarray(devices), ("core",))
    donate = tuple(range(n_params, n_params + n_outs))
    sharded = jax.jit(
        shard_map(_body, mesh=mesh,
                  in_specs=(PartitionSpec("core"),) * (n_params + n_outs),
                  out_specs=(PartitionSpec("core"),) * n_outs,
                  check_rep=False),
        donate_argnums=donate, keep_unused=True)
    entry = dict(sharded=sharded, in_names=in_names, out_names=out_names,
                 zero_shapes=zero_shapes, mesh=mesh, n_params=n_params)
    _sharded_cache[steps] = entry
    return entry


def _concat_inputs(entry, in_maps):
    return [np.concatenate([np.asarray(in_maps[c][n]) for c in range(NCORES)], axis=0)
            for n in entry["in_names"]]


def _run(entry, concat_in):
    zeros = [np.zeros((NCORES * s[0], *s[1:]), d) for s, d in entry["zero_shapes"]]
    out_arrs = entry["sharded"](*concat_in, *zeros)
    return out_arrs


def kernel(y, W, Theta, S, Dx, unroll_steps):
    steps = int(unroll_steps)
    entry = _get_sharded(steps)
    in_maps = _prep_in_maps(y, W, Theta, S, Dx)
    out_arrs = _run(entry, _concat_inputs(entry, in_maps))
    idx = entry["out_names"].index("out")
    return np.ascontiguousarray(np.asarray(out_arrs[idx]))  # [NCORES*BSH, DIN]


def time_kernel(np_inputs, iters=6):
    """Steady-state wall time per NEFF execution (ns), device-resident inputs."""
    import jax
    from jax.sharding import NamedSharding, PartitionSpec
    steps = int(np_inputs["unroll_steps"])
    entry = _get_sharded(steps)
    in_maps = _prep_in_maps(np_inputs["y"], np_inputs["W"], np_inputs["Theta"],
                            np_inputs["S"], np_inputs["Dx"])
    concat_in = _concat_inputs(entry, in_maps)
    sh = NamedSharding(entry["mesh"], PartitionSpec("core"))
    dev_in = [jax.device_put(a, sh) for a in concat_in]
    import time as _time
    times = []
    for it in range(iters):
        zeros = [jax.device_put(np.zeros((NCORES * s[0], *s[1:]), d), sh)
                 for s, d in entry["zero_shapes"]]
        for z in zeros:
            z.block_until_ready()
        t0 = _time.perf_counter()
        outs = entry["sharded"](*dev_in, *zeros)
        for o in outs:
            o.block_until_ready()
        times.append(_time.perf_counter() - t0)
    best = min(times[1:]) if len(times) > 1 else times[0]
    print("  per-iter times (ms):", [f"{t*1e3:.1f}" for t in times])
    return best * 1e9


if __name__ == "__main__":
    rng = np.random.default_rng(0)
    inputs = dict(
        y=rng.standard_normal((B_FULL, DIN), dtype=np.float32),
        W=(rng.standard_normal((DIN, DD)) * 0.02).astype(np.float32),
        Theta=rng.random(DD, dtype=np.float32),
        S=(rng.standard_normal((DD, DD)) * 0.02).astype(np.float32),
        Dx=(rng.standard_normal((DD, DIN)) * 0.02).astype(np.float32),
        unroll_steps=16,
    )
    out = kernel(**inputs)
    print("out", out.shape, out.dtype, np.abs(out).max())


# revision 36
# speedup vs baseline: 4.6932x; 1.0005x over previous
"""Trainium2 Bass kernel for a LISTA layer (nn_ListaLayer).

Reference computation (jax, fp32):
    th = relu(Theta) + 1e-7
    xW = (y @ W) / th
    repeat 16: z = xW + (unit_threshold(z) * th @ S) / th
    out = (unit_threshold(z) * th) @ Dx
where unit_threshold(v) = sign(v) * relu(|v| - 1).

Algebraic restructure (exact): track v = z * th:
    v0 = y @ W
    repeat 16:  u = soft_threshold(v, th) = sign(v) * relu(|v| - th)
                v = v0 + u @ S
    out = soft_threshold(v, th) @ Dx

Distribution: data-parallel over batch rows, 8 NeuronCores, 2048 rows each.
W/th/S/Dx replicated; no collectives.

Numerics / performance scheme (v-space carried SCALED by 32 in fp16):
  - A:  v~0 = y16 @ f16(32*W)   (fp16 matmul, fp32 PSUM)
  - B:  16 soft-threshold + u@S steps, all matmuls fp8-e4m3 DoubleRow
        (2 dict-tiles contracted per pass). First 12 steps single-pass
        (u8 @ S8H); last NSF8=4 steps "split-fp8": uh8 @ (S8H+S8L) +
        ul8 @ S8H, the cross terms folded into per-j DoubleRow matmuls by
        pairing weight slots (S8L[j],S8H[j]) against moving slots
        (uh[j],ul[j]) - DoubleRow multiplies same-index slots, so the u8
        tile stores (uh, ul) while the S tile stores (lo, hi).
        S8H = e4m3(32*S), S8L = e4m3(32*S - S8H); u is consumed UNSCALED
        (ACT applies the 2^-5 descale when emitting fp8), so psum comes out
        scaled 32 and adds directly onto v~0.
        f8-step shrink per pair of dict tiles: DVE add (psum+v~0 -> f16),
        GPSIMD fused clamp (tensor_scalar min/max, per-partition +-32*th),
        DVE sub (f16 2x), ACT copy->fp8 (scale 2^-5).
  - C:  out = (ah8 @ (DxH+DxL) + al8 @ DxH) * 2^-10, DoubleRow fp8 with the
        final shrink emitting a scaled-32 hi/lo split (exact: e4m3(32a) ==
        32*e4m3(a)); the 2^-10 descale rides the PSUM->SBUF ACT copy.
All phases fused per 256-column batch chunk; two chunks interleaved so the
tensor engine never waits on a shrink chain, and each pair's C matmuls are
deferred until after the next pair's A phase to fill the final-shrink
latency. Phase A and C have dedicated PSUM pools so the B-step psum ring
never blocks them.
"""

import numpy as np
from contextlib import ExitStack

import concourse.bass as bass
import concourse.bacc as bacc
import concourse.tile as tile
import concourse.mybir as mybir
from concourse.bass import ts, ds

P = 128
NCORES = 8
B_FULL, DIN, DD = 16384, 1024, 2048
BSH = B_FULL // NCORES      # 2048 batch rows per core
CH = 256                    # batch columns per chunk
NCH = BSH // CH             # 8 chunks
IT = DD // P                # 16 dict tiles
JP = IT // 2                # 8 DoubleRow pairs
KW = DIN // P               # 8 d_in tiles
CN = 512                    # free dim of phase-C matmuls
SC = 32.0                   # global scale 2^5
NSF8 = 4                    # trailing split-fp8 steps (rest single fp8)
GR = 2                      # dict tiles per shrink group
GRP = 4                     # dict tiles per psum/add group

F8 = mybir.dt.float8e4
F16 = mybir.dt.float16
F32 = mybir.dt.float32
ADD = mybir.AluOpType.add
SUB = mybir.AluOpType.subtract
MIN = mybir.AluOpType.min
MAX = mybir.AluOpType.max
RELU = mybir.ActivationFunctionType.Relu
COPY = mybir.ActivationFunctionType.Copy
DR = mybir.MatmulPerfMode.DoubleRow

_built = {}


def _build(steps: int):
    nc = bacc.Bacc("TRN2", target_bir_lowering=False, debug=False, num_devices=NCORES)

    def inp(name, shape, dt):
        return nc.dram_tensor(name, shape, dt, kind="ExternalInput").ap()

    yT16 = inp("yT16", (DIN, BSH), F16)
    W16_d = inp("W16", (DIN, DD), F16)        # f16(32*W)
    S8_d = inp("S8", (DD, 2, DD), F8)         # [j, (lo,hi), :] interleaved
    Dx8_d = inp("Dx8", (DD, 2, DIN), F8)        # [j, (lo,hi), :] * 32
    pth_d = inp("pth", (DD,), F32)            # +32*th
    nth_d = inp("nth", (DD,), F32)            # -32*th
    nthu_d = inp("nthu", (DD,), F32)          # -th (unscaled)
    out_d = nc.dram_tensor("out", (BSH, DIN), F32, kind="ExternalOutput").ap()

    n_sf8 = min(NSF8, steps)
    n_f8 = steps - n_sf8
    mode = ["f8"] * n_f8 + ["sf8"] * n_sf8    # mode[t] for step t (0-based)

    with tile.TileContext(nc) as tc, ExitStack() as top:
        thp = top.enter_context(tc.tile_pool(name="thp", bufs=1))
        pth_t = thp.tile([P, IT], F32)
        nth_t = thp.tile([P, IT], F32)
        nthu_t = thp.tile([P, IT], F32)

        wp = top.enter_context(tc.tile_pool(name="wp", bufs=1))
        W_t = wp.tile([P, KW, DD], F16, name="W_t")
        for k in range(KW):
            nc.sync.dma_start(W_t[:, k, :], W16_d[ts(k, P), :])

        yp = top.enter_context(tc.tile_pool(name="yp", bufs=2))
        y_pre = {}
        for c in (0, 1):
            y_t = yp.tile([P, KW, CH], F16, tag="y", name="y_pre")
            for k in range(KW):
                nc.sync.dma_start(y_t[:, k, :], yT16[ts(k, P), ds(c * CH, CH)])
            y_pre[c] = y_t

        nc.sync.dma_start(pth_t[:], pth_d.rearrange("(io p) -> p io", p=P))
        nc.sync.dma_start(nth_t[:], nth_d.rearrange("(io p) -> p io", p=P))
        nc.sync.dma_start(nthu_t[:], nthu_d.rearrange("(io p) -> p io", p=P))

        sp = top.enter_context(tc.tile_pool(name="sp", bufs=1))
        S_t = sp.tile([P, IT, 2, DD], F8, name="S_t")   # slot0=lo, slot1=hi
        for j in range(IT):
            nc.sync.dma_start(S_t[:, j, :, :], S8_d[ts(j, P), :, :])

        dxp = top.enter_context(tc.tile_pool(name="dxp", bufs=1))
        Dx_t = dxp.tile([P, IT, 2, DIN], F8, name="Dx_t")   # slot0=lo, slot1=hi
        for io in range(IT):
            nc.sync.dma_start(Dx_t[:, io, :, :], Dx8_d[ts(io, P), :, :])
        v0p = top.enter_context(tc.tile_pool(name="v0p", bufs=2))
        up = top.enter_context(tc.tile_pool(name="up", bufs=4))
        psp = top.enter_context(tc.tile_pool(name="psp", bufs=2, space="PSUM"))
        psap = top.enter_context(tc.tile_pool(name="psap", bufs=2, space="PSUM"))
        pscp = top.enter_context(tc.tile_pool(name="pscp", bufs=2, space="PSUM"))
        vp = top.enter_context(tc.tile_pool(name="vp", bufs=2))
        cp = top.enter_context(tc.tile_pool(name="cp", bufs=2))
        u16p = top.enter_context(tc.tile_pool(name="u16p", bufs=2))
        # NOTE: SBUF is within ~1KB of full; keep pool sizes in sync with budget
        pqp = top.enter_context(tc.tile_pool(name="pqp", bufs=2))
        u32p = top.enter_context(tc.tile_pool(name="u32p", bufs=2))
        stp = top.enter_context(tc.tile_pool(name="stp", bufs=2))

        def shrink_f8(vsrc_quad, q, u8_t):
            """vsrc_quad: [P,4,CH] f16 AP (scaled-32 v). Writes uh into slot 0."""
            c_t = cp.tile([P, GR, CH], F16, tag="c")
            for s in range(GR):
                i = GR * q + s
                nc.gpsimd.tensor_scalar(
                    c_t[:, s, :], vsrc_quad[:, s, :],
                    pth_t[:, i:i + 1], nth_t[:, i:i + 1], MIN, op1=MAX)
            u16_t = u16p.tile([P, GR, CH], F16, tag="u16")
            nc.vector.tensor_tensor(u16_t[:], vsrc_quad, c_t[:], SUB)
            nc.scalar.activation(u8_t[:, GR * q:GR * q + GR, 0, :], u16_t[:],
                                 COPY, scale=1.0 / SC)

        def shrink_sf8(vsrc_quad, q, u8_t):
            """Split-fp8 shrink: uh -> slot0, ul -> slot1 (u8 slots REVERSED vs S8)."""
            for s in range(GR):
                i = GR * q + s
                bias = nthu_t[:, i:i + 1]
                p_t = pqp.tile([P, CH], F32, tag="p")
                q_t = pqp.tile([P, CH], F32, tag="q")
                nc.scalar.activation(p_t[:], vsrc_quad[:, s, :], RELU,
                                     bias=bias, scale=1.0 / SC)
                nc.scalar.activation(q_t[:], vsrc_quad[:, s, :], RELU,
                                     bias=bias, scale=-1.0 / SC)
                u32 = u32p.tile([P, CH], F32, tag="u32")
                nc.vector.tensor_tensor(u32[:], p_t[:], q_t[:], SUB)
                nc.vector.tensor_copy(u8_t[:, i, 0, :], u32[:])
                nc.gpsimd.tensor_tensor(u8_t[:, i, 1, :], u32[:], u8_t[:, i, 0, :], SUB)

        def shrink_af8(vsrc_pair, q, a_t):
            """Final shrink -> SCALED-32 split-fp8 a (ah8 slot0, al8 slot1).
            e4m3(32*a) == 32*e4m3(a) exactly, so phase C just descales by 2^-10."""
            c_t = cp.tile([P, GR, CH], F16, tag="c")
            for s in range(GR):
                i = GR * q + s
                nc.gpsimd.tensor_scalar(
                    c_t[:, s, :], vsrc_pair[:, s, :],
                    pth_t[:, i:i + 1], nth_t[:, i:i + 1], MIN, op1=MAX)
            u16_t = u16p.tile([P, GR, CH], F16, tag="u16")
            nc.vector.tensor_tensor(u16_t[:], vsrc_pair, c_t[:], SUB)
            nc.scalar.activation(a_t[:, GR * q:GR * q + GR, 0, :], u16_t[:],
                                 COPY, scale=1.0)
            nc.gpsimd.tensor_tensor(a_t[:, GR * q:GR * q + GR, 1, :], u16_t[:],
                                    a_t[:, GR * q:GR * q + GR, 0, :], SUB)

        NQ = IT // GR   # shrink groups per step

        def phase_a(c):
            cs = ds(c * CH, CH)
            if c in y_pre:
                y_t = y_pre.pop(c)
            else:
                y_t = yp.tile([P, KW, CH], F16, tag="y")
                for k in range(KW):
                    nc.sync.dma_start(y_t[:, k, :], yT16[ts(k, P), cs])
            v0_t = v0p.tile([P, IT, CH], F16, tag="v0")
            for q in range(NQ):
                ps = psap.tile([P, GR, CH], F32, tag="psA")
                for s in range(GR):
                    i = GR * q + s
                    for k in range(KW):
                        nc.tensor.matmul(ps[:, s, :], W_t[:, k, ts(i, P)],
                                         y_t[:, k, :],
                                         start=(k == 0), stop=(k == KW - 1))
                nc.scalar.activation(v0_t[:, GR * q:GR * q + GR, :], ps[:],
                                     COPY, scale=1.0)
            return v0_t

        def first_shrink(v0_t, u8_t, fmt):
            for q in range(NQ):
                if fmt == "f8":
                    shrink_f8(v0_t[:, GR * q:GR * q + GR, :], q, u8_t)
                else:
                    shrink_sf8(v0_t[:, GR * q:GR * q + GR, :], q, u8_t)

        def step(t, v0_t, u8_t, u8_n, a_t):
            """One B step: psum = u@S (fp8 DR), v~ = ps + v~0, shrink -> u8_n/a_t."""
            m = mode[t]
            last = t == steps - 1
            nxt = None if last else mode[t + 1]
            for qq in range(IT // GRP):
                ps = psp.tile([P, GRP, CH], F32, tag="ps")
                for s in range(GRP):
                    i = GRP * qq + s
                    if m == "f8":
                        n_mm = JP
                        for jp in range(JP):
                            nc.tensor.matmul(
                                ps[:, s, :],
                                S_t[:, 2 * jp:2 * jp + 2, 1, ts(i, P)],
                                u8_t[:, 2 * jp:2 * jp + 2, 0, :],
                                start=(jp == 0), stop=(jp == n_mm - 1),
                                perf_mode=DR)
                    else:
                        n_mm = JP + IT
                        k = 0
                        for jp in range(JP):
                            nc.tensor.matmul(
                                ps[:, s, :],
                                S_t[:, 2 * jp:2 * jp + 2, 1, ts(i, P)],
                                u8_t[:, 2 * jp:2 * jp + 2, 0, :],
                                start=(k == 0), stop=(k == n_mm - 1),
                                perf_mode=DR)
                            k += 1
                        for j in range(IT):
                            nc.tensor.matmul(
                                ps[:, s, :],
                                S_t[:, j, :, ts(i, P)],
                                u8_t[:, j, :, :],
                                start=False, stop=(k == n_mm - 1),
                                perf_mode=DR)
                            k += 1
                v_t = vp.tile([P, GRP, CH], F16, tag="v")
                nc.vector.tensor_tensor(v_t[:], ps[:],
                                        v0_t[:, GRP * qq:GRP * qq + GRP, :], ADD)
                for h in range(GRP // GR):
                    q = (GRP // GR) * qq + h
                    v_pair = v_t[:, GR * h:GR * h + GR, :]
                    if last:
                        shrink_af8(v_pair, q, a_t)
                    elif nxt == "f8":
                        shrink_f8(v_pair, q, u8_n)
                    else:
                        shrink_sf8(v_pair, q, u8_n)

        def phase_c(c, a_t):
            # out = (ah @ (DxH + DxL) + al @ DxH) / 32, all DoubleRow fp8
            cs_base = c * CH
            for bt in range(CH // P):
                for dn in range(DIN // CN):
                    ps = pscp.tile([P, CN], F32, tag="psC")
                    n_mm = JP + IT
                    k = 0
                    for jp in range(JP):
                        nc.tensor.matmul(
                            ps[:], a_t[:, 2 * jp:2 * jp + 2, 0, ts(bt, P)],
                            Dx_t[:, 2 * jp:2 * jp + 2, 1, ts(dn, CN)],
                            start=(k == 0), stop=(k == n_mm - 1), perf_mode=DR)
                        k += 1
                    for j in range(IT):
                        nc.tensor.matmul(
                            ps[:], a_t[:, j, :, ts(bt, P)],
                            Dx_t[:, j, :, ts(dn, CN)],
                            start=False, stop=(k == n_mm - 1), perf_mode=DR)
                        k += 1
                    st = stp.tile([P, CN], F32, tag="st")
                    nc.scalar.activation(st[:], ps[:], COPY, scale=1.0 / (SC * SC))
                    nc.sync.dma_start(out_d[ds(cs_base + bt * P, P), ts(dn, CN)],
                                      st[:])

        # ---- main schedule: chunk pairs; previous pair's C is emitted after
        # the next pair's A so its matmuls fill the shrink-chain latency ----
        pending_c = []
        for cp0 in range(0, NCH, 2):
            pair = (cp0, cp0 + 1)
            v0s, u8s, ats = {}, {}, {}
            for c in pair:
                v0s[c] = phase_a(c)
                u8s[c] = up.tile([P, IT, 2, CH], F8, tag="u8", name="u8_t")
                first_shrink(v0s[c], u8s[c], mode[0])
                ats[c] = None
            for nxt_c in (cp0 + 2, cp0 + 3):
                if nxt_c < NCH and nxt_c not in y_pre:
                    y_t = yp.tile([P, KW, CH], F16, tag="y", name="y_nxt")
                    for k in range(KW):
                        nc.sync.dma_start(y_t[:, k, :],
                                          yT16[ts(k, P), ds(nxt_c * CH, CH)])
                    y_pre[nxt_c] = y_t
            for c, a_t in pending_c:
                phase_c(c, a_t)
            pending_c = []
            for t in range(steps):
                for c in pair:
                    last = t == steps - 1
                    if last:
                        u8_n = None
                        ats[c] = up.tile([P, IT, 2, CH], F8, tag="u8", name="a_t")
                    else:
                        u8_n = up.tile([P, IT, 2, CH], F8, tag="u8", name="u8_n")
                    step(t, v0s[c], u8s[c], u8_n, ats[c])
                    u8s[c] = u8_n
            pending_c = [(c, ats[c]) for c in pair]
        for c, a_t in pending_c:
            phase_c(c, a_t)

    nc.compile()
    return nc


def _prep_in_maps(y, W, Theta, S, Dx):
    import ml_dtypes
    E4 = ml_dtypes.float8_e4m3  # TRN flavor (max normal 240)

    y = np.asarray(y, dtype=np.float32)
    W = np.asarray(W, dtype=np.float32)
    Theta = np.asarray(Theta, dtype=np.float32)
    S = np.asarray(S, dtype=np.float32)
    Dx = np.asarray(Dx, dtype=np.float32)
    assert y.shape == (B_FULL, DIN) and W.shape == (DIN, DD)
    assert S.shape == (DD, DD) and Dx.shape == (DD, DIN)

    th = np.maximum(Theta, 0.0) + np.float32(1e-7)
    W16 = (SC * W).astype(np.float16)
    Ss = np.float32(SC) * S
    S8H = np.clip(Ss, -240, 240).astype(E4)
    S8L = np.clip(Ss - S8H.astype(np.float32), -240, 240).astype(E4)
    S8 = np.stack([S8L, S8H], axis=1)         # [DD, 2, DD], slot0=lo slot1=hi
    Dxs = np.float32(SC) * Dx
    Dx8H = np.clip(Dxs, -240, 240).astype(E4)
    Dx8L = np.clip(Dxs - Dx8H.astype(np.float32), -240, 240).astype(E4)
    Dx8 = np.ascontiguousarray(np.stack([Dx8L, Dx8H], axis=1))
    yT16 = np.ascontiguousarray(y.T).astype(np.float16)

    shared = dict(
        W16=W16, S8=np.ascontiguousarray(S8), Dx8=Dx8,
        pth=(SC * th).astype(np.float32),
        nth=(-SC * th).astype(np.float32),
        nthu=(-th).astype(np.float32),
    )
    in_maps = []
    for c in range(NCORES):
        sl = slice(c * BSH, (c + 1) * BSH)
        in_maps.append(dict(shared, yT16=np.ascontiguousarray(yT16[:, sl])))
    return in_maps


_sharded_cache = {}


def _get_sharded(steps: int):
    """Build (once) the jitted shard_map executable for the compiled NEFF."""
    if steps in _sharded_cache:
        return _sharded_cache[steps]
    import jax
    from jax.experimental.shard_map import shard_map
    from jax.sharding import Mesh, PartitionSpec
    from concourse import bass2jax

    if steps not in _built:
        _built[steps] = _build(steps)
    nc = _built[steps]
    bass2jax.install_neuronx_cc_hook()
    assert nc.dbg_addr is None
    partition_name = nc.partition_id_tensor.name if nc.partition_id_tensor else None

    in_names, out_names, out_avals, zero_shapes = [], [], [], []
    for alloc in nc.m.functions[0].allocations:
        if not isinstance(alloc, mybir.MemoryLocationSet):
            continue
        name = alloc.memorylocations[0].name
        if alloc.kind == "ExternalInput":
            if name != partition_name:
                in_names.append(name)
        elif alloc.kind == "ExternalOutput":
            out_names.append(name)
            shape = tuple(alloc.tensor_shape)
            dtype = mybir.dt.np(alloc.dtype)
            out_avals.append(jax.core.ShapedArray(shape, dtype))
            zero_shapes.append((shape, dtype))
    n_params = len(in_names)
    n_outs = len(out_names)
    all_in_names = in_names + out_names
    if partition_name is not None:
        all_in_names.append(partition_name)

    def _body(*args):
        operands = list(args)
        if partition_name is not None:
            operands.append(bass2jax.partition_id_tensor())
        outs = bass2jax._bass_exec_p.bind(
            *operands,
            out_avals=tuple(out_avals),
            in_names=tuple(all_in_names),
            out_names=tuple(out_names),
            lowering_input_output_aliases=(),
            sim_require_finite=True,
            sim_require_nnan=True,
            nc=nc,
        )
        return tuple(outs)

    devices = jax.devices()[:NCORES]
    mesh = Mesh(np.asarray(devices), ("core",))
    donate = tuple(range(n_params, n_params + n_outs))
    sharded = jax.jit(
        shard_map(_body, mesh=mesh,
                  in_specs=(PartitionSpec("core"),) * (n_params + n_outs),
                  out_specs=(PartitionSpec("core"),) * n_outs,
                  check_rep=False),
        donate_argnums=donate, keep_unused=True)
    entry = dict(sharded=sharded, in_names=in_names, out_names=out_names,
                 zero_shapes=zero_shapes, mesh=mesh, n_params=n_params)
    _sharded_cache[steps] = entry
    return entry


def _concat_inputs(entry, in_maps):
    return [np.concatenate([np.asarray(in_maps[c][n]) for c in range(NCORES)], axis=0)
            for n in entry["in_names"]]


def _run(entry, concat_in):
    zeros = [np.zeros((NCORES * s[0], *s[1:]), d) for s, d in entry["zero_shapes"]]
    out_arrs = entry["sharded"](*concat_in, *zeros)
    return out_arrs


def kernel(y, W, Theta, S, Dx, unroll_steps):
    steps = int(unroll_steps)
    entry = _get_sharded(steps)
    in_maps = _prep_in_maps(y, W, Theta, S, Dx)
    out_arrs = _run(entry, _concat_inputs(entry, in_maps))
    idx = entry["out_names"].index("out")
    return np.ascontiguousarray(np.asarray(out_arrs[idx]))  # [NCORES*BSH, DIN]


def time_kernel(np_inputs, iters=6):
    """Steady-state wall time per NEFF execution (ns), device-resident inputs."""
    import jax
    from jax.sharding import NamedSharding, PartitionSpec
    steps = int(np_inputs["unroll_steps"])
    entry = _get_sharded(steps)
    in_maps = _prep_in_maps(np_inputs["y"], np_inputs["W"], np_inputs["Theta"],
                            np_inputs["S"], np_inputs["Dx"])
    concat_in = _concat_inputs(entry, in_maps)
    sh = NamedSharding(entry["mesh"], PartitionSpec("core"))
    dev_in = [jax.device_put(a, sh) for a in concat_in]
    import time as _time
    times = []
    for it in range(iters):
        zeros = [jax.device_put(np.zeros((NCORES * s[0], *s[1:]), d), sh)
                 for s, d in entry["zero_shapes"]]
        for z in zeros:
            z.block_until_ready()
        t0 = _time.perf_counter()
        outs = entry["sharded"](*dev_in, *zeros)
        for o in outs:
            o.block_until_ready()
        times.append(_time.perf_counter() - t0)
    best = min(times[1:]) if len(times) > 1 else times[0]
    print("  per-iter times (ms):", [f"{t*1e3:.1f}" for t in times])
    return best * 1e9


if __name__ == "__main__":
    rng = np.random.default_rng(0)
    inputs = dict(
        y=rng.standard_normal((B_FULL, DIN), dtype=np.float32),
        W=(rng.standard_normal((DIN, DD)) * 0.02).astype(np.float32),
        Theta=rng.random(DD, dtype=np.float32),
        S=(rng.standard_normal((DD, DD)) * 0.02).astype(np.float32),
        Dx=(rng.standard_normal((DD, DIN)) * 0.02).astype(np.float32),
        unroll_steps=16,
    )
    out = kernel(**inputs)
    print("out", out.shape, out.dtype, np.abs(out).max())


# revision 41
# speedup vs baseline: 4.6934x; 1.0001x over previous
"""Trainium2 Bass kernel for a LISTA layer (nn_ListaLayer).

Reference computation (jax, fp32):
    th = relu(Theta) + 1e-7
    xW = (y @ W) / th
    repeat 16: z = xW + (unit_threshold(z) * th @ S) / th
    out = (unit_threshold(z) * th) @ Dx
where unit_threshold(v) = sign(v) * relu(|v| - 1).

Algebraic restructure (exact): track v = z * th:
    v0 = y @ W
    repeat 16:  u = soft_threshold(v, th) = sign(v) * relu(|v| - th)
                v = v0 + u @ S
    out = soft_threshold(v, th) @ Dx

Distribution: data-parallel over batch rows, 8 NeuronCores, 2048 rows each.
W/th/S/Dx replicated; no collectives.

Numerics / performance scheme (v-space carried SCALED by 32 in fp16):
  - A:  v~0 = y16 @ f16(32*W)   (fp16 matmul, fp32 PSUM)
  - B:  16 soft-threshold + u@S steps, all matmuls fp8-e4m3 DoubleRow
        (2 dict-tiles contracted per pass). First 12 steps single-pass
        (u8 @ S8H); last NSF8=4 steps "split-fp8": uh8 @ (S8H+S8L) +
        ul8 @ S8H, the cross terms folded into per-j DoubleRow matmuls by
        pairing weight slots (S8L[j],S8H[j]) against moving slots
        (uh[j],ul[j]) - DoubleRow multiplies same-index slots, so the u8
        tile stores (uh, ul) while the S tile stores (lo, hi).
        S8H = e4m3(32*S), S8L = e4m3(32*S - S8H); u is consumed UNSCALED
        (ACT applies the 2^-5 descale when emitting fp8), so psum comes out
        scaled 32 and adds directly onto v~0.
        f8-step shrink per pair of dict tiles: DVE add (psum+v~0 -> f16),
        GPSIMD fused clamp (tensor_scalar min/max, per-partition +-32*th),
        DVE sub (f16 2x), ACT copy->fp8 (scale 2^-5).
  - C:  out = (ah8 @ (DxH+DxL) + al8 @ DxH) * 2^-10, DoubleRow fp8 with the
        final shrink emitting a scaled-32 hi/lo split (exact: e4m3(32a) ==
        32*e4m3(a)); the 2^-10 descale rides the PSUM->SBUF ACT copy.
All phases fused per 256-column batch chunk; two chunks interleaved so the
tensor engine never waits on a shrink chain, and each pair's C matmuls are
deferred until after the next pair's A phase to fill the final-shrink
latency. Phase A and C have dedicated PSUM pools so the B-step psum ring
never blocks them.
"""

import numpy as np
from contextlib import ExitStack

import concourse.bass as bass
import concourse.bacc as bacc
import concourse.tile as tile
import concourse.mybir as mybir
from concourse.bass import ts, ds

P = 128
NCORES = 8
B_FULL, DIN, DD = 16384, 1024, 2048
BSH = B_FULL // NCORES      # 2048 batch rows per core
CH = 256                    # batch columns per chunk
NCH = BSH // CH             # 8 chunks
IT = DD // P                # 16 dict tiles
JP = IT // 2                # 8 DoubleRow pairs
KW = DIN // P               # 8 d_in tiles
CN = 512                    # free dim of phase-C matmuls
SC = 32.0                   # global scale 2^5
NSF8 = 4                    # trailing split-fp8 steps (rest single fp8)
GR = 2                      # dict tiles per shrink group
GRP = 4                     # dict tiles per psum/add group

F8 = mybir.dt.float8e4
F16 = mybir.dt.float16
F32 = mybir.dt.float32
ADD = mybir.AluOpType.add
SUB = mybir.AluOpType.subtract
MIN = mybir.AluOpType.min
MAX = mybir.AluOpType.max
RELU = mybir.ActivationFunctionType.Relu
COPY = mybir.ActivationFunctionType.Copy
DR = mybir.MatmulPerfMode.DoubleRow

_built = {}


def _build(steps: int):
    nc = bacc.Bacc("TRN2", target_bir_lowering=False, debug=False, num_devices=NCORES)

    def inp(name, shape, dt):
        return nc.dram_tensor(name, shape, dt, kind="ExternalInput").ap()

    yT16 = inp("yT16", (DIN, BSH), F16)
    W16_d = inp("W16", (DIN, DD), F16)        # f16(32*W)
    S8_d = inp("S8", (DD, 2, DD), F8)         # [j, (lo,hi), :] interleaved
    Dx8_d = inp("Dx8", (DD, 2, DIN), F8)        # [j, (lo,hi), :] * 32
    pth_d = inp("pth", (DD,), F32)            # +32*th
    nth_d = inp("nth", (DD,), F32)            # -32*th
    nthu_d = inp("nthu", (DD,), F32)          # -th (unscaled)
    out_d = nc.dram_tensor("out", (BSH, DIN), F32, kind="ExternalOutput").ap()

    n_sf8 = min(NSF8, steps)
    n_f8 = steps - n_sf8
    mode = ["f8"] * n_f8 + ["sf8"] * n_sf8    # mode[t] for step t (0-based)

    with tile.TileContext(nc) as tc, ExitStack() as top:
        thp = top.enter_context(tc.tile_pool(name="thp", bufs=1))
        pth_t = thp.tile([P, IT], F32)
        nth_t = thp.tile([P, IT], F32)
        nthu_t = thp.tile([P, IT], F32)

        wp = top.enter_context(tc.tile_pool(name="wp", bufs=1))
        W_t = wp.tile([P, KW, DD], F16, name="W_t")
        for k in range(KW):
            nc.sync.dma_start(W_t[:, k, :], W16_d[ts(k, P), :])

        yp = top.enter_context(tc.tile_pool(name="yp", bufs=2))
        y_pre = {}
        for c in (0, 1):
            y_t = yp.tile([P, KW, CH], F16, tag="y", name="y_pre")
            for k in range(KW):
                nc.sync.dma_start(y_t[:, k, :], yT16[ts(k, P), ds(c * CH, CH)])
            y_pre[c] = y_t

        nc.sync.dma_start(pth_t[:], pth_d.rearrange("(io p) -> p io", p=P))
        nc.sync.dma_start(nth_t[:], nth_d.rearrange("(io p) -> p io", p=P))
        nc.sync.dma_start(nthu_t[:], nthu_d.rearrange("(io p) -> p io", p=P))

        sp = top.enter_context(tc.tile_pool(name="sp", bufs=1))
        S_t = sp.tile([P, IT, 2, DD], F8, name="S_t")   # slot0=lo, slot1=hi
        for j in range(IT):
            nc.sync.dma_start(S_t[:, j, :, :], S8_d[ts(j, P), :, :])

        dxp = top.enter_context(tc.tile_pool(name="dxp", bufs=1))
        Dx_t = dxp.tile([P, IT, 2, DIN], F8, name="Dx_t")   # slot0=lo, slot1=hi
        for io in range(IT):
            nc.sync.dma_start(Dx_t[:, io, :, :], Dx8_d[ts(io, P), :, :])
        v0p = top.enter_context(tc.tile_pool(name="v0p", bufs=2))
        up = top.enter_context(tc.tile_pool(name="up", bufs=4))
        psp = top.enter_context(tc.tile_pool(name="psp", bufs=2, space="PSUM"))
        psap = top.enter_context(tc.tile_pool(name="psap", bufs=2, space="PSUM"))
        pscp = top.enter_context(tc.tile_pool(name="pscp", bufs=2, space="PSUM"))
        vp = top.enter_context(tc.tile_pool(name="vp", bufs=2))
        cp = top.enter_context(tc.tile_pool(name="cp", bufs=2))
        u16p = top.enter_context(tc.tile_pool(name="u16p", bufs=3))
        # NOTE: SBUF is within ~1KB of full; keep pool sizes in sync with budget
        pqp = top.enter_context(tc.tile_pool(name="pqp", bufs=2))
        u32p = top.enter_context(tc.tile_pool(name="u32p", bufs=2))
        stp = top.enter_context(tc.tile_pool(name="stp", bufs=2))

        def shrink_f8(vsrc_quad, q, u8_t):
            """vsrc_quad: [P,4,CH] f16 AP (scaled-32 v). Writes uh into slot 0."""
            c_t = cp.tile([P, GR, CH], F16, tag="c")
            for s in range(GR):
                i = GR * q + s
                nc.gpsimd.tensor_scalar(
                    c_t[:, s, :], vsrc_quad[:, s, :],
                    pth_t[:, i:i + 1], nth_t[:, i:i + 1], MIN, op1=MAX)
            u16_t = u16p.tile([P, GR, CH], F16, tag="u16")
            nc.vector.tensor_tensor(u16_t[:], vsrc_quad, c_t[:], SUB)
            nc.scalar.activation(u8_t[:, GR * q:GR * q + GR, 0, :], u16_t[:],
                                 COPY, scale=1.0 / SC)

        def shrink_sf8(vsrc_quad, q, u8_t):
            """Split-fp8 shrink: uh -> slot0, ul -> slot1 (u8 slots REVERSED vs S8)."""
            for s in range(GR):
                i = GR * q + s
                bias = nthu_t[:, i:i + 1]
                p_t = pqp.tile([P, CH], F32, tag="p")
                q_t = pqp.tile([P, CH], F32, tag="q")
                nc.scalar.activation(p_t[:], vsrc_quad[:, s, :], RELU,
                                     bias=bias, scale=1.0 / SC)
                nc.scalar.activation(q_t[:], vsrc_quad[:, s, :], RELU,
                                     bias=bias, scale=-1.0 / SC)
                u32 = u32p.tile([P, CH], F32, tag="u32")
                nc.vector.tensor_tensor(u32[:], p_t[:], q_t[:], SUB)
                nc.vector.tensor_copy(u8_t[:, i, 0, :], u32[:])
                nc.gpsimd.tensor_tensor(u8_t[:, i, 1, :], u32[:], u8_t[:, i, 0, :], SUB)

        def shrink_af8(vsrc_pair, q, a_t):
            """Final shrink -> SCALED-32 split-fp8 a (ah8 slot0, al8 slot1).
            e4m3(32*a) == 32*e4m3(a) exactly, so phase C just descales by 2^-10."""
            c_t = cp.tile([P, GR, CH], F16, tag="c")
            for s in range(GR):
                i = GR * q + s
                nc.gpsimd.tensor_scalar(
                    c_t[:, s, :], vsrc_pair[:, s, :],
                    pth_t[:, i:i + 1], nth_t[:, i:i + 1], MIN, op1=MAX)
            u16_t = u16p.tile([P, GR, CH], F16, tag="u16")
            nc.vector.tensor_tensor(u16_t[:], vsrc_pair, c_t[:], SUB)
            nc.scalar.activation(a_t[:, GR * q:GR * q + GR, 0, :], u16_t[:],
                                 COPY, scale=1.0)
            nc.gpsimd.tensor_tensor(a_t[:, GR * q:GR * q + GR, 1, :], u16_t[:],
                                    a_t[:, GR * q:GR * q + GR, 0, :], SUB)

        NQ = IT // GR   # shrink groups per step

        def phase_a(c):
            cs = ds(c * CH, CH)
            if c in y_pre:
                y_t = y_pre.pop(c)
            else:
                y_t = yp.tile([P, KW, CH], F16, tag="y")
                for k in range(KW):
                    nc.sync.dma_start(y_t[:, k, :], yT16[ts(k, P), cs])
            v0_t = v0p.tile([P, IT, CH], F16, tag="v0")
            for q in range(NQ):
                ps = psap.tile([P, GR, CH], F32, tag="psA")
                for s in range(GR):
                    i = GR * q + s
                    for k in range(KW):
                        nc.tensor.matmul(ps[:, s, :], W_t[:, k, ts(i, P)],
                                         y_t[:, k, :],
                                         start=(k == 0), stop=(k == KW - 1))
                nc.scalar.activation(v0_t[:, GR * q:GR * q + GR, :], ps[:],
                                     COPY, scale=1.0)
            return v0_t

        def first_shrink(v0_t, u8_t, fmt):
            for q in range(NQ):
                if fmt == "f8":
                    shrink_f8(v0_t[:, GR * q:GR * q + GR, :], q, u8_t)
                else:
                    shrink_sf8(v0_t[:, GR * q:GR * q + GR, :], q, u8_t)

        def step_pair(t, pair, v0s, u8s, u8ns, ats):
            """One B step for both chunks of the pair (chunk-sequential)."""
            m = mode[t]
            last = t == steps - 1
            nxt = None if last else mode[t + 1]
            for c in pair:
              for qq in range(IT // GRP):
                v0_t, u8_t = v0s[c], u8s[c]
                u8_n = None if last else u8ns[c]
                a_t = ats[c] if last else None
                ps = psp.tile([P, GRP, CH], F32, tag="ps")
                for s in range(GRP):
                    i = GRP * qq + s
                    if m == "f8":
                        n_mm = JP
                        for jp in range(JP):
                            nc.tensor.matmul(
                                ps[:, s, :],
                                S_t[:, 2 * jp:2 * jp + 2, 1, ts(i, P)],
                                u8_t[:, 2 * jp:2 * jp + 2, 0, :],
                                start=(jp == 0), stop=(jp == n_mm - 1),
                                perf_mode=DR)
                    else:
                        n_mm = JP + IT
                        k = 0
                        for jp in range(JP):
                            nc.tensor.matmul(
                                ps[:, s, :],
                                S_t[:, 2 * jp:2 * jp + 2, 1, ts(i, P)],
                                u8_t[:, 2 * jp:2 * jp + 2, 0, :],
                                start=(k == 0), stop=(k == n_mm - 1),
                                perf_mode=DR)
                            k += 1
                        for j in range(IT):
                            nc.tensor.matmul(
                                ps[:, s, :],
                                S_t[:, j, :, ts(i, P)],
                                u8_t[:, j, :, :],
                                start=False, stop=(k == n_mm - 1),
                                perf_mode=DR)
                            k += 1
                v_t = vp.tile([P, GRP, CH], F16, tag="v")
                nc.vector.tensor_tensor(v_t[:], ps[:],
                                        v0_t[:, GRP * qq:GRP * qq + GRP, :], ADD)
                for h in range(GRP // GR):
                    q = (GRP // GR) * qq + h
                    v_pair = v_t[:, GR * h:GR * h + GR, :]
                    if last:
                        shrink_af8(v_pair, q, a_t)
                    elif nxt == "f8":
                        shrink_f8(v_pair, q, u8_n)
                    else:
                        shrink_sf8(v_pair, q, u8_n)

        def phase_c(c, a_t):
            # out = (ah @ (DxH + DxL) + al @ DxH) / 32, all DoubleRow fp8
            cs_base = c * CH
            for bt in range(CH // P):
                for dn in range(DIN // CN):
                    ps = pscp.tile([P, CN], F32, tag="psC")
                    n_mm = JP + IT
                    k = 0
                    for jp in range(JP):
                        nc.tensor.matmul(
                            ps[:], a_t[:, 2 * jp:2 * jp + 2, 0, ts(bt, P)],
                            Dx_t[:, 2 * jp:2 * jp + 2, 1, ts(dn, CN)],
                            start=(k == 0), stop=(k == n_mm - 1), perf_mode=DR)
                        k += 1
                    for j in range(IT):
                        nc.tensor.matmul(
                            ps[:], a_t[:, j, :, ts(bt, P)],
                            Dx_t[:, j, :, ts(dn, CN)],
                            start=False, stop=(k == n_mm - 1), perf_mode=DR)
                        k += 1
                    st = stp.tile([P, CN], F32, tag="st")
                    nc.scalar.activation(st[:], ps[:], COPY, scale=1.0 / (SC * SC))
                    nc.sync.dma_start(out_d[ds(cs_base + bt * P, P), ts(dn, CN)],
                                      st[:])

        # ---- main schedule: chunk pairs; previous pair's C is emitted after
        # the next pair's A so its matmuls fill the shrink-chain latency ----
        pending_c = []
        for cp0 in range(0, NCH, 2):
            pair = (cp0, cp0 + 1)
            v0s, u8s, ats = {}, {}, {}
            for c in pair:
                v0s[c] = phase_a(c)
                u8s[c] = up.tile([P, IT, 2, CH], F8, tag="u8", name="u8_t")
                first_shrink(v0s[c], u8s[c], mode[0])
                ats[c] = None
            for nxt_c in (cp0 + 2, cp0 + 3):
                if nxt_c < NCH and nxt_c not in y_pre:
                    y_t = yp.tile([P, KW, CH], F16, tag="y", name="y_nxt")
                    for k in range(KW):
                        nc.sync.dma_start(y_t[:, k, :],
                                          yT16[ts(k, P), ds(nxt_c * CH, CH)])
                    y_pre[nxt_c] = y_t
            for c, a_t in pending_c:
                phase_c(c, a_t)
            pending_c = []
            for t in range(steps):
                last = t == steps - 1
                u8ns = {}
                for c in pair:
                    if last:
                        u8ns[c] = None
                        ats[c] = up.tile([P, IT, 2, CH], F8, tag="u8", name="a_t")
                    else:
                        u8ns[c] = up.tile([P, IT, 2, CH], F8, tag="u8", name="u8_n")
                step_pair(t, pair, v0s, u8s, u8ns, ats)
                for c in pair:
                    u8s[c] = u8ns[c]
            pending_c = [(c, ats[c]) for c in pair]
        for c, a_t in pending_c:
            phase_c(c, a_t)

    nc.compile()
    return nc


def _prep_in_maps(y, W, Theta, S, Dx):
    import ml_dtypes
    E4 = ml_dtypes.float8_e4m3  # TRN flavor (max normal 240)

    y = np.asarray(y, dtype=np.float32)
    W = np.asarray(W, dtype=np.float32)
    Theta = np.asarray(Theta, dtype=np.float32)
    S = np.asarray(S, dtype=np.float32)
    Dx = np.asarray(Dx, dtype=np.float32)
    assert y.shape == (B_FULL, DIN) and W.shape == (DIN, DD)
    assert S.shape == (DD, DD) and Dx.shape == (DD, DIN)

    th = np.maximum(Theta, 0.0) + np.float32(1e-7)
    W16 = (SC * W).astype(np.float16)
    Ss = np.float32(SC) * S
    S8H = np.clip(Ss, -240, 240).astype(E4)
    S8L = np.clip(Ss - S8H.astype(np.float32), -240, 240).astype(E4)
    S8 = np.stack([S8L, S8H], axis=1)         # [DD, 2, DD], slot0=lo slot1=hi
    Dxs = np.float32(SC) * Dx
    Dx8H = np.clip(Dxs, -240, 240).astype(E4)
    Dx8L = np.clip(Dxs - Dx8H.astype(np.float32), -240, 240).astype(E4)
    Dx8 = np.ascontiguousarray(np.stack([Dx8L, Dx8H], axis=1))
    yT16 = np.ascontiguousarray(y.T).astype(np.float16)

    shared = dict(
        W16=W16, S8=np.ascontiguousarray(S8), Dx8=Dx8,
        pth=(SC * th).astype(np.float32),
        nth=(-SC * th).astype(np.float32),
        nthu=(-th).astype(np.float32),
    )
    in_maps = []
    for c in range(NCORES):
        sl = slice(c * BSH, (c + 1) * BSH)
        in_maps.append(dict(shared, yT16=np.ascontiguousarray(yT16[:, sl])))
    return in_maps


_sharded_cache = {}


def _get_sharded(steps: int):
    """Build (once) the jitted shard_map executable for the compiled NEFF."""
    if steps in _sharded_cache:
        return _sharded_cache[steps]
    import jax
    from jax.experimental.shard_map import shard_map
    from jax.sharding import Mesh, PartitionSpec
    from concourse import bass2jax

    if steps not in _built:
        _built[steps] = _build(steps)
    nc = _built[steps]
    bass2jax.install_neuronx_cc_hook()
    assert nc.dbg_addr is None
    partition_name = nc.partition_id_tensor.name if nc.partition_id_tensor else None

    in_names, out_names, out_avals, zero_shapes = [], [], [], []
    for alloc in nc.m.functions[0].allocations:
        if not isinstance(alloc, mybir.MemoryLocationSet):
            continue
        name = alloc.memorylocations[0].name
        if alloc.kind == "ExternalInput":
            if name != partition_name:
                in_names.append(name)
        elif alloc.kind == "ExternalOutput":
            out_names.append(name)
            shape = tuple(alloc.tensor_shape)
            dtype = mybir.dt.np(alloc.dtype)
            out_avals.append(jax.core.ShapedArray(shape, dtype))
            zero_shapes.append((shape, dtype))
    n_params = len(in_names)
    n_outs = len(out_names)
    all_in_names = in_names + out_names
    if partition_name is not None:
        all_in_names.append(partition_name)

    def _body(*args):
        operands = list(args)
        if partition_name is not None:
            operands.append(bass2jax.partition_id_tensor())
        outs = bass2jax._bass_exec_p.bind(
            *operands,
            out_avals=tuple(out_avals),
            in_names=tuple(all_in_names),
            out_names=tuple(out_names),
            lowering_input_output_aliases=(),
            sim_require_finite=True,
            sim_require_nnan=True,
            nc=nc,
        )
        return tuple(outs)

    devices = jax.devices()[:NCORES]
    mesh = Mesh(np.asarray(devices), ("core",))
    donate = tuple(range(n_params, n_params + n_outs))
    sharded = jax.jit(
        shard_map(_body, mesh=mesh,
                  in_specs=(PartitionSpec("core"),) * (n_params + n_outs),
                  out_specs=(PartitionSpec("core"),) * n_outs,
                  check_rep=False),
        donate_argnums=donate, keep_unused=True)
    entry = dict(sharded=sharded, in_names=in_names, out_names=out_names,
                 zero_shapes=zero_shapes, mesh=mesh, n_params=n_params)
    _sharded_cache[steps] = entry
    return entry


def _concat_inputs(entry, in_maps):
    return [np.concatenate([np.asarray(in_maps[c][n]) for c in range(NCORES)], axis=0)
            for n in entry["in_names"]]


def _run(entry, concat_in):
    zeros = [np.zeros((NCORES * s[0], *s[1:]), d) for s, d in entry["zero_shapes"]]
    out_arrs = entry["sharded"](*concat_in, *zeros)
    return out_arrs


def kernel(y, W, Theta, S, Dx, unroll_steps):
    steps = int(unroll_steps)
    entry = _get_sharded(steps)
    in_maps = _prep_in_maps(y, W, Theta, S, Dx)
    out_arrs = _run(entry, _concat_inputs(entry, in_maps))
    idx = entry["out_names"].index("out")
    return np.ascontiguousarray(np.asarray(out_arrs[idx]))  # [NCORES*BSH, DIN]


def time_kernel(np_inputs, iters=6):
    """Steady-state wall time per NEFF execution (ns), device-resident inputs."""
    import jax
    from jax.sharding import NamedSharding, PartitionSpec
    steps = int(np_inputs["unroll_steps"])
    entry = _get_sharded(steps)
    in_maps = _prep_in_maps(np_inputs["y"], np_inputs["W"], np_inputs["Theta"],
                            np_inputs["S"], np_inputs["Dx"])
    concat_in = _concat_inputs(entry, in_maps)
    sh = NamedSharding(entry["mesh"], PartitionSpec("core"))
    dev_in = [jax.device_put(a, sh) for a in concat_in]
    import time as _time
    times = []
    for it in range(iters):
        zeros = [jax.device_put(np.zeros((NCORES * s[0], *s[1:]), d), sh)
                 for s, d in entry["zero_shapes"]]
        for z in zeros:
            z.block_until_ready()
        t0 = _time.perf_counter()
        outs = entry["sharded"](*dev_in, *zeros)
        for o in outs:
            o.block_until_ready()
        times.append(_time.perf_counter() - t0)
    best = min(times[1:]) if len(times) > 1 else times[0]
    print("  per-iter times (ms):", [f"{t*1e3:.1f}" for t in times])
    return best * 1e9


if __name__ == "__main__":
    rng = np.random.default_rng(0)
    inputs = dict(
        y=rng.standard_normal((B_FULL, DIN), dtype=np.float32),
        W=(rng.standard_normal((DIN, DD)) * 0.02).astype(np.float32),
        Theta=rng.random(DD, dtype=np.float32),
        S=(rng.standard_normal((DD, DD)) * 0.02).astype(np.float32),
        Dx=(rng.standard_normal((DD, DIN)) * 0.02).astype(np.float32),
        unroll_steps=16,
    )
    out = kernel(**inputs)
    print("out", out.shape, out.dtype, np.abs(out).max())


# revision 42
# speedup vs baseline: 4.6972x; 1.0008x over previous
"""Trainium2 Bass kernel for a LISTA layer (nn_ListaLayer).

Reference computation (jax, fp32):
    th = relu(Theta) + 1e-7
    xW = (y @ W) / th
    repeat 16: z = xW + (unit_threshold(z) * th @ S) / th
    out = (unit_threshold(z) * th) @ Dx
where unit_threshold(v) = sign(v) * relu(|v| - 1).

Algebraic restructure (exact): track v = z * th:
    v0 = y @ W
    repeat 16:  u = soft_threshold(v, th) = sign(v) * relu(|v| - th)
                v = v0 + u @ S
    out = soft_threshold(v, th) @ Dx

Distribution: data-parallel over batch rows, 8 NeuronCores, 2048 rows each.
W/th/S/Dx replicated; no collectives.

Numerics / performance scheme (v-space carried SCALED by 32 in fp16):
  - A:  v~0 = y16 @ f16(32*W)   (fp16 matmul, fp32 PSUM)
  - B:  16 soft-threshold + u@S steps, all matmuls fp8-e4m3 DoubleRow
        (2 dict-tiles contracted per pass). First 12 steps single-pass
        (u8 @ S8H); last NSF8=4 steps "split-fp8": uh8 @ (S8H+S8L) +
        ul8 @ S8H, the cross terms folded into per-j DoubleRow matmuls by
        pairing weight slots (S8L[j],S8H[j]) against moving slots
        (uh[j],ul[j]) - DoubleRow multiplies same-index slots, so the u8
        tile stores (uh, ul) while the S tile stores (lo, hi).
        S8H = e4m3(32*S), S8L = e4m3(32*S - S8H); u is consumed UNSCALED
        (ACT applies the 2^-5 descale when emitting fp8), so psum comes out
        scaled 32 and adds directly onto v~0.
        f8-step shrink per pair of dict tiles: DVE add (psum+v~0 -> f16),
        GPSIMD fused clamp (tensor_scalar min/max, per-partition +-32*th),
        DVE sub (f16 2x), ACT copy->fp8 (scale 2^-5).
  - C:  out = (ah8 @ (DxH+DxL) + al8 @ DxH) * 2^-10, DoubleRow fp8 with the
        final shrink emitting a scaled-32 hi/lo split (exact: e4m3(32a) ==
        32*e4m3(a)); the 2^-10 descale rides the PSUM->SBUF ACT copy.
All phases fused per 256-column batch chunk; two chunks interleaved so the
tensor engine never waits on a shrink chain, and each pair's C matmuls are
deferred until after the next pair's A phase to fill the final-shrink
latency. Phase A and C have dedicated PSUM pools so the B-step psum ring
never blocks them.
"""

import numpy as np
from contextlib import ExitStack

import concourse.bass as bass
import concourse.bacc as bacc
import concourse.tile as tile
import concourse.mybir as mybir
from concourse.bass import ts, ds

P = 128
NCORES = 8
B_FULL, DIN, DD = 16384, 1024, 2048
BSH = B_FULL // NCORES      # 2048 batch rows per core
CH = 256                    # batch columns per chunk
NCH = BSH // CH             # 8 chunks
IT = DD // P                # 16 dict tiles
JP = IT // 2                # 8 DoubleRow pairs
KW = DIN // P               # 8 d_in tiles
CN = 512                    # free dim of phase-C matmuls
SC = 32.0                   # global scale 2^5
NSF8 = 4                    # trailing split-fp8 steps (rest single fp8)
GR = 2                      # dict tiles per shrink group
GRP = 4                     # dict tiles per psum/add group

F8 = mybir.dt.float8e4
F16 = mybir.dt.float16
F32 = mybir.dt.float32
ADD = mybir.AluOpType.add
SUB = mybir.AluOpType.subtract
MIN = mybir.AluOpType.min
MAX = mybir.AluOpType.max
RELU = mybir.ActivationFunctionType.Relu
COPY = mybir.ActivationFunctionType.Copy
DR = mybir.MatmulPerfMode.DoubleRow

_built = {}


def _build(steps: int):
    nc = bacc.Bacc("TRN2", target_bir_lowering=False, debug=False, num_devices=NCORES)

    def inp(name, shape, dt):
        return nc.dram_tensor(name, shape, dt, kind="ExternalInput").ap()

    yT16 = inp("yT16", (DIN, BSH), F16)
    W16_d = inp("W16", (DIN, DD), F16)        # f16(32*W)
    S8_d = inp("S8", (DD, 2, DD), F8)         # [j, (lo,hi), :] interleaved
    Dx8_d = inp("Dx8", (DD, 2, DIN), F8)        # [j, (lo,hi), :] * 32
    pth_d = inp("pth", (DD,), F32)            # +32*th
    nth_d = inp("nth", (DD,), F32)            # -32*th
    nthu_d = inp("nthu", (DD,), F32)          # -th (unscaled)
    out_d = nc.dram_tensor("out", (BSH, DIN), F32, kind="ExternalOutput").ap()

    n_sf8 = min(NSF8, steps)
    n_f8 = steps - n_sf8
    mode = ["f8"] * n_f8 + ["sf8"] * n_sf8    # mode[t] for step t (0-based)

    with tile.TileContext(nc) as tc, ExitStack() as top:
        thp = top.enter_context(tc.tile_pool(name="thp", bufs=1))
        pth_t = thp.tile([P, IT], F32)
        nth_t = thp.tile([P, IT], F32)
        nthu_t = thp.tile([P, IT], F32)

        # y(chunk 0) first, then W in half-tiles: phase A starts ~2.5us in and
        # overlaps the remaining W DMA instead of waiting for all of it.
        wp = top.enter_context(tc.tile_pool(name="wp", bufs=1))
        W_t = wp.tile([P, KW, DD], F16, name="W_t")
        yp = top.enter_context(tc.tile_pool(name="yp", bufs=2))
        y_pre = {}
        for c in (0, 1):
            y_t = yp.tile([P, KW, CH], F16, tag="y", name="y_pre")
            y_pre[c] = y_t
        for k in range(KW):
            nc.sync.dma_start(y_pre[0][:, k, :], yT16[ts(k, P), ds(0, CH)])
        for h in range(2):
            for k in range(KW):
                nc.sync.dma_start(W_t[:, k, ts(h, DD // 2)],
                                  W16_d[ts(k, P), ts(h, DD // 2)])
        for k in range(KW):
            nc.sync.dma_start(y_pre[1][:, k, :], yT16[ts(k, P), ds(CH, CH)])

        nc.sync.dma_start(pth_t[:], pth_d.rearrange("(io p) -> p io", p=P))
        nc.sync.dma_start(nth_t[:], nth_d.rearrange("(io p) -> p io", p=P))
        nc.sync.dma_start(nthu_t[:], nthu_d.rearrange("(io p) -> p io", p=P))

        sp = top.enter_context(tc.tile_pool(name="sp", bufs=1))
        S_t = sp.tile([P, IT, 2, DD], F8, name="S_t")   # slot0=lo, slot1=hi
        for j in range(IT):
            nc.sync.dma_start(S_t[:, j, :, :], S8_d[ts(j, P), :, :])

        dxp = top.enter_context(tc.tile_pool(name="dxp", bufs=1))
        Dx_t = dxp.tile([P, IT, 2, DIN], F8, name="Dx_t")   # slot0=lo, slot1=hi
        for io in range(IT):
            nc.sync.dma_start(Dx_t[:, io, :, :], Dx8_d[ts(io, P), :, :])
        v0p = top.enter_context(tc.tile_pool(name="v0p", bufs=2))
        up = top.enter_context(tc.tile_pool(name="up", bufs=4))
        psp = top.enter_context(tc.tile_pool(name="psp", bufs=2, space="PSUM"))
        psap = top.enter_context(tc.tile_pool(name="psap", bufs=2, space="PSUM"))
        pscp = top.enter_context(tc.tile_pool(name="pscp", bufs=2, space="PSUM"))
        vp = top.enter_context(tc.tile_pool(name="vp", bufs=2))
        cp = top.enter_context(tc.tile_pool(name="cp", bufs=2))
        u16p = top.enter_context(tc.tile_pool(name="u16p", bufs=3))
        # NOTE: SBUF is within ~1KB of full; keep pool sizes in sync with budget
        pqp = top.enter_context(tc.tile_pool(name="pqp", bufs=2))
        u32p = top.enter_context(tc.tile_pool(name="u32p", bufs=2))
        stp = top.enter_context(tc.tile_pool(name="stp", bufs=2))

        def shrink_f8(vsrc_quad, q, u8_t):
            """vsrc_quad: [P,4,CH] f16 AP (scaled-32 v). Writes uh into slot 0."""
            c_t = cp.tile([P, GR, CH], F16, tag="c")
            for s in range(GR):
                i = GR * q + s
                nc.gpsimd.tensor_scalar(
                    c_t[:, s, :], vsrc_quad[:, s, :],
                    pth_t[:, i:i + 1], nth_t[:, i:i + 1], MIN, op1=MAX)
            u16_t = u16p.tile([P, GR, CH], F16, tag="u16")
            nc.vector.tensor_tensor(u16_t[:], vsrc_quad, c_t[:], SUB)
            nc.scalar.activation(u8_t[:, GR * q:GR * q + GR, 0, :], u16_t[:],
                                 COPY, scale=1.0 / SC)

        def shrink_sf8(vsrc_quad, q, u8_t):
            """Split-fp8 shrink: uh -> slot0, ul -> slot1 (u8 slots REVERSED vs S8)."""
            for s in range(GR):
                i = GR * q + s
                bias = nthu_t[:, i:i + 1]
                p_t = pqp.tile([P, CH], F32, tag="p")
                q_t = pqp.tile([P, CH], F32, tag="q")
                nc.scalar.activation(p_t[:], vsrc_quad[:, s, :], RELU,
                                     bias=bias, scale=1.0 / SC)
                nc.scalar.activation(q_t[:], vsrc_quad[:, s, :], RELU,
                                     bias=bias, scale=-1.0 / SC)
                u32 = u32p.tile([P, CH], F32, tag="u32")
                nc.vector.tensor_tensor(u32[:], p_t[:], q_t[:], SUB)
                nc.vector.tensor_copy(u8_t[:, i, 0, :], u32[:])
                nc.gpsimd.tensor_tensor(u8_t[:, i, 1, :], u32[:], u8_t[:, i, 0, :], SUB)

        def shrink_af8(vsrc_pair, q, a_t):
            """Final shrink -> SCALED-32 split-fp8 a (ah8 slot0, al8 slot1).
            e4m3(32*a) == 32*e4m3(a) exactly, so phase C just descales by 2^-10."""
            c_t = cp.tile([P, GR, CH], F16, tag="c")
            for s in range(GR):
                i = GR * q + s
                nc.gpsimd.tensor_scalar(
                    c_t[:, s, :], vsrc_pair[:, s, :],
                    pth_t[:, i:i + 1], nth_t[:, i:i + 1], MIN, op1=MAX)
            u16_t = u16p.tile([P, GR, CH], F16, tag="u16")
            nc.vector.tensor_tensor(u16_t[:], vsrc_pair, c_t[:], SUB)
            nc.scalar.activation(a_t[:, GR * q:GR * q + GR, 0, :], u16_t[:],
                                 COPY, scale=1.0)
            nc.gpsimd.tensor_tensor(a_t[:, GR * q:GR * q + GR, 1, :], u16_t[:],
                                    a_t[:, GR * q:GR * q + GR, 0, :], SUB)

        NQ = IT // GR   # shrink groups per step

        def phase_a(c):
            cs = ds(c * CH, CH)
            if c in y_pre:
                y_t = y_pre.pop(c)
            else:
                y_t = yp.tile([P, KW, CH], F16, tag="y")
                for k in range(KW):
                    nc.sync.dma_start(y_t[:, k, :], yT16[ts(k, P), cs])
            v0_t = v0p.tile([P, IT, CH], F16, tag="v0")
            for q in range(NQ):
                ps = psap.tile([P, GR, CH], F32, tag="psA")
                for s in range(GR):
                    i = GR * q + s
                    for k in range(KW):
                        nc.tensor.matmul(ps[:, s, :], W_t[:, k, ts(i, P)],
                                         y_t[:, k, :],
                                         start=(k == 0), stop=(k == KW - 1))
                nc.scalar.activation(v0_t[:, GR * q:GR * q + GR, :], ps[:],
                                     COPY, scale=1.0)
            return v0_t

        def first_shrink(v0_t, u8_t, fmt):
            for q in range(NQ):
                if fmt == "f8":
                    shrink_f8(v0_t[:, GR * q:GR * q + GR, :], q, u8_t)
                else:
                    shrink_sf8(v0_t[:, GR * q:GR * q + GR, :], q, u8_t)

        def step_pair(t, pair, v0s, u8s, u8ns, ats):
            """One B step for both chunks of the pair (chunk-sequential)."""
            m = mode[t]
            last = t == steps - 1
            nxt = None if last else mode[t + 1]
            for c in pair:
              for qq in range(IT // GRP):
                v0_t, u8_t = v0s[c], u8s[c]
                u8_n = None if last else u8ns[c]
                a_t = ats[c] if last else None
                ps = psp.tile([P, GRP, CH], F32, tag="ps")
                for s in range(GRP):
                    i = GRP * qq + s
                    if m == "f8":
                        n_mm = JP
                        for jp in range(JP):
                            nc.tensor.matmul(
                                ps[:, s, :],
                                S_t[:, 2 * jp:2 * jp + 2, 1, ts(i, P)],
                                u8_t[:, 2 * jp:2 * jp + 2, 0, :],
                                start=(jp == 0), stop=(jp == n_mm - 1),
                                perf_mode=DR)
                    else:
                        n_mm = JP + IT
                        k = 0
                        for jp in range(JP):
                            nc.tensor.matmul(
                                ps[:, s, :],
                                S_t[:, 2 * jp:2 * jp + 2, 1, ts(i, P)],
                                u8_t[:, 2 * jp:2 * jp + 2, 0, :],
                                start=(k == 0), stop=(k == n_mm - 1),
                                perf_mode=DR)
                            k += 1
                        for j in range(IT):
                            nc.tensor.matmul(
                                ps[:, s, :],
                                S_t[:, j, :, ts(i, P)],
                                u8_t[:, j, :, :],
                                start=False, stop=(k == n_mm - 1),
                                perf_mode=DR)
                            k += 1
                v_t = vp.tile([P, GRP, CH], F16, tag="v")
                nc.vector.tensor_tensor(v_t[:], ps[:],
                                        v0_t[:, GRP * qq:GRP * qq + GRP, :], ADD)
                for h in range(GRP // GR):
                    q = (GRP // GR) * qq + h
                    v_pair = v_t[:, GR * h:GR * h + GR, :]
                    if last:
                        shrink_af8(v_pair, q, a_t)
                    elif nxt == "f8":
                        shrink_f8(v_pair, q, u8_n)
                    else:
                        shrink_sf8(v_pair, q, u8_n)

        def phase_c(c, a_t):
            # out = (ah @ (DxH + DxL) + al @ DxH) / 32, all DoubleRow fp8
            cs_base = c * CH
            for bt in range(CH // P):
                for dn in range(DIN // CN):
                    ps = pscp.tile([P, CN], F32, tag="psC")
                    n_mm = JP + IT
                    k = 0
                    for jp in range(JP):
                        nc.tensor.matmul(
                            ps[:], a_t[:, 2 * jp:2 * jp + 2, 0, ts(bt, P)],
                            Dx_t[:, 2 * jp:2 * jp + 2, 1, ts(dn, CN)],
                            start=(k == 0), stop=(k == n_mm - 1), perf_mode=DR)
                        k += 1
                    for j in range(IT):
                        nc.tensor.matmul(
                            ps[:], a_t[:, j, :, ts(bt, P)],
                            Dx_t[:, j, :, ts(dn, CN)],
                            start=False, stop=(k == n_mm - 1), perf_mode=DR)
                        k += 1
                    st = stp.tile([P, CN], F32, tag="st")
                    nc.scalar.activation(st[:], ps[:], COPY, scale=1.0 / (SC * SC))
                    nc.sync.dma_start(out_d[ds(cs_base + bt * P, P), ts(dn, CN)],
                                      st[:])

        # ---- main schedule: chunk pairs; previous pair's C is emitted after
        # the next pair's A so its matmuls fill the shrink-chain latency ----
        pending_c = []
        for cp0 in range(0, NCH, 2):
            pair = (cp0, cp0 + 1)
            v0s, u8s, ats = {}, {}, {}
            for c in pair:
                v0s[c] = phase_a(c)
                u8s[c] = up.tile([P, IT, 2, CH], F8, tag="u8", name="u8_t")
                first_shrink(v0s[c], u8s[c], mode[0])
                ats[c] = None
            for nxt_c in (cp0 + 2, cp0 + 3):
                if nxt_c < NCH and nxt_c not in y_pre:
                    y_t = yp.tile([P, KW, CH], F16, tag="y", name="y_nxt")
                    for k in range(KW):
                        nc.sync.dma_start(y_t[:, k, :],
                                          yT16[ts(k, P), ds(nxt_c * CH, CH)])
                    y_pre[nxt_c] = y_t
            for c, a_t in pending_c:
                phase_c(c, a_t)
            pending_c = []
            for t in range(steps):
                last = t == steps - 1
                u8ns = {}
                for c in pair:
                    if last:
                        u8ns[c] = None
                        ats[c] = up.tile([P, IT, 2, CH], F8, tag="u8", name="a_t")
                    else:
                        u8ns[c] = up.tile([P, IT, 2, CH], F8, tag="u8", name="u8_n")
                step_pair(t, pair, v0s, u8s, u8ns, ats)
                for c in pair:
                    u8s[c] = u8ns[c]
            pending_c = [(c, ats[c]) for c in pair]
        for c, a_t in pending_c:
            phase_c(c, a_t)

    nc.compile()
    return nc


def _prep_in_maps(y, W, Theta, S, Dx):
    import ml_dtypes
    E4 = ml_dtypes.float8_e4m3  # TRN flavor (max normal 240)

    y = np.asarray(y, dtype=np.float32)
    W = np.asarray(W, dtype=np.float32)
    Theta = np.asarray(Theta, dtype=np.float32)
    S = np.asarray(S, dtype=np.float32)
    Dx = np.asarray(Dx, dtype=np.float32)
    assert y.shape == (B_FULL, DIN) and W.shape == (DIN, DD)
    assert S.shape == (DD, DD) and Dx.shape == (DD, DIN)

    th = np.maximum(Theta, 0.0) + np.float32(1e-7)
    W16 = (SC * W).astype(np.float16)
    Ss = np.float32(SC) * S
    S8H = np.clip(Ss, -240, 240).astype(E4)
    S8L = np.clip(Ss - S8H.astype(np.float32), -240, 240).astype(E4)
    S8 = np.stack([S8L, S8H], axis=1)         # [DD, 2, DD], slot0=lo slot1=hi
    Dxs = np.float32(SC) * Dx
    Dx8H = np.clip(Dxs, -240, 240).astype(E4)
    Dx8L = np.clip(Dxs - Dx8H.astype(np.float32), -240, 240).astype(E4)
    Dx8 = np.ascontiguousarray(np.stack([Dx8L, Dx8H], axis=1))
    yT16 = np.ascontiguousarray(y.T).astype(np.float16)

    shared = dict(
        W16=W16, S8=np.ascontiguousarray(S8), Dx8=Dx8,
        pth=(SC * th).astype(np.float32),
        nth=(-SC * th).astype(np.float32),
        nthu=(-th).astype(np.float32),
    )
    in_maps = []
    for c in range(NCORES):
        sl = slice(c * BSH, (c + 1) * BSH)
        in_maps.append(dict(shared, yT16=np.ascontiguousarray(yT16[:, sl])))
    return in_maps


_sharded_cache = {}


def _get_sharded(steps: int):
    """Build (once) the jitted shard_map executable for the compiled NEFF."""
    if steps in _sharded_cache:
        return _sharded_cache[steps]
    import jax
    from jax.experimental.shard_map import shard_map
    from jax.sharding import Mesh, PartitionSpec
    from concourse import bass2jax

    if steps not in _built:
        _built[steps] = _build(steps)
    nc = _built[steps]
    bass2jax.install_neuronx_cc_hook()
    assert nc.dbg_addr is None
    partition_name = nc.partition_id_tensor.name if nc.partition_id_tensor else None

    in_names, out_names, out_avals, zero_shapes = [], [], [], []
    for alloc in nc.m.functions[0].allocations:
        if not isinstance(alloc, mybir.MemoryLocationSet):
            continue
        name = alloc.memorylocations[0].name
        if alloc.kind == "ExternalInput":
            if name != partition_name:
                in_names.append(name)
        elif alloc.kind == "ExternalOutput":
            out_names.append(name)
            shape = tuple(alloc.tensor_shape)
            dtype = mybir.dt.np(alloc.dtype)
            out_avals.append(jax.core.ShapedArray(shape, dtype))
            zero_shapes.append((shape, dtype))
    n_params = len(in_names)
    n_outs = len(out_names)
    all_in_names = in_names + out_names
    if partition_name is not None:
        all_in_names.append(partition_name)

    def _body(*args):
        operands = list(args)
        if partition_name is not None:
            operands.append(bass2jax.partition_id_tensor())
        outs = bass2jax._bass_exec_p.bind(
            *operands,
            out_avals=tuple(out_avals),
            in_names=tuple(all_in_names),
            out_names=tuple(out_names),
            lowering_input_output_aliases=(),
            sim_require_finite=True,
            sim_require_nnan=True,
            nc=nc,
        )
        return tuple(outs)

    devices = jax.devices()[:NCORES]
    mesh = Mesh(np.asarray(devices), ("core",))
    donate = tuple(range(n_params, n_params + n_outs))
    sharded = jax.jit(
        shard_map(_body, mesh=mesh,
                  in_specs=(PartitionSpec("core"),) * (n_params + n_outs),
                  out_specs=(PartitionSpec("core"),) * n_outs,
                  check_rep=False),
        donate_argnums=donate, keep_unused=True)
    entry = dict(sharded=sharded, in_names=in_names, out_names=out_names,
                 zero_shapes=zero_shapes, mesh=mesh, n_params=n_params)
    _sharded_cache[steps] = entry
    return entry


def _concat_inputs(entry, in_maps):
    return [np.concatenate([np.asarray(in_maps[c][n]) for c in range(NCORES)], axis=0)
            for n in entry["in_names"]]


def _run(entry, concat_in):
    zeros = [np.zeros((NCORES * s[0], *s[1:]), d) for s, d in entry["zero_shapes"]]
    out_arrs = entry["sharded"](*concat_in, *zeros)
    return out_arrs


def kernel(y, W, Theta, S, Dx, unroll_steps):
    steps = int(unroll_steps)
    entry = _get_sharded(steps)
    in_maps = _prep_in_maps(y, W, Theta, S, Dx)
    out_arrs = _run(entry, _concat_inputs(entry, in_maps))
    idx = entry["out_names"].index("out")
    return np.ascontiguousarray(np.asarray(out_arrs[idx]))  # [NCORES*BSH, DIN]


def time_kernel(np_inputs, iters=6):
    """Steady-state wall time per NEFF execution (ns), device-resident inputs."""
    import jax
    from jax.sharding import NamedSharding, PartitionSpec
    steps = int(np_inputs["unroll_steps"])
    entry = _get_sharded(steps)
    in_maps = _prep_in_maps(np_inputs["y"], np_inputs["W"], np_inputs["Theta"],
                            np_inputs["S"], np_inputs["Dx"])
    concat_in = _concat_inputs(entry, in_maps)
    sh = NamedSharding(entry["mesh"], PartitionSpec("core"))
    dev_in = [jax.device_put(a, sh) for a in concat_in]
    import time as _time
    times = []
    for it in range(iters):
        zeros = [jax.device_put(np.zeros((NCORES * s[0], *s[1:]), d), sh)
                 for s, d in entry["zero_shapes"]]
        for z in zeros:
            z.block_until_ready()
        t0 = _time.perf_counter()
        outs = entry["sharded"](*dev_in, *zeros)
        for o in outs:
            o.block_until_ready()
        times.append(_time.perf_counter() - t0)
    best = min(times[1:]) if len(times) > 1 else times[0]
    print("  per-iter times (ms):", [f"{t*1e3:.1f}" for t in times])
    return best * 1e9


if __name__ == "__main__":
    rng = np.random.default_rng(0)
    inputs = dict(
        y=rng.standard_normal((B_FULL, DIN), dtype=np.float32),
        W=(rng.standard_normal((DIN, DD)) * 0.02).astype(np.float32),
        Theta=rng.random(DD, dtype=np.float32),
        S=(rng.standard_normal((DD, DD)) * 0.02).astype(np.float32),
        Dx=(rng.standard_normal((DD, DIN)) * 0.02).astype(np.float32),
        unroll_steps=16,
    )
    out = kernel(**inputs)
    print("out", out.shape, out.dtype, np.abs(out).max())


# revision 47
# speedup vs baseline: 4.7503x; 1.0113x over previous
"""Trainium2 Bass kernel for a LISTA layer (nn_ListaLayer).

Reference computation (jax, fp32):
    th = relu(Theta) + 1e-7
    xW = (y @ W) / th
    repeat 16: z = xW + (unit_threshold(z) * th @ S) / th
    out = (unit_threshold(z) * th) @ Dx
where unit_threshold(v) = sign(v) * relu(|v| - 1).

Algebraic restructure (exact): track v = z * th:
    v0 = y @ W
    repeat 16:  u = soft_threshold(v, th) = sign(v) * relu(|v| - th)
                v = v0 + u @ S
    out = soft_threshold(v, th) @ Dx

Distribution: data-parallel over batch rows, 8 NeuronCores, 2048 rows each.
W/th/S/Dx replicated; no collectives.

Numerics / performance scheme (v-space carried SCALED by 32 in fp16):
  - A:  v~0 = y16 @ f16(32*W)   (fp16 matmul, fp32 PSUM)
  - B:  16 soft-threshold + u@S steps, all matmuls fp8-e4m3 DoubleRow
        (2 dict-tiles contracted per pass). First 12 steps single-pass
        (u8 @ S8H); last NSF8=4 steps "split-fp8": uh8 @ (S8H+S8L) +
        ul8 @ S8H, the cross terms folded into per-j DoubleRow matmuls by
        pairing weight slots (S8L[j],S8H[j]) against moving slots
        (uh[j],ul[j]) - DoubleRow multiplies same-index slots, so the u8
        tile stores (uh, ul) while the S tile stores (lo, hi).
        S8H = e4m3(32*S), S8L = e4m3(32*S - S8H); u is consumed UNSCALED
        (ACT applies the 2^-5 descale when emitting fp8), so psum comes out
        scaled 32 and adds directly onto v~0.
        f8-step shrink per pair of dict tiles: DVE add (psum+v~0 -> f16),
        GPSIMD fused clamp (tensor_scalar min/max, per-partition +-32*th),
        DVE sub (f16 2x), ACT copy->fp8 (scale 2^-5).
  - C:  out = (ah8 @ (DxH+DxL) + al8 @ DxH) * 2^-10, DoubleRow fp8 with the
        final shrink emitting a scaled-32 hi/lo split (exact: e4m3(32a) ==
        32*e4m3(a)); the 2^-10 descale rides the PSUM->SBUF ACT copy.
All phases fused per 256-column batch chunk; two chunks interleaved so the
tensor engine never waits on a shrink chain, and each pair's C matmuls are
deferred until after the next pair's A phase to fill the final-shrink
latency. Phase A and C have dedicated PSUM pools so the B-step psum ring
never blocks them.
"""

import numpy as np
from contextlib import ExitStack

import concourse.bass as bass
import concourse.bacc as bacc
import concourse.tile as tile
import concourse.mybir as mybir
from concourse.bass import ts, ds

P = 128
NCORES = 8
B_FULL, DIN, DD = 16384, 1024, 2048
BSH = B_FULL // NCORES      # 2048 batch rows per core
CH = 256                    # batch columns per chunk
NCH = BSH // CH             # 8 chunks
IT = DD // P                # 16 dict tiles
JP = IT // 2                # 8 DoubleRow pairs
KW = DIN // P               # 8 d_in tiles
CN = 512                    # free dim of phase-C matmuls
SC = 32.0                   # global scale 2^5
NSF8 = 4                    # trailing split-fp8 steps (rest single fp8)
GR = 2                      # dict tiles per shrink group
GRP = 4                     # dict tiles per psum/add group

F8 = mybir.dt.float8e4
F16 = mybir.dt.float16
F32 = mybir.dt.float32
ADD = mybir.AluOpType.add
SUB = mybir.AluOpType.subtract
MIN = mybir.AluOpType.min
MAX = mybir.AluOpType.max
RELU = mybir.ActivationFunctionType.Relu
COPY = mybir.ActivationFunctionType.Copy
DR = mybir.MatmulPerfMode.DoubleRow

_built = {}


def _build(steps: int):
    nc = bacc.Bacc("TRN2", target_bir_lowering=False, debug=False, num_devices=NCORES)

    def inp(name, shape, dt):
        return nc.dram_tensor(name, shape, dt, kind="ExternalInput").ap()

    yT16 = inp("yT16", (DIN, BSH), F16)
    W16_d = inp("W16", (DIN, DD), F16)        # f16(32*W)
    S8_d = inp("S8", (DD, 2, DD), F8)         # [j, (lo,hi), :] interleaved
    Dx8_d = inp("Dx8", (DD, 2, DIN), F8)        # [j, (lo,hi), :] * 32
    pth_d = inp("pth", (DD,), F32)            # +32*th
    nth_d = inp("nth", (DD,), F32)            # -32*th
    nthu_d = inp("nthu", (DD,), F32)          # -th (unscaled)
    out_d = nc.dram_tensor("out", (BSH, DIN), F32, kind="ExternalOutput").ap()

    n_sf8 = min(NSF8, steps)
    n_f8 = steps - n_sf8
    mode = ["f8"] * n_f8 + ["sf8"] * n_sf8    # mode[t] for step t (0-based)

    with tile.TileContext(nc) as tc, ExitStack() as top:
        thp = top.enter_context(tc.tile_pool(name="thp", bufs=1))
        pth_t = thp.tile([P, IT], F32)
        nth_t = thp.tile([P, IT], F32)
        nthu_t = thp.tile([P, IT], F32)

        # y(chunk 0) first, then W in half-tiles: phase A starts ~2.5us in and
        # overlaps the remaining W DMA instead of waiting for all of it.
        wp = top.enter_context(tc.tile_pool(name="wp", bufs=1))
        W_t = wp.tile([P, KW, DD], F16, name="W_t")
        yp = top.enter_context(tc.tile_pool(name="yp", bufs=2))
        y_pre = {}
        for c in (0, 1):
            y_t = yp.tile([P, KW, CH], F16, tag="y", name="y_pre")
            y_pre[c] = y_t
        for k in range(KW):
            nc.sync.dma_start(y_pre[0][:, k, :], yT16[ts(k, P), ds(0, CH)])
        for h in range(2):
            for k in range(KW):
                nc.sync.dma_start(W_t[:, k, ts(h, DD // 2)],
                                  W16_d[ts(k, P), ts(h, DD // 2)])
        for k in range(KW):
            nc.sync.dma_start(y_pre[1][:, k, :], yT16[ts(k, P), ds(CH, CH)])

        nc.sync.dma_start(pth_t[:], pth_d.rearrange("(io p) -> p io", p=P))
        nc.sync.dma_start(nth_t[:], nth_d.rearrange("(io p) -> p io", p=P))
        nc.sync.dma_start(nthu_t[:], nthu_d.rearrange("(io p) -> p io", p=P))

        sp = top.enter_context(tc.tile_pool(name="sp", bufs=1))
        S_t = sp.tile([P, IT, 2, DD], F8, name="S_t")   # slot0=lo, slot1=hi
        for j in range(IT):
            nc.sync.dma_start(S_t[:, j, :, :], S8_d[ts(j, P), :, :])

        dxp = top.enter_context(tc.tile_pool(name="dxp", bufs=1))
        Dx_t = dxp.tile([P, IT, 2, DIN], F8, name="Dx_t")   # slot0=lo, slot1=hi
        for io in range(IT):
            nc.sync.dma_start(Dx_t[:, io, :, :], Dx8_d[ts(io, P), :, :])
        v0p = top.enter_context(tc.tile_pool(name="v0p", bufs=2))
        up = top.enter_context(tc.tile_pool(name="up", bufs=4))
        psp = top.enter_context(tc.tile_pool(name="psp", bufs=2, space="PSUM"))
        psap = top.enter_context(tc.tile_pool(name="psap", bufs=2, space="PSUM"))
        pscp = top.enter_context(tc.tile_pool(name="pscp", bufs=2, space="PSUM"))
        vp = top.enter_context(tc.tile_pool(name="vp", bufs=2))
        cp = top.enter_context(tc.tile_pool(name="cp", bufs=2))
        u16p = top.enter_context(tc.tile_pool(name="u16p", bufs=4))
        # NOTE: SBUF is within ~1KB of full; keep pool sizes in sync with budget
        pqp = top.enter_context(tc.tile_pool(name="pqp", bufs=3))
        u32p = top.enter_context(tc.tile_pool(name="u32p", bufs=3))
        stp = top.enter_context(tc.tile_pool(name="stp", bufs=2))

        def shrink_f8(vsrc_quad, q, u8_t):
            """vsrc_quad: [P,4,CH] f16 AP (scaled-32 v). Writes uh into slot 0."""
            c_t = cp.tile([P, GR, CH], F16, tag="c")
            for s in range(GR):
                i = GR * q + s
                nc.gpsimd.tensor_scalar(
                    c_t[:, s, :], vsrc_quad[:, s, :],
                    pth_t[:, i:i + 1], nth_t[:, i:i + 1], MIN, op1=MAX)
            u16_t = u16p.tile([P, GR, CH], F16, tag="u16")
            nc.vector.tensor_tensor(u16_t[:], vsrc_quad, c_t[:], SUB)
            nc.scalar.activation(u8_t[:, GR * q:GR * q + GR, 0, :], u16_t[:],
                                 COPY, scale=1.0 / SC)

        def shrink_sf8(vsrc_quad, q, u8_t):
            """Split-fp8 shrink: uh -> slot0, ul -> slot1 (u8 slots REVERSED vs S8)."""
            for s in range(GR):
                i = GR * q + s
                bias = nthu_t[:, i:i + 1]
                p_t = pqp.tile([P, CH], F32, tag="p")
                q_t = pqp.tile([P, CH], F32, tag="q")
                nc.scalar.activation(p_t[:], vsrc_quad[:, s, :], RELU,
                                     bias=bias, scale=1.0 / SC)
                nc.scalar.activation(q_t[:], vsrc_quad[:, s, :], RELU,
                                     bias=bias, scale=-1.0 / SC)
                u32 = u32p.tile([P, CH], F32, tag="u32")
                nc.vector.tensor_tensor(u32[:], p_t[:], q_t[:], SUB)
                nc.vector.tensor_copy(u8_t[:, i, 0, :], u32[:])
                nc.gpsimd.tensor_tensor(u8_t[:, i, 1, :], u32[:], u8_t[:, i, 0, :], SUB)

        def shrink_af8(vsrc_pair, q, a_t):
            """Final shrink -> SCALED-32 split-fp8 a (ah8 slot0, al8 slot1).
            e4m3(32*a) == 32*e4m3(a) exactly, so phase C just descales by 2^-10."""
            c_t = cp.tile([P, GR, CH], F16, tag="c")
            for s in range(GR):
                i = GR * q + s
                nc.gpsimd.tensor_scalar(
                    c_t[:, s, :], vsrc_pair[:, s, :],
                    pth_t[:, i:i + 1], nth_t[:, i:i + 1], MIN, op1=MAX)
            u16_t = u16p.tile([P, GR, CH], F16, tag="u16")
            nc.vector.tensor_tensor(u16_t[:], vsrc_pair, c_t[:], SUB)
            nc.scalar.activation(a_t[:, GR * q:GR * q + GR, 0, :], u16_t[:],
                                 COPY, scale=1.0)
            nc.gpsimd.tensor_tensor(a_t[:, GR * q:GR * q + GR, 1, :], u16_t[:],
                                    a_t[:, GR * q:GR * q + GR, 0, :], SUB)

        NQ = IT // GR   # shrink groups per step

        def phase_a(c):
            cs = ds(c * CH, CH)
            if c in y_pre:
                y_t = y_pre.pop(c)
            else:
                y_t = yp.tile([P, KW, CH], F16, tag="y")
                for k in range(KW):
                    nc.sync.dma_start(y_t[:, k, :], yT16[ts(k, P), cs])
            v0_t = v0p.tile([P, IT, CH], F16, tag="v0")
            for q in range(NQ):
                ps = psap.tile([P, GR, CH], F32, tag="psA")
                for s in range(GR):
                    i = GR * q + s
                    for k in range(KW):
                        nc.tensor.matmul(ps[:, s, :], W_t[:, k, ts(i, P)],
                                         y_t[:, k, :],
                                         start=(k == 0), stop=(k == KW - 1))
                nc.scalar.activation(v0_t[:, GR * q:GR * q + GR, :], ps[:],
                                     COPY, scale=1.0)
            return v0_t

        def first_shrink(v0_t, u8_t, fmt):
            for q in range(NQ):
                if fmt == "f8":
                    shrink_f8(v0_t[:, GR * q:GR * q + GR, :], q, u8_t)
                else:
                    shrink_sf8(v0_t[:, GR * q:GR * q + GR, :], q, u8_t)

        def step_pair(t, pair, v0s, u8s, u8ns, ats):
            """One B step for both chunks of the pair (chunk-sequential)."""
            m = mode[t]
            last = t == steps - 1
            nxt = None if last else mode[t + 1]
            for c in pair:
              for qq in range(IT // GRP):
                v0_t, u8_t = v0s[c], u8s[c]
                u8_n = None if last else u8ns[c]
                a_t = ats[c] if last else None
                ps = psp.tile([P, GRP, CH], F32, tag="ps")
                for s in range(GRP):
                    i = GRP * qq + s
                    if m == "f8":
                        n_mm = JP
                        for jp in range(JP):
                            nc.tensor.matmul(
                                ps[:, s, :],
                                S_t[:, 2 * jp:2 * jp + 2, 1, ts(i, P)],
                                u8_t[:, 2 * jp:2 * jp + 2, 0, :],
                                start=(jp == 0), stop=(jp == n_mm - 1),
                                perf_mode=DR)
                    else:
                        n_mm = JP + IT
                        k = 0
                        for jp in range(JP):
                            nc.tensor.matmul(
                                ps[:, s, :],
                                S_t[:, 2 * jp:2 * jp + 2, 1, ts(i, P)],
                                u8_t[:, 2 * jp:2 * jp + 2, 0, :],
                                start=(k == 0), stop=(k == n_mm - 1),
                                perf_mode=DR)
                            k += 1
                        for j in range(IT):
                            nc.tensor.matmul(
                                ps[:, s, :],
                                S_t[:, j, :, ts(i, P)],
                                u8_t[:, j, :, :],
                                start=False, stop=(k == n_mm - 1),
                                perf_mode=DR)
                            k += 1
                v_t = vp.tile([P, GRP, CH], F16, tag="v")
                nc.vector.tensor_tensor(v_t[:], ps[:],
                                        v0_t[:, GRP * qq:GRP * qq + GRP, :], ADD)
                for h in range(GRP // GR):
                    q = (GRP // GR) * qq + h
                    v_pair = v_t[:, GR * h:GR * h + GR, :]
                    if last:
                        shrink_af8(v_pair, q, a_t)
                    elif nxt == "f8":
                        shrink_f8(v_pair, q, u8_n)
                    else:
                        shrink_sf8(v_pair, q, u8_n)

        def phase_c(c, a_t):
            # out = (ah @ (DxH + DxL) + al @ DxH) / 32, all DoubleRow fp8
            cs_base = c * CH
            for bt in range(CH // P):
                for dn in range(DIN // CN):
                    ps = pscp.tile([P, CN], F32, tag="psC")
                    n_mm = JP + IT
                    k = 0
                    for jp in range(JP):
                        nc.tensor.matmul(
                            ps[:], a_t[:, 2 * jp:2 * jp + 2, 0, ts(bt, P)],
                            Dx_t[:, 2 * jp:2 * jp + 2, 1, ts(dn, CN)],
                            start=(k == 0), stop=(k == n_mm - 1), perf_mode=DR)
                        k += 1
                    for j in range(IT):
                        nc.tensor.matmul(
                            ps[:], a_t[:, j, :, ts(bt, P)],
                            Dx_t[:, j, :, ts(dn, CN)],
                            start=False, stop=(k == n_mm - 1), perf_mode=DR)
                        k += 1
                    st = stp.tile([P, CN], F32, tag="st")
                    nc.scalar.activation(st[:], ps[:], COPY, scale=1.0 / (SC * SC))
                    nc.sync.dma_start(out_d[ds(cs_base + bt * P, P), ts(dn, CN)],
                                      st[:])

        # ---- main schedule: chunk pairs; previous pair's C is emitted after
        # the next pair's A so its matmuls fill the shrink-chain latency ----
        pending_c = []
        for cp0 in range(0, NCH, 2):
            pair = (cp0, cp0 + 1)
            v0s, u8s, ats = {}, {}, {}
            for c in pair:
                v0s[c] = phase_a(c)
                u8s[c] = up.tile([P, IT, 2, CH], F8, tag="u8", name="u8_t")
                first_shrink(v0s[c], u8s[c], mode[0])
                ats[c] = None
            for nxt_c in (cp0 + 2, cp0 + 3):
                if nxt_c < NCH and nxt_c not in y_pre:
                    y_t = yp.tile([P, KW, CH], F16, tag="y", name="y_nxt")
                    for k in range(KW):
                        nc.sync.dma_start(y_t[:, k, :],
                                          yT16[ts(k, P), ds(nxt_c * CH, CH)])
                    y_pre[nxt_c] = y_t
            for c, a_t in pending_c:
                phase_c(c, a_t)
            pending_c = []
            for t in range(steps):
                last = t == steps - 1
                u8ns = {}
                for c in pair:
                    if last:
                        u8ns[c] = None
                        ats[c] = up.tile([P, IT, 2, CH], F8, tag="u8", name="a_t")
                    else:
                        u8ns[c] = up.tile([P, IT, 2, CH], F8, tag="u8", name="u8_n")
                step_pair(t, pair, v0s, u8s, u8ns, ats)
                for c in pair:
                    u8s[c] = u8ns[c]
            pending_c = [(c, ats[c]) for c in pair]
        for c, a_t in pending_c:
            phase_c(c, a_t)

    nc.compile()
    return nc


def _prep_in_maps(y, W, Theta, S, Dx):
    import ml_dtypes
    E4 = ml_dtypes.float8_e4m3  # TRN flavor (max normal 240)

    y = np.asarray(y, dtype=np.float32)
    W = np.asarray(W, dtype=np.float32)
    Theta = np.asarray(Theta, dtype=np.float32)
    S = np.asarray(S, dtype=np.float32)
    Dx = np.asarray(Dx, dtype=np.float32)
    assert y.shape == (B_FULL, DIN) and W.shape == (DIN, DD)
    assert S.shape == (DD, DD) and Dx.shape == (DD, DIN)

    th = np.maximum(Theta, 0.0) + np.float32(1e-7)
    W16 = (SC * W).astype(np.float16)
    Ss = np.float32(SC) * S
    S8H = np.clip(Ss, -240, 240).astype(E4)
    S8L = np.clip(Ss - S8H.astype(np.float32), -240, 240).astype(E4)
    S8 = np.stack([S8L, S8H], axis=1)         # [DD, 2, DD], slot0=lo slot1=hi
    Dxs = np.float32(SC) * Dx
    Dx8H = np.clip(Dxs, -240, 240).astype(E4)
    Dx8L = np.clip(Dxs - Dx8H.astype(np.float32), -240, 240).astype(E4)
    Dx8 = np.ascontiguousarray(np.stack([Dx8L, Dx8H], axis=1))
    yT16 = np.ascontiguousarray(y.T).astype(np.float16)

    shared = dict(
        W16=W16, S8=np.ascontiguousarray(S8), Dx8=Dx8,
        pth=(SC * th).astype(np.float32),
        nth=(-SC * th).astype(np.float32),
        nthu=(-th).astype(np.float32),
    )
    in_maps = []
    for c in range(NCORES):
        sl = slice(c * BSH, (c + 1) * BSH)
        in_maps.append(dict(shared, yT16=np.ascontiguousarray(yT16[:, sl])))
    return in_maps


_sharded_cache = {}


def _get_sharded(steps: int):
    """Build (once) the jitted shard_map executable for the compiled NEFF."""
    if steps in _sharded_cache:
        return _sharded_cache[steps]
    import jax
    from jax.experimental.shard_map import shard_map
    from jax.sharding import Mesh, PartitionSpec
    from concourse import bass2jax

    if steps not in _built:
        _built[steps] = _build(steps)
    nc = _built[steps]
    bass2jax.install_neuronx_cc_hook()
    assert nc.dbg_addr is None
    partition_name = nc.partition_id_tensor.name if nc.partition_id_tensor else None

    in_names, out_names, out_avals, zero_shapes = [], [], [], []
    for alloc in nc.m.functions[0].allocations:
        if not isinstance(alloc, mybir.MemoryLocationSet):
            continue
        name = alloc.memorylocations[0].name
        if alloc.kind == "ExternalInput":
            if name != partition_name:
                in_names.append(name)
        elif alloc.kind == "ExternalOutput":
            out_names.append(name)
            shape = tuple(alloc.tensor_shape)
            dtype = mybir.dt.np(alloc.dtype)
            out_avals.append(jax.core.ShapedArray(shape, dtype))
            zero_shapes.append((shape, dtype))
    n_params = len(in_names)
    n_outs = len(out_names)
    all_in_names = in_names + out_names
    if partition_name is not None:
        all_in_names.append(partition_name)

    def _body(*args):
        operands = list(args)
        if partition_name is not None:
            operands.append(bass2jax.partition_id_tensor())
        outs = bass2jax._bass_exec_p.bind(
            *operands,
            out_avals=tuple(out_avals),
            in_names=tuple(all_in_names),
            out_names=tuple(out_names),
            lowering_input_output_aliases=(),
            sim_require_finite=True,
            sim_require_nnan=True,
            nc=nc,
        )
        return tuple(outs)

    devices = jax.devices()[:NCORES]
    mesh = Mesh(np.asarray(devices), ("core",))
    donate = tuple(range(n_params, n_params + n_outs))
    sharded = jax.jit(
        shard_map(_body, mesh=mesh,
                  in_specs=(PartitionSpec("core"),) * (n_params + n_outs),
                  out_specs=(PartitionSpec("core"),) * n_outs,
                  check_rep=False),
        donate_argnums=donate, keep_unused=True)
    entry = dict(sharded=sharded, in_names=in_names, out_names=out_names,
                 zero_shapes=zero_shapes, mesh=mesh, n_params=n_params)
    _sharded_cache[steps] = entry
    return entry


def _concat_inputs(entry, in_maps):
    return [np.concatenate([np.asarray(in_maps[c][n]) for c in range(NCORES)], axis=0)
            for n in entry["in_names"]]


def _run(entry, concat_in):
    zeros = [np.zeros((NCORES * s[0], *s[1:]), d) for s, d in entry["zero_shapes"]]
    out_arrs = entry["sharded"](*concat_in, *zeros)
    return out_arrs


def kernel(y, W, Theta, S, Dx, unroll_steps):
    steps = int(unroll_steps)
    entry = _get_sharded(steps)
    in_maps = _prep_in_maps(y, W, Theta, S, Dx)
    out_arrs = _run(entry, _concat_inputs(entry, in_maps))
    idx = entry["out_names"].index("out")
    return np.ascontiguousarray(np.asarray(out_arrs[idx]))  # [NCORES*BSH, DIN]


def time_kernel(np_inputs, iters=6):
    """Steady-state wall time per NEFF execution (ns), device-resident inputs."""
    import jax
    from jax.sharding import NamedSharding, PartitionSpec
    steps = int(np_inputs["unroll_steps"])
    entry = _get_sharded(steps)
    in_maps = _prep_in_maps(np_inputs["y"], np_inputs["W"], np_inputs["Theta"],
                            np_inputs["S"], np_inputs["Dx"])
    concat_in = _concat_inputs(entry, in_maps)
    sh = NamedSharding(entry["mesh"], PartitionSpec("core"))
    dev_in = [jax.device_put(a, sh) for a in concat_in]
    import time as _time
    times = []
    for it in range(iters):
        zeros = [jax.device_put(np.zeros((NCORES * s[0], *s[1:]), d), sh)
                 for s, d in entry["zero_shapes"]]
        for z in zeros:
            z.block_until_ready()
        t0 = _time.perf_counter()
        outs = entry["sharded"](*dev_in, *zeros)
        for o in outs:
            o.block_until_ready()
        times.append(_time.perf_counter() - t0)
    best = min(times[1:]) if len(times) > 1 else times[0]
    print("  per-iter times (ms):", [f"{t*1e3:.1f}" for t in times])
    return best * 1e9


if __name__ == "__main__":
    rng = np.random.default_rng(0)
    inputs = dict(
        y=rng.standard_normal((B_FULL, DIN), dtype=np.float32),
        W=(rng.standard_normal((DIN, DD)) * 0.02).astype(np.float32),
        Theta=rng.random(DD, dtype=np.float32),
        S=(rng.standard_normal((DD, DD)) * 0.02).astype(np.float32),
        Dx=(rng.standard_normal((DD, DIN)) * 0.02).astype(np.float32),
        unroll_steps=16,
    )
    out = kernel(**inputs)
    print("out", out.shape, out.dtype, np.abs(out).max())


# revision 51
# speedup vs baseline: 4.7520x; 1.0004x over previous
"""Trainium2 Bass kernel for a LISTA layer (nn_ListaLayer).

Reference computation (jax, fp32):
    th = relu(Theta) + 1e-7
    xW = (y @ W) / th
    repeat 16: z = xW + (unit_threshold(z) * th @ S) / th
    out = (unit_threshold(z) * th) @ Dx
where unit_threshold(v) = sign(v) * relu(|v| - 1).

Algebraic restructure (exact): track v = z * th:
    v0 = y @ W
    repeat 16:  u = soft_threshold(v, th) = sign(v) * relu(|v| - th)
                v = v0 + u @ S
    out = soft_threshold(v, th) @ Dx

Distribution: data-parallel over batch rows, 8 NeuronCores, 2048 rows each.
W/th/S/Dx replicated; no collectives.

Numerics / performance scheme (v-space carried SCALED by 32 in fp16):
  - A:  v~0 = y16 @ f16(32*W)   (fp16 matmul, fp32 PSUM)
  - B:  16 soft-threshold + u@S steps, all matmuls fp8-e4m3 DoubleRow
        (2 dict-tiles contracted per pass). First 12 steps single-pass
        (u8 @ S8H); last NSF8=4 steps "split-fp8": uh8 @ (S8H+S8L) +
        ul8 @ S8H, the cross terms folded into per-j DoubleRow matmuls by
        pairing weight slots (S8L[j],S8H[j]) against moving slots
        (uh[j],ul[j]) - DoubleRow multiplies same-index slots, so the u8
        tile stores (uh, ul) while the S tile stores (lo, hi).
        S8H = e4m3(32*S), S8L = e4m3(32*S - S8H); u is consumed UNSCALED
        (ACT applies the 2^-5 descale when emitting fp8), so psum comes out
        scaled 32 and adds directly onto v~0.
        f8-step shrink per pair of dict tiles: DVE add (psum+v~0 -> f16),
        GPSIMD fused clamp (tensor_scalar min/max, per-partition +-32*th),
        DVE sub (f16 2x), ACT copy->fp8 (scale 2^-5).
  - C:  out = (ah8 @ (DxH+DxL) + al8 @ DxH) * 2^-10, DoubleRow fp8 with the
        final shrink emitting a scaled-32 hi/lo split (exact: e4m3(32a) ==
        32*e4m3(a)); the 2^-10 descale rides the PSUM->SBUF ACT copy.
All phases fused per 256-column batch chunk; two chunks interleaved so the
tensor engine never waits on a shrink chain, and each pair's C matmuls are
deferred until after the next pair's A phase to fill the final-shrink
latency. Phase A and C have dedicated PSUM pools so the B-step psum ring
never blocks them.
"""

import numpy as np
from contextlib import ExitStack

import concourse.bass as bass
import concourse.bacc as bacc
import concourse.tile as tile
import concourse.mybir as mybir
from concourse.bass import ts, ds

P = 128
NCORES = 8
B_FULL, DIN, DD = 16384, 1024, 2048
BSH = B_FULL // NCORES      # 2048 batch rows per core
CH = 256                    # batch columns per chunk
NCH = BSH // CH             # 8 chunks
IT = DD // P                # 16 dict tiles
JP = IT // 2                # 8 DoubleRow pairs
KW = DIN // P               # 8 d_in tiles
CN = 512                    # free dim of phase-C matmuls
SC = 32.0                   # global scale 2^5
NSF8 = 4                    # trailing split-fp8 steps (rest single fp8)
GR = 2                      # dict tiles per shrink group
GRP = 4                     # dict tiles per psum/add group

F8 = mybir.dt.float8e4
F16 = mybir.dt.float16
F32 = mybir.dt.float32
ADD = mybir.AluOpType.add
SUB = mybir.AluOpType.subtract
MIN = mybir.AluOpType.min
MAX = mybir.AluOpType.max
RELU = mybir.ActivationFunctionType.Relu
COPY = mybir.ActivationFunctionType.Copy
DR = mybir.MatmulPerfMode.DoubleRow

_built = {}


def _build(steps: int):
    nc = bacc.Bacc("TRN2", target_bir_lowering=False, debug=False, num_devices=NCORES)

    def inp(name, shape, dt):
        return nc.dram_tensor(name, shape, dt, kind="ExternalInput").ap()

    yT16 = inp("yT16", (DIN, BSH), F16)
    W16_d = inp("W16", (DIN, DD), F16)        # f16(32*W)
    S8_d = inp("S8", (DD, 2, DD), F8)         # [j, (lo,hi), :] interleaved
    Dx8_d = inp("Dx8", (DD, 2, DIN), F8)        # [j, (lo,hi), :] * 32
    pth_d = inp("pth", (DD,), F32)            # +32*th
    nth_d = inp("nth", (DD,), F32)            # -32*th
    nthu_d = inp("nthu", (DD,), F32)          # -th (unscaled)
    out_d = nc.dram_tensor("out", (BSH, DIN), F32, kind="ExternalOutput").ap()

    n_sf8 = min(NSF8, steps)
    n_f8 = steps - n_sf8
    mode = ["f8"] * n_f8 + ["sf8"] * n_sf8    # mode[t] for step t (0-based)

    with tile.TileContext(nc) as tc, ExitStack() as top:
        thp = top.enter_context(tc.tile_pool(name="thp", bufs=1))
        pth_t = thp.tile([P, IT], F32)
        nth_t = thp.tile([P, IT], F32)
        nthu_t = thp.tile([P, IT], F32)

        # y(chunk 0) first, then W in half-tiles: phase A starts ~2.5us in and
        # overlaps the remaining W DMA instead of waiting for all of it.
        wp = top.enter_context(tc.tile_pool(name="wp", bufs=1))
        W_t = wp.tile([P, KW, DD], F16, name="W_t")
        yp = top.enter_context(tc.tile_pool(name="yp", bufs=2))
        y_pre = {}
        for c in (0, 1):
            y_t = yp.tile([P, KW, CH], F16, tag="y", name="y_pre")
            y_pre[c] = y_t
        for k in range(KW):
            nc.sync.dma_start(y_pre[0][:, k, :], yT16[ts(k, P), ds(0, CH)])
        for h in range(2):
            for k in range(KW):
                nc.sync.dma_start(W_t[:, k, ts(h, DD // 2)],
                                  W16_d[ts(k, P), ts(h, DD // 2)])
        for k in range(KW):
            nc.sync.dma_start(y_pre[1][:, k, :], yT16[ts(k, P), ds(CH, CH)])

        nc.sync.dma_start(pth_t[:], pth_d.rearrange("(io p) -> p io", p=P))
        nc.sync.dma_start(nth_t[:], nth_d.rearrange("(io p) -> p io", p=P))
        nc.sync.dma_start(nthu_t[:], nthu_d.rearrange("(io p) -> p io", p=P))

        sp = top.enter_context(tc.tile_pool(name="sp", bufs=1))
        S_t = sp.tile([P, IT, 2, DD], F8, name="S_t")   # slot0=lo, slot1=hi
        for j in range(IT):
            nc.sync.dma_start(S_t[:, j, :, :], S8_d[ts(j, P), :, :])

        dxp = top.enter_context(tc.tile_pool(name="dxp", bufs=1))
        Dx_t = dxp.tile([P, IT, 2, DIN], F8, name="Dx_t")   # slot0=lo, slot1=hi
        for io in range(IT):
            nc.sync.dma_start(Dx_t[:, io, :, :], Dx8_d[ts(io, P), :, :])
        v0p = top.enter_context(tc.tile_pool(name="v0p", bufs=2))
        up = top.enter_context(tc.tile_pool(name="up", bufs=4))
        psp = top.enter_context(tc.tile_pool(name="psp", bufs=2, space="PSUM"))
        psap = top.enter_context(tc.tile_pool(name="psap", bufs=2, space="PSUM"))
        pscp = top.enter_context(tc.tile_pool(name="pscp", bufs=2, space="PSUM"))
        vp = top.enter_context(tc.tile_pool(name="vp", bufs=2))
        cp = top.enter_context(tc.tile_pool(name="cp", bufs=2))
        u16p = top.enter_context(tc.tile_pool(name="u16p", bufs=3))
        # NOTE: SBUF is within ~1KB of full; keep pool sizes in sync with budget
        pqp = top.enter_context(tc.tile_pool(name="pqp", bufs=3))
        u32p = top.enter_context(tc.tile_pool(name="u32p", bufs=4))
        stp = top.enter_context(tc.tile_pool(name="stp", bufs=2))

        def shrink_f8(vsrc_quad, q, u8_t):
            """vsrc_quad: [P,4,CH] f16 AP (scaled-32 v). Writes uh into slot 0."""
            c_t = cp.tile([P, GR, CH], F16, tag="c")
            for s in range(GR):
                i = GR * q + s
                nc.gpsimd.tensor_scalar(
                    c_t[:, s, :], vsrc_quad[:, s, :],
                    pth_t[:, i:i + 1], nth_t[:, i:i + 1], MIN, op1=MAX)
            u16_t = u16p.tile([P, GR, CH], F16, tag="u16")
            nc.vector.tensor_tensor(u16_t[:], vsrc_quad, c_t[:], SUB)
            nc.scalar.activation(u8_t[:, GR * q:GR * q + GR, 0, :], u16_t[:],
                                 COPY, scale=1.0 / SC)

        def shrink_sf8(vsrc_quad, q, u8_t):
            """Split-fp8 shrink: uh -> slot0, ul -> slot1 (u8 slots REVERSED vs S8)."""
            for s in range(GR):
                i = GR * q + s
                bias = nthu_t[:, i:i + 1]
                p_t = pqp.tile([P, CH], F32, tag="p")
                q_t = pqp.tile([P, CH], F32, tag="q")
                nc.scalar.activation(p_t[:], vsrc_quad[:, s, :], RELU,
                                     bias=bias, scale=1.0 / SC)
                nc.scalar.activation(q_t[:], vsrc_quad[:, s, :], RELU,
                                     bias=bias, scale=-1.0 / SC)
                u32 = u32p.tile([P, CH], F32, tag="u32")
                nc.vector.tensor_tensor(u32[:], p_t[:], q_t[:], SUB)
                nc.vector.tensor_copy(u8_t[:, i, 0, :], u32[:])
                nc.gpsimd.tensor_tensor(u8_t[:, i, 1, :], u32[:], u8_t[:, i, 0, :], SUB)

        def shrink_af8(vsrc_pair, q, a_t):
            """Final shrink -> SCALED-32 split-fp8 a (ah8 slot0, al8 slot1).
            e4m3(32*a) == 32*e4m3(a) exactly, so phase C just descales by 2^-10."""
            c_t = cp.tile([P, GR, CH], F16, tag="c")
            for s in range(GR):
                i = GR * q + s
                nc.gpsimd.tensor_scalar(
                    c_t[:, s, :], vsrc_pair[:, s, :],
                    pth_t[:, i:i + 1], nth_t[:, i:i + 1], MIN, op1=MAX)
            u16_t = u16p.tile([P, GR, CH], F16, tag="u16")
            nc.vector.tensor_tensor(u16_t[:], vsrc_pair, c_t[:], SUB)
            nc.scalar.activation(a_t[:, GR * q:GR * q + GR, 0, :], u16_t[:],
                                 COPY, scale=1.0)
            nc.gpsimd.tensor_tensor(a_t[:, GR * q:GR * q + GR, 1, :], u16_t[:],
                                    a_t[:, GR * q:GR * q + GR, 0, :], SUB)

        NQ = IT // GR   # shrink groups per step

        def phase_a(c):
            cs = ds(c * CH, CH)
            if c in y_pre:
                y_t = y_pre.pop(c)
            else:
                y_t = yp.tile([P, KW, CH], F16, tag="y")
                for k in range(KW):
                    nc.sync.dma_start(y_t[:, k, :], yT16[ts(k, P), cs])
            v0_t = v0p.tile([P, IT, CH], F16, tag="v0")
            for q in range(NQ):
                ps = psap.tile([P, GR, CH], F32, tag="psA")
                for s in range(GR):
                    i = GR * q + s
                    for k in range(KW):
                        nc.tensor.matmul(ps[:, s, :], W_t[:, k, ts(i, P)],
                                         y_t[:, k, :],
                                         start=(k == 0), stop=(k == KW - 1))
                nc.scalar.activation(v0_t[:, GR * q:GR * q + GR, :], ps[:],
                                     COPY, scale=1.0)
            return v0_t

        def first_shrink(v0_t, u8_t, fmt):
            for q in range(NQ):
                if fmt == "f8":
                    shrink_f8(v0_t[:, GR * q:GR * q + GR, :], q, u8_t)
                else:
                    shrink_sf8(v0_t[:, GR * q:GR * q + GR, :], q, u8_t)

        def step_pair(t, pair, v0s, u8s, u8ns, ats):
            """One B step for both chunks of the pair (chunk-sequential)."""
            m = mode[t]
            last = t == steps - 1
            nxt = None if last else mode[t + 1]
            for c in pair:
              for qq in range(IT // GRP):
                v0_t, u8_t = v0s[c], u8s[c]
                u8_n = None if last else u8ns[c]
                a_t = ats[c] if last else None
                ps = psp.tile([P, GRP, CH], F32, tag="ps")
                for s in range(GRP):
                    i = GRP * qq + s
                    if m == "f8":
                        n_mm = JP
                        for jp in range(JP):
                            nc.tensor.matmul(
                                ps[:, s, :],
                                S_t[:, 2 * jp:2 * jp + 2, 1, ts(i, P)],
                                u8_t[:, 2 * jp:2 * jp + 2, 0, :],
                                start=(jp == 0), stop=(jp == n_mm - 1),
                                perf_mode=DR)
                    else:
                        n_mm = JP + IT
                        k = 0
                        for jp in range(JP):
                            nc.tensor.matmul(
                                ps[:, s, :],
                                S_t[:, 2 * jp:2 * jp + 2, 1, ts(i, P)],
                                u8_t[:, 2 * jp:2 * jp + 2, 0, :],
                                start=(k == 0), stop=(k == n_mm - 1),
                                perf_mode=DR)
                            k += 1
                        for j in range(IT):
                            nc.tensor.matmul(
                                ps[:, s, :],
                                S_t[:, j, :, ts(i, P)],
                                u8_t[:, j, :, :],
                                start=False, stop=(k == n_mm - 1),
                                perf_mode=DR)
                            k += 1
                v_t = vp.tile([P, GRP, CH], F16, tag="v")
                nc.vector.tensor_tensor(v_t[:], ps[:],
                                        v0_t[:, GRP * qq:GRP * qq + GRP, :], ADD)
                for h in range(GRP // GR):
                    q = (GRP // GR) * qq + h
                    v_pair = v_t[:, GR * h:GR * h + GR, :]
                    if last:
                        shrink_af8(v_pair, q, a_t)
                    elif nxt == "f8":
                        shrink_f8(v_pair, q, u8_n)
                    else:
                        shrink_sf8(v_pair, q, u8_n)

        def phase_c(c, a_t):
            # out = (ah @ (DxH + DxL) + al @ DxH) / 32, all DoubleRow fp8
            cs_base = c * CH
            for bt in range(CH // P):
                for dn in range(DIN // CN):
                    ps = pscp.tile([P, CN], F32, tag="psC")
                    n_mm = JP + IT
                    k = 0
                    for jp in range(JP):
                        nc.tensor.matmul(
                            ps[:], a_t[:, 2 * jp:2 * jp + 2, 0, ts(bt, P)],
                            Dx_t[:, 2 * jp:2 * jp + 2, 1, ts(dn, CN)],
                            start=(k == 0), stop=(k == n_mm - 1), perf_mode=DR)
                        k += 1
                    for j in range(IT):
                        nc.tensor.matmul(
                            ps[:], a_t[:, j, :, ts(bt, P)],
                            Dx_t[:, j, :, ts(dn, CN)],
                            start=False, stop=(k == n_mm - 1), perf_mode=DR)
                        k += 1
                    st = stp.tile([P, CN], F32, tag="st")
                    nc.scalar.activation(st[:], ps[:], COPY, scale=1.0 / (SC * SC))
                    nc.sync.dma_start(out_d[ds(cs_base + bt * P, P), ts(dn, CN)],
                                      st[:])

        # ---- main schedule: chunk pairs; previous pair's C is emitted after
        # the next pair's A so its matmuls fill the shrink-chain latency ----
        pending_c = []
        for cp0 in range(0, NCH, 2):
            pair = (cp0, cp0 + 1)
            v0s, u8s, ats = {}, {}, {}
            for c in pair:
                v0s[c] = phase_a(c)
                u8s[c] = up.tile([P, IT, 2, CH], F8, tag="u8", name="u8_t")
                first_shrink(v0s[c], u8s[c], mode[0])
                ats[c] = None
            for nxt_c in (cp0 + 2, cp0 + 3):
                if nxt_c < NCH and nxt_c not in y_pre:
                    y_t = yp.tile([P, KW, CH], F16, tag="y", name="y_nxt")
                    for k in range(KW):
                        nc.sync.dma_start(y_t[:, k, :],
                                          yT16[ts(k, P), ds(nxt_c * CH, CH)])
                    y_pre[nxt_c] = y_t
            for c, a_t in pending_c:
                phase_c(c, a_t)
            pending_c = []
            for t in range(steps):
                last = t == steps - 1
                u8ns = {}
                for c in pair:
                    if last:
                        u8ns[c] = None
                        ats[c] = up.tile([P, IT, 2, CH], F8, tag="u8", name="a_t")
                    else:
                        u8ns[c] = up.tile([P, IT, 2, CH], F8, tag="u8", name="u8_n")
                step_pair(t, pair, v0s, u8s, u8ns, ats)
                for c in pair:
                    u8s[c] = u8ns[c]
            pending_c = [(c, ats[c]) for c in pair]
        for c, a_t in pending_c:
            phase_c(c, a_t)

    nc.compile()
    return nc


def _prep_in_maps(y, W, Theta, S, Dx):
    import ml_dtypes
    E4 = ml_dtypes.float8_e4m3  # TRN flavor (max normal 240)

    y = np.asarray(y, dtype=np.float32)
    W = np.asarray(W, dtype=np.float32)
    Theta = np.asarray(Theta, dtype=np.float32)
    S = np.asarray(S, dtype=np.float32)
    Dx = np.asarray(Dx, dtype=np.float32)
    assert y.shape == (B_FULL, DIN) and W.shape == (DIN, DD)
    assert S.shape == (DD, DD) and Dx.shape == (DD, DIN)

    th = np.maximum(Theta, 0.0) + np.float32(1e-7)
    W16 = (SC * W).astype(np.float16)
    Ss = np.float32(SC) * S
    S8H = np.clip(Ss, -240, 240).astype(E4)
    S8L = np.clip(Ss - S8H.astype(np.float32), -240, 240).astype(E4)
    S8 = np.stack([S8L, S8H], axis=1)         # [DD, 2, DD], slot0=lo slot1=hi
    Dxs = np.float32(SC) * Dx
    Dx8H = np.clip(Dxs, -240, 240).astype(E4)
    Dx8L = np.clip(Dxs - Dx8H.astype(np.float32), -240, 240).astype(E4)
    Dx8 = np.ascontiguousarray(np.stack([Dx8L, Dx8H], axis=1))
    yT16 = np.ascontiguousarray(y.T).astype(np.float16)

    shared = dict(
        W16=W16, S8=np.ascontiguousarray(S8), Dx8=Dx8,
        pth=(SC * th).astype(np.float32),
        nth=(-SC * th).astype(np.float32),
        nthu=(-th).astype(np.float32),
    )
    in_maps = []
    for c in range(NCORES):
        sl = slice(c * BSH, (c + 1) * BSH)
        in_maps.append(dict(shared, yT16=np.ascontiguousarray(yT16[:, sl])))
    return in_maps


_sharded_cache = {}


def _get_sharded(steps: int):
    """Build (once) the jitted shard_map executable for the compiled NEFF."""
    if steps in _sharded_cache:
        return _sharded_cache[steps]
    import jax
    from jax.experimental.shard_map import shard_map
    from jax.sharding import Mesh, PartitionSpec
    from concourse import bass2jax

    if steps not in _built:
        _built[steps] = _build(steps)
    nc = _built[steps]
    bass2jax.install_neuronx_cc_hook()
    assert nc.dbg_addr is None
    partition_name = nc.partition_id_tensor.name if nc.partition_id_tensor else None

    in_names, out_names, out_avals, zero_shapes = [], [], [], []
    for alloc in nc.m.functions[0].allocations:
        if not isinstance(alloc, mybir.MemoryLocationSet):
            continue
        name = alloc.memorylocations[0].name
        if alloc.kind == "ExternalInput":
            if name != partition_name:
                in_names.append(name)
        elif alloc.kind == "ExternalOutput":
            out_names.append(name)
            shape = tuple(alloc.tensor_shape)
            dtype = mybir.dt.np(alloc.dtype)
            out_avals.append(jax.core.ShapedArray(shape, dtype))
            zero_shapes.append((shape, dtype))
    n_params = len(in_names)
    n_outs = len(out_names)
    all_in_names = in_names + out_names
    if partition_name is not None:
        all_in_names.append(partition_name)

    def _body(*args):
        operands = list(args)
        if partition_name is not None:
            operands.append(bass2jax.partition_id_tensor())
        outs = bass2jax._bass_exec_p.bind(
            *operands,
            out_avals=tuple(out_avals),
            in_names=tuple(all_in_names),
            out_names=tuple(out_names),
            lowering_input_output_aliases=(),
            sim_require_finite=True,
            sim_require_nnan=True,
            nc=nc,
        )
        return tuple(outs)

    devices = jax.devices()[:NCORES]
    mesh = Mesh(np.asarray(devices), ("core",))
    donate = tuple(range(n_params, n_params + n_outs))
    sharded = jax.jit(
        shard_map(_body, mesh=mesh,
                  in_specs=(PartitionSpec("core"),) * (n_params + n_outs),
                  out_specs=(PartitionSpec("core"),) * n_outs,
                  check_rep=False),
        donate_argnums=donate, keep_unused=True)
    entry = dict(sharded=sharded, in_names=in_names, out_names=out_names,
                 zero_shapes=zero_shapes, mesh=mesh, n_params=n_params)
    _sharded_cache[steps] = entry
    return entry


def _concat_inputs(entry, in_maps):
    return [np.concatenate([np.asarray(in_maps[c][n]) for c in range(NCORES)], axis=0)
            for n in entry["in_names"]]


def _run(entry, concat_in):
    zeros = [np.zeros((NCORES * s[0], *s[1:]), d) for s, d in entry["zero_shapes"]]
    out_arrs = entry["sharded"](*concat_in, *zeros)
    return out_arrs


def kernel(y, W, Theta, S, Dx, unroll_steps):
    steps = int(unroll_steps)
    entry = _get_sharded(steps)
    in_maps = _prep_in_maps(y, W, Theta, S, Dx)
    out_arrs = _run(entry, _concat_inputs(entry, in_maps))
    idx = entry["out_names"].index("out")
    return np.ascontiguousarray(np.asarray(out_arrs[idx]))  # [NCORES*BSH, DIN]


def time_kernel(np_inputs, iters=6):
    """Steady-state wall time per NEFF execution (ns), device-resident inputs."""
    import jax
    from jax.sharding import NamedSharding, PartitionSpec
    steps = int(np_inputs["unroll_steps"])
    entry = _get_sharded(steps)
    in_maps = _prep_in_maps(np_inputs["y"], np_inputs["W"], np_inputs["Theta"],
                            np_inputs["S"], np_inputs["Dx"])
    concat_in = _concat_inputs(entry, in_maps)
    sh = NamedSharding(entry["mesh"], PartitionSpec("core"))
    dev_in = [jax.device_put(a, sh) for a in concat_in]
    import time as _time
    times = []
    for it in range(iters):
        zeros = [jax.device_put(np.zeros((NCORES * s[0], *s[1:]), d), sh)
                 for s, d in entry["zero_shapes"]]
        for z in zeros:
            z.block_until_ready()
        t0 = _time.perf_counter()
        outs = entry["sharded"](*dev_in, *zeros)
        for o in outs:
            o.block_until_ready()
        times.append(_time.perf_counter() - t0)
    best = min(times[1:]) if len(times) > 1 else times[0]
    print("  per-iter times (ms):", [f"{t*1e3:.1f}" for t in times])
    return best * 1e9


if __name__ == "__main__":
    rng = np.random.default_rng(0)
    inputs = dict(
        y=rng.standard_normal((B_FULL, DIN), dtype=np.float32),
        W=(rng.standard_normal((DIN, DD)) * 0.02).astype(np.float32),
        Theta=rng.random(DD, dtype=np.float32),
        S=(rng.standard_normal((DD, DD)) * 0.02).astype(np.float32),
        Dx=(rng.standard_normal((DD, DIN)) * 0.02).astype(np.float32),
        unroll_steps=16,
    )
    out = kernel(**inputs)
    print("out", out.shape, out.dtype, np.abs(out).max())


# revision 57
# speedup vs baseline: 4.9157x; 1.0344x over previous
"""Trainium2 Bass kernel for a LISTA layer (nn_ListaLayer).

Reference computation (jax, fp32):
    th = relu(Theta) + 1e-7
    xW = (y @ W) / th
    repeat 16: z = xW + (unit_threshold(z) * th @ S) / th
    out = (unit_threshold(z) * th) @ Dx
where unit_threshold(v) = sign(v) * relu(|v| - 1).

Algebraic restructure (exact): track v = z * th:
    v0 = y @ W
    repeat 16:  u = soft_threshold(v, th) = sign(v) * relu(|v| - th)
                v = v0 + u @ S
    out = soft_threshold(v, th) @ Dx

Distribution: data-parallel over batch rows, 8 NeuronCores, 2048 rows each.
W/th/S/Dx replicated; no collectives.

Numerics / performance scheme (v-space carried SCALED by 32 in fp16):
  - A:  v~0 = y16 @ f16(32*W)   (fp16 matmul, fp32 PSUM)
  - B:  16 soft-threshold + u@S steps, all matmuls fp8-e4m3 DoubleRow
        (2 dict-tiles contracted per pass). First 12 steps single-pass
        (u8 @ S8H); last NSF8=4 steps "split-fp8": uh8 @ (S8H+S8L) +
        ul8 @ S8H, the cross terms folded into per-j DoubleRow matmuls by
        pairing weight slots (S8L[j],S8H[j]) against moving slots
        (uh[j],ul[j]) - DoubleRow multiplies same-index slots, so the u8
        tile stores (uh, ul) while the S tile stores (lo, hi).
        S8H = e4m3(32*S), S8L = e4m3(32*S - S8H); u is consumed UNSCALED
        (ACT applies the 2^-5 descale when emitting fp8), so psum comes out
        scaled 32 and adds directly onto v~0.
        f8-step shrink per pair of dict tiles: DVE add (psum+v~0 -> f16),
        GPSIMD fused clamp (tensor_scalar min/max, per-partition +-32*th),
        DVE sub (f16 2x), ACT copy->fp8 (scale 2^-5).
  - C:  out = (ah8 @ (DxH+DxL) + al8 @ DxH) * 2^-10, DoubleRow fp8 with the
        final shrink emitting a scaled-32 hi/lo split (exact: e4m3(32a) ==
        32*e4m3(a)); the 2^-10 descale rides the PSUM->SBUF ACT copy.
All phases fused per 256-column batch chunk; two chunks interleaved so the
tensor engine never waits on a shrink chain, and each pair's C matmuls are
deferred until after the next pair's A phase to fill the final-shrink
latency. Phase A and C have dedicated PSUM pools so the B-step psum ring
never blocks them.
"""

import numpy as np
from contextlib import ExitStack

import concourse.bass as bass
import concourse.bacc as bacc
import concourse.tile as tile
import concourse.mybir as mybir
from concourse.bass import ts, ds

P = 128
NCORES = 8
B_FULL, DIN, DD = 16384, 1024, 2048
BSH = B_FULL // NCORES      # 2048 batch rows per core
CH = 256                    # batch columns per chunk
NCH = BSH // CH             # 8 chunks
IT = DD // P                # 16 dict tiles
JP = IT // 2                # 8 DoubleRow pairs
KW = DIN // P               # 8 d_in tiles
CN = 512                    # free dim of phase-C matmuls
SC = 32.0                   # global scale 2^5
NSF8 = 4                    # trailing split-fp8 steps (rest single fp8)
GR = 2                      # dict tiles per shrink group
GRP = 4                     # dict tiles per psum/add group

F8 = mybir.dt.float8e4
F16 = mybir.dt.float16
F32 = mybir.dt.float32
ADD = mybir.AluOpType.add
SUB = mybir.AluOpType.subtract
MIN = mybir.AluOpType.min
MAX = mybir.AluOpType.max
RELU = mybir.ActivationFunctionType.Relu
COPY = mybir.ActivationFunctionType.Copy
DR = mybir.MatmulPerfMode.DoubleRow

_built = {}


def _build(steps: int):
    nc = bacc.Bacc("TRN2", target_bir_lowering=False, debug=False, num_devices=NCORES)

    def inp(name, shape, dt):
        return nc.dram_tensor(name, shape, dt, kind="ExternalInput").ap()

    yT16 = inp("yT16", (DIN, BSH), F16)
    W16_d = inp("W16", (DIN, DD), F16)        # f16(32*W)
    S8_d = inp("S8", (DD, 2, DD), F8)         # [j, (lo,hi), :] interleaved
    Dx8_d = inp("Dx8", (DD, 2, DIN), F8)        # [j, (lo,hi), :] * 32
    pth_d = inp("pth", (DD,), F32)            # +32*th
    nth_d = inp("nth", (DD,), F32)            # -32*th
    nthu_d = inp("nthu", (DD,), F32)          # -th (unscaled)
    out_d = nc.dram_tensor("out", (BSH, DIN), F32, kind="ExternalOutput").ap()

    n_sf8 = min(NSF8, steps)
    n_f8 = steps - n_sf8
    mode = ["f8"] * n_f8 + ["sf8"] * n_sf8    # mode[t] for step t (0-based)

    with tile.TileContext(nc) as tc, ExitStack() as top:
        thp = top.enter_context(tc.tile_pool(name="thp", bufs=1))
        pth_t = thp.tile([P, IT], F32)
        nth_t = thp.tile([P, IT], F32)
        nthu_t = thp.tile([P, IT], F32)

        # y(chunk 0) first, then W in half-tiles: phase A starts ~2.5us in and
        # overlaps the remaining W DMA instead of waiting for all of it.
        wp = top.enter_context(tc.tile_pool(name="wp", bufs=1))
        W_t = wp.tile([P, KW, DD], F16, name="W_t")
        yp = top.enter_context(tc.tile_pool(name="yp", bufs=2))
        y_pre = {}
        for c in (0, 1):
            y_t = yp.tile([P, KW, CH], F16, tag="y", name="y_pre")
            y_pre[c] = y_t
        for k in range(KW):
            nc.sync.dma_start(y_pre[0][:, k, :], yT16[ts(k, P), ds(0, CH)])
        for h in range(2):
            for k in range(KW):
                nc.sync.dma_start(W_t[:, k, ts(h, DD // 2)],
                                  W16_d[ts(k, P), ts(h, DD // 2)])
        for k in range(KW):
            nc.sync.dma_start(y_pre[1][:, k, :], yT16[ts(k, P), ds(CH, CH)])

        nc.sync.dma_start(pth_t[:], pth_d.rearrange("(io p) -> p io", p=P))
        nc.sync.dma_start(nth_t[:], nth_d.rearrange("(io p) -> p io", p=P))
        nc.sync.dma_start(nthu_t[:], nthu_d.rearrange("(io p) -> p io", p=P))

        sp = top.enter_context(tc.tile_pool(name="sp", bufs=1))
        S_t = sp.tile([P, IT, 2, DD], F8, name="S_t")   # slot0=lo, slot1=hi
        for j in range(IT):
            nc.sync.dma_start(S_t[:, j, :, :], S8_d[ts(j, P), :, :])

        dxp = top.enter_context(tc.tile_pool(name="dxp", bufs=1))
        Dx_t = dxp.tile([P, IT, 2, DIN], F8, name="Dx_t")   # slot0=lo, slot1=hi
        for io in range(IT):
            nc.sync.dma_start(Dx_t[:, io, :, :], Dx8_d[ts(io, P), :, :])
        v0p = top.enter_context(tc.tile_pool(name="v0p", bufs=2))
        up = top.enter_context(tc.tile_pool(name="up", bufs=3))
        psp = top.enter_context(tc.tile_pool(name="psp", bufs=2, space="PSUM"))
        psap = top.enter_context(tc.tile_pool(name="psap", bufs=2, space="PSUM"))
        pscp = top.enter_context(tc.tile_pool(name="pscp", bufs=2, space="PSUM"))
        vp = top.enter_context(tc.tile_pool(name="vp", bufs=4))
        cp = top.enter_context(tc.tile_pool(name="cp", bufs=2))
        u16p = top.enter_context(tc.tile_pool(name="u16p", bufs=4))
        # NOTE: SBUF is within ~1KB of full; keep pool sizes in sync with budget
        pqp = top.enter_context(tc.tile_pool(name="pqp", bufs=3))
        u32p = top.enter_context(tc.tile_pool(name="u32p", bufs=4))
        stp = top.enter_context(tc.tile_pool(name="stp", bufs=2))

        def shrink_f8(vsrc_quad, q, u8_t):
            """vsrc_quad: [P,4,CH] f16 AP (scaled-32 v). Writes uh into slot 0."""
            c_t = cp.tile([P, GR, CH], F16, tag="c")
            for s in range(GR):
                i = GR * q + s
                nc.gpsimd.tensor_scalar(
                    c_t[:, s, :], vsrc_quad[:, s, :],
                    pth_t[:, i:i + 1], nth_t[:, i:i + 1], MIN, op1=MAX)
            u16_t = u16p.tile([P, GR, CH], F16, tag="u16")
            nc.vector.tensor_tensor(u16_t[:], vsrc_quad, c_t[:], SUB)
            nc.scalar.activation(u8_t[:, GR * q:GR * q + GR, 0, :], u16_t[:],
                                 COPY, scale=1.0 / SC)

        def shrink_sf8(vsrc_quad, q, u8_t):
            """Split-fp8 shrink: uh -> slot0, ul -> slot1 (u8 slots REVERSED vs S8)."""
            for s in range(GR):
                i = GR * q + s
                bias = nthu_t[:, i:i + 1]
                p_t = pqp.tile([P, CH], F32, tag="p")
                q_t = pqp.tile([P, CH], F32, tag="q")
                nc.scalar.activation(p_t[:], vsrc_quad[:, s, :], RELU,
                                     bias=bias, scale=1.0 / SC)
                nc.scalar.activation(q_t[:], vsrc_quad[:, s, :], RELU,
                                     bias=bias, scale=-1.0 / SC)
                u32 = u32p.tile([P, CH], F32, tag="u32")
                nc.vector.tensor_tensor(u32[:], p_t[:], q_t[:], SUB)
                nc.vector.tensor_copy(u8_t[:, i, 0, :], u32[:])
                nc.gpsimd.tensor_tensor(u8_t[:, i, 1, :], u32[:], u8_t[:, i, 0, :], SUB)

        def shrink_af8(vsrc_pair, q, a_t):
            """Final shrink -> SCALED-32 split-fp8 a (ah8 slot0, al8 slot1).
            e4m3(32*a) == 32*e4m3(a) exactly, so phase C just descales by 2^-10."""
            c_t = cp.tile([P, GR, CH], F16, tag="c")
            for s in range(GR):
                i = GR * q + s
                nc.gpsimd.tensor_scalar(
                    c_t[:, s, :], vsrc_pair[:, s, :],
                    pth_t[:, i:i + 1], nth_t[:, i:i + 1], MIN, op1=MAX)
            u16_t = u16p.tile([P, GR, CH], F16, tag="u16")
            nc.vector.tensor_tensor(u16_t[:], vsrc_pair, c_t[:], SUB)
            nc.scalar.activation(a_t[:, GR * q:GR * q + GR, 0, :], u16_t[:],
                                 COPY, scale=1.0)
            nc.gpsimd.tensor_tensor(a_t[:, GR * q:GR * q + GR, 1, :], u16_t[:],
                                    a_t[:, GR * q:GR * q + GR, 0, :], SUB)

        NQ = IT // GR   # shrink groups per step

        def phase_a(c):
            cs = ds(c * CH, CH)
            if c in y_pre:
                y_t = y_pre.pop(c)
            else:
                y_t = yp.tile([P, KW, CH], F16, tag="y")
                for k in range(KW):
                    nc.sync.dma_start(y_t[:, k, :], yT16[ts(k, P), cs])
            v0_t = v0p.tile([P, IT, CH], F16, tag="v0")
            for q in range(NQ):
                ps = psap.tile([P, GR, CH], F32, tag="psA")
                for s in range(GR):
                    i = GR * q + s
                    for k in range(KW):
                        nc.tensor.matmul(ps[:, s, :], W_t[:, k, ts(i, P)],
                                         y_t[:, k, :],
                                         start=(k == 0), stop=(k == KW - 1))
                nc.scalar.activation(v0_t[:, GR * q:GR * q + GR, :], ps[:],
                                     COPY, scale=1.0)
            return v0_t

        def first_shrink(v0_t, u8_t, fmt):
            for q in range(NQ):
                if fmt == "f8":
                    shrink_f8(v0_t[:, GR * q:GR * q + GR, :], q, u8_t)
                else:
                    shrink_sf8(v0_t[:, GR * q:GR * q + GR, :], q, u8_t)

        def step_pair(t, pair, v0s, u8s, u8ns, ats):
            """One B step for both chunks of the pair (chunk-sequential)."""
            m = mode[t]
            last = t == steps - 1
            nxt = None if last else mode[t + 1]
            for c in pair:
              for qq in range(IT // GRP):
                v0_t, u8_t = v0s[c], u8s[c]
                u8_n = None if last else u8ns[c]
                a_t = ats[c] if last else None
                ps = psp.tile([P, GRP, CH], F32, tag="ps")
                for s in range(GRP):
                    i = GRP * qq + s
                    if m == "f8":
                        n_mm = JP
                        for jp in range(JP):
                            nc.tensor.matmul(
                                ps[:, s, :],
                                S_t[:, 2 * jp:2 * jp + 2, 1, ts(i, P)],
                                u8_t[:, 2 * jp:2 * jp + 2, 0, :],
                                start=(jp == 0), stop=(jp == n_mm - 1),
                                perf_mode=DR)
                    else:
                        n_mm = JP + IT
                        k = 0
                        for jp in range(JP):
                            nc.tensor.matmul(
                                ps[:, s, :],
                                S_t[:, 2 * jp:2 * jp + 2, 1, ts(i, P)],
                                u8_t[:, 2 * jp:2 * jp + 2, 0, :],
                                start=(k == 0), stop=(k == n_mm - 1),
                                perf_mode=DR)
                            k += 1
                        for j in range(IT):
                            nc.tensor.matmul(
                                ps[:, s, :],
                                S_t[:, j, :, ts(i, P)],
                                u8_t[:, j, :, :],
                                start=False, stop=(k == n_mm - 1),
                                perf_mode=DR)
                            k += 1
                v_t = vp.tile([P, GRP, CH], F16, tag="v")
                nc.vector.tensor_tensor(v_t[:], ps[:],
                                        v0_t[:, GRP * qq:GRP * qq + GRP, :], ADD)
                for h in range(GRP // GR):
                    q = (GRP // GR) * qq + h
                    v_pair = v_t[:, GR * h:GR * h + GR, :]
                    if last:
                        shrink_af8(v_pair, q, a_t)
                    elif nxt == "f8":
                        shrink_f8(v_pair, q, u8_n)
                    else:
                        shrink_sf8(v_pair, q, u8_n)

        def phase_c(c, a_t):
            # out = (ah @ (DxH + DxL) + al @ DxH) / 32, all DoubleRow fp8
            cs_base = c * CH
            for bt in range(CH // P):
                for dn in range(DIN // CN):
                    ps = pscp.tile([P, CN], F32, tag="psC")
                    n_mm = JP + IT
                    k = 0
                    for jp in range(JP):
                        nc.tensor.matmul(
                            ps[:], a_t[:, 2 * jp:2 * jp + 2, 0, ts(bt, P)],
                            Dx_t[:, 2 * jp:2 * jp + 2, 1, ts(dn, CN)],
                            start=(k == 0), stop=(k == n_mm - 1), perf_mode=DR)
                        k += 1
                    for j in range(IT):
                        nc.tensor.matmul(
                            ps[:], a_t[:, j, :, ts(bt, P)],
                            Dx_t[:, j, :, ts(dn, CN)],
                            start=False, stop=(k == n_mm - 1), perf_mode=DR)
                        k += 1
                    st = stp.tile([P, CN], F32, tag="st")
                    nc.scalar.activation(st[:], ps[:], COPY, scale=1.0 / (SC * SC))
                    nc.sync.dma_start(out_d[ds(cs_base + bt * P, P), ts(dn, CN)],
                                      st[:])

        # ---- main schedule: chunk pairs; previous pair's C is emitted after
        # the next pair's A so its matmuls fill the shrink-chain latency ----
        pending_c = []
        for cp0 in range(0, NCH, 2):
            pair = (cp0, cp0 + 1)
            v0s, u8s, ats = {}, {}, {}
            for c in pair:
                v0s[c] = phase_a(c)
                u8s[c] = up.tile([P, IT, 2, CH], F8, tag="u8", name="u8_t")
                first_shrink(v0s[c], u8s[c], mode[0])
                ats[c] = None
            for nxt_c in (cp0 + 2, cp0 + 3):
                if nxt_c < NCH and nxt_c not in y_pre:
                    y_t = yp.tile([P, KW, CH], F16, tag="y", name="y_nxt")
                    for k in range(KW):
                        nc.sync.dma_start(y_t[:, k, :],
                                          yT16[ts(k, P), ds(nxt_c * CH, CH)])
                    y_pre[nxt_c] = y_t
            for c, a_t in pending_c:
                phase_c(c, a_t)
            pending_c = []
            for t in range(steps):
                last = t == steps - 1
                u8ns = {}
                for c in pair:
                    if last:
                        u8ns[c] = None
                        ats[c] = up.tile([P, IT, 2, CH], F8, tag="u8", name="a_t")
                    else:
                        u8ns[c] = up.tile([P, IT, 2, CH], F8, tag="u8", name="u8_n")
                step_pair(t, pair, v0s, u8s, u8ns, ats)
                for c in pair:
                    u8s[c] = u8ns[c]
            pending_c = [(c, ats[c]) for c in pair]
        for c, a_t in pending_c:
            phase_c(c, a_t)

    nc.compile()
    return nc


def _prep_in_maps(y, W, Theta, S, Dx):
    import ml_dtypes
    E4 = ml_dtypes.float8_e4m3  # TRN flavor (max normal 240)

    y = np.asarray(y, dtype=np.float32)
    W = np.asarray(W, dtype=np.float32)
    Theta = np.asarray(Theta, dtype=np.float32)
    S = np.asarray(S, dtype=np.float32)
    Dx = np.asarray(Dx, dtype=np.float32)
    assert y.shape == (B_FULL, DIN) and W.shape == (DIN, DD)
    assert S.shape == (DD, DD) and Dx.shape == (DD, DIN)

    th = np.maximum(Theta, 0.0) + np.float32(1e-7)
    W16 = (SC * W).astype(np.float16)
    Ss = np.float32(SC) * S
    S8H = np.clip(Ss, -240, 240).astype(E4)
    S8L = np.clip(Ss - S8H.astype(np.float32), -240, 240).astype(E4)
    S8 = np.stack([S8L, S8H], axis=1)         # [DD, 2, DD], slot0=lo slot1=hi
    Dxs = np.float32(SC) * Dx
    Dx8H = np.clip(Dxs, -240, 240).astype(E4)
    Dx8L = np.clip(Dxs - Dx8H.astype(np.float32), -240, 240).astype(E4)
    Dx8 = np.ascontiguousarray(np.stack([Dx8L, Dx8H], axis=1))
    yT16 = np.ascontiguousarray(y.T).astype(np.float16)

    shared = dict(
        W16=W16, S8=np.ascontiguousarray(S8), Dx8=Dx8,
        pth=(SC * th).astype(np.float32),
        nth=(-SC * th).astype(np.float32),
        nthu=(-th).astype(np.float32),
    )
    in_maps = []
    for c in range(NCORES):
        sl = slice(c * BSH, (c + 1) * BSH)
        in_maps.append(dict(shared, yT16=np.ascontiguousarray(yT16[:, sl])))
    return in_maps


_sharded_cache = {}


def _get_sharded(steps: int):
    """Build (once) the jitted shard_map executable for the compiled NEFF."""
    if steps in _sharded_cache:
        return _sharded_cache[steps]
    import jax
    from jax.experimental.shard_map import shard_map
    from jax.sharding import Mesh, PartitionSpec
    from concourse import bass2jax

    if steps not in _built:
        _built[steps] = _build(steps)
    nc = _built[steps]
    bass2jax.install_neuronx_cc_hook()
    assert nc.dbg_addr is None
    partition_name = nc.partition_id_tensor.name if nc.partition_id_tensor else None

    in_names, out_names, out_avals, zero_shapes = [], [], [], []
    for alloc in nc.m.functions[0].allocations:
        if not isinstance(alloc, mybir.MemoryLocationSet):
            continue
        name = alloc.memorylocations[0].name
        if alloc.kind == "ExternalInput":
            if name != partition_name:
                in_names.append(name)
        elif alloc.kind == "ExternalOutput":
            out_names.append(name)
            shape = tuple(alloc.tensor_shape)
            dtype = mybir.dt.np(alloc.dtype)
            out_avals.append(jax.core.ShapedArray(shape, dtype))
            zero_shapes.append((shape, dtype))
    n_params = len(in_names)
    n_outs = len(out_names)
    all_in_names = in_names + out_names
    if partition_name is not None:
        all_in_names.append(partition_name)

    def _body(*args):
        operands = list(args)
        if partition_name is not None:
            operands.append(bass2jax.partition_id_tensor())
        outs = bass2jax._bass_exec_p.bind(
            *operands,
            out_avals=tuple(out_avals),
            in_names=tuple(all_in_names),
            out_names=tuple(out_names),
            lowering_input_output_aliases=(),
            sim_require_finite=True,
            sim_require_nnan=True,
            nc=nc,
        )
        return tuple(outs)

    devices = jax.devices()[:NCORES]
    mesh = Mesh(np.asarray(devices), ("core",))
    donate = tuple(range(n_params, n_params + n_outs))
    sharded = jax.jit(
        shard_map(_body, mesh=mesh,
                  in_specs=(PartitionSpec("core"),) * (n_params + n_outs),
                  out_specs=(PartitionSpec("core"),) * n_outs,
                  check_rep=False),
        donate_argnums=donate, keep_unused=True)
    entry = dict(sharded=sharded, in_names=in_names, out_names=out_names,
                 zero_shapes=zero_shapes, mesh=mesh, n_params=n_params)
    _sharded_cache[steps] = entry
    return entry


def _concat_inputs(entry, in_maps):
    return [np.concatenate([np.asarray(in_maps[c][n]) for c in range(NCORES)], axis=0)
            for n in entry["in_names"]]


def _run(entry, concat_in):
    zeros = [np.zeros((NCORES * s[0], *s[1:]), d) for s, d in entry["zero_shapes"]]
    out_arrs = entry["sharded"](*concat_in, *zeros)
    return out_arrs


def kernel(y, W, Theta, S, Dx, unroll_steps):
    steps = int(unroll_steps)
    entry = _get_sharded(steps)
    in_maps = _prep_in_maps(y, W, Theta, S, Dx)
    out_arrs = _run(entry, _concat_inputs(entry, in_maps))
    idx = entry["out_names"].index("out")
    return np.ascontiguousarray(np.asarray(out_arrs[idx]))  # [NCORES*BSH, DIN]


def time_kernel(np_inputs, iters=6):
    """Steady-state wall time per NEFF execution (ns), device-resident inputs."""
    import jax
    from jax.sharding import NamedSharding, PartitionSpec
    steps = int(np_inputs["unroll_steps"])
    entry = _get_sharded(steps)
    in_maps = _prep_in_maps(np_inputs["y"], np_inputs["W"], np_inputs["Theta"],
                            np_inputs["S"], np_inputs["Dx"])
    concat_in = _concat_inputs(entry, in_maps)
    sh = NamedSharding(entry["mesh"], PartitionSpec("core"))
    dev_in = [jax.device_put(a, sh) for a in concat_in]
    import time as _time
    times = []
    for it in range(iters):
        zeros = [jax.device_put(np.zeros((NCORES * s[0], *s[1:]), d), sh)
                 for s, d in entry["zero_shapes"]]
        for z in zeros:
            z.block_until_ready()
        t0 = _time.perf_counter()
        outs = entry["sharded"](*dev_in, *zeros)
        for o in outs:
            o.block_until_ready()
        times.append(_time.perf_counter() - t0)
    best = min(times[1:]) if len(times) > 1 else times[0]
    print("  per-iter times (ms):", [f"{t*1e3:.1f}" for t in times])
    return best * 1e9


if __name__ == "__main__":
    rng = np.random.default_rng(0)
    inputs = dict(
        y=rng.standard_normal((B_FULL, DIN), dtype=np.float32),
        W=(rng.standard_normal((DIN, DD)) * 0.02).astype(np.float32),
        Theta=rng.random(DD, dtype=np.float32),
        S=(rng.standard_normal((DD, DD)) * 0.02).astype(np.float32),
        Dx=(rng.standard_normal((DD, DIN)) * 0.02).astype(np.float32),
        unroll_steps=16,
    )
    out = kernel(**inputs)
    print("out", out.shape, out.dtype, np.abs(out).max())


# revision 61
# speedup vs baseline: 4.9256x; 1.0020x over previous
"""Trainium2 Bass kernel for a LISTA layer (nn_ListaLayer).

Reference computation (jax, fp32):
    th = relu(Theta) + 1e-7
    xW = (y @ W) / th
    repeat 16: z = xW + (unit_threshold(z) * th @ S) / th
    out = (unit_threshold(z) * th) @ Dx
where unit_threshold(v) = sign(v) * relu(|v| - 1).

Algebraic restructure (exact): track v = z * th:
    v0 = y @ W
    repeat 16:  u = soft_threshold(v, th) = sign(v) * relu(|v| - th)
                v = v0 + u @ S
    out = soft_threshold(v, th) @ Dx

Distribution: data-parallel over batch rows, 8 NeuronCores, 2048 rows each.
W/th/S/Dx replicated; no collectives.

Numerics / performance scheme (v-space carried SCALED by 32 in fp16):
  - A:  v~0 = y16 @ f16(32*W)   (fp16 matmul, fp32 PSUM)
  - B:  16 soft-threshold + u@S steps, all matmuls fp8-e4m3 DoubleRow
        (2 dict-tiles contracted per pass). First 12 steps single-pass
        (u8 @ S8H); last NSF8=4 steps "split-fp8": uh8 @ (S8H+S8L) +
        ul8 @ S8H, the cross terms folded into per-j DoubleRow matmuls by
        pairing weight slots (S8L[j],S8H[j]) against moving slots
        (uh[j],ul[j]) - DoubleRow multiplies same-index slots, so the u8
        tile stores (uh, ul) while the S tile stores (lo, hi).
        S8H = e4m3(32*S), S8L = e4m3(32*S - S8H); u is consumed UNSCALED
        (ACT applies the 2^-5 descale when emitting fp8), so psum comes out
        scaled 32 and adds directly onto v~0.
        f8-step shrink per pair of dict tiles: DVE add (psum+v~0 -> f16),
        GPSIMD fused clamp (tensor_scalar min/max, per-partition +-32*th),
        DVE sub (f16 2x), ACT copy->fp8 (scale 2^-5).
  - C:  out = (ah8 @ (DxH+DxL) + al8 @ DxH) * 2^-10, DoubleRow fp8 with the
        final shrink emitting a scaled-32 hi/lo split (exact: e4m3(32a) ==
        32*e4m3(a)); the 2^-10 descale rides the PSUM->SBUF ACT copy.
All phases fused per 256-column batch chunk; two chunks interleaved so the
tensor engine never waits on a shrink chain, and each pair's C matmuls are
deferred until after the next pair's A phase to fill the final-shrink
latency. Phase A and C have dedicated PSUM pools so the B-step psum ring
never blocks them.
"""

import numpy as np
from contextlib import ExitStack

import concourse.bass as bass
import concourse.bacc as bacc
import concourse.tile as tile
import concourse.mybir as mybir
from concourse.bass import ts, ds

P = 128
NCORES = 8
B_FULL, DIN, DD = 16384, 1024, 2048
BSH = B_FULL // NCORES      # 2048 batch rows per core
CH = 256                    # batch columns per chunk
NCH = BSH // CH             # 8 chunks
IT = DD // P                # 16 dict tiles
JP = IT // 2                # 8 DoubleRow pairs
KW = DIN // P               # 8 d_in tiles
CN = 512                    # free dim of phase-C matmuls
SC = 32.0                   # global scale 2^5
NSF8 = 4                    # trailing split-fp8 steps (rest single fp8)
GR = 2                      # dict tiles per shrink group
GRP = 4                     # dict tiles per psum/add group

F8 = mybir.dt.float8e4
F16 = mybir.dt.float16
F32 = mybir.dt.float32
ADD = mybir.AluOpType.add
SUB = mybir.AluOpType.subtract
MIN = mybir.AluOpType.min
MAX = mybir.AluOpType.max
RELU = mybir.ActivationFunctionType.Relu
COPY = mybir.ActivationFunctionType.Copy
DR = mybir.MatmulPerfMode.DoubleRow

_built = {}


def _build(steps: int):
    nc = bacc.Bacc("TRN2", target_bir_lowering=False, debug=False, num_devices=NCORES)

    def inp(name, shape, dt):
        return nc.dram_tensor(name, shape, dt, kind="ExternalInput").ap()

    yT16 = inp("yT16", (DIN, BSH), F16)
    W16_d = inp("W16", (DIN, DD), F16)        # f16(32*W)
    S8_d = inp("S8", (DD, 2, DD), F8)         # [j, (lo,hi), :] interleaved
    Dx8_d = inp("Dx8", (DD, 2, DIN), F8)        # [j, (lo,hi), :] * 32
    pth_d = inp("pth", (DD,), F32)            # +32*th
    nth_d = inp("nth", (DD,), F32)            # -32*th
    nthu_d = inp("nthu", (DD,), F32)          # -th (unscaled)
    out_d = nc.dram_tensor("out", (BSH, DIN), F32, kind="ExternalOutput").ap()

    n_sf8 = min(NSF8, steps)
    n_f8 = steps - n_sf8
    mode = ["f8"] * n_f8 + ["sf8"] * n_sf8    # mode[t] for step t (0-based)

    with tile.TileContext(nc) as tc, ExitStack() as top:
        thp = top.enter_context(tc.tile_pool(name="thp", bufs=1))
        pth_t = thp.tile([P, IT], F32)
        nth_t = thp.tile([P, IT], F32)
        nthu_t = thp.tile([P, IT], F32)

        # y(chunk 0) first, then W in half-tiles: phase A starts ~2.5us in and
        # overlaps the remaining W DMA instead of waiting for all of it.
        wp = top.enter_context(tc.tile_pool(name="wp", bufs=1))
        W_t = wp.tile([P, KW, DD], F16, name="W_t")
        yp = top.enter_context(tc.tile_pool(name="yp", bufs=2))
        y_pre = {}
        for c in (0, 1):
            y_t = yp.tile([P, KW, CH], F16, tag="y", name="y_pre")
            y_pre[c] = y_t
        for k in range(KW):
            nc.sync.dma_start(y_pre[0][:, k, :], yT16[ts(k, P), ds(0, CH)])
        for h in range(2):
            for k in range(KW):
                nc.sync.dma_start(W_t[:, k, ts(h, DD // 2)],
                                  W16_d[ts(k, P), ts(h, DD // 2)])
        for k in range(KW):
            nc.sync.dma_start(y_pre[1][:, k, :], yT16[ts(k, P), ds(CH, CH)])

        nc.sync.dma_start(pth_t[:], pth_d.rearrange("(io p) -> p io", p=P))
        nc.sync.dma_start(nth_t[:], nth_d.rearrange("(io p) -> p io", p=P))
        nc.sync.dma_start(nthu_t[:], nthu_d.rearrange("(io p) -> p io", p=P))

        sp = top.enter_context(tc.tile_pool(name="sp", bufs=1))
        S_t = sp.tile([P, IT, 2, DD], F8, name="S_t")   # slot0=lo, slot1=hi
        for j in range(IT):
            nc.sync.dma_start(S_t[:, j, :, :], S8_d[ts(j, P), :, :])

        dxp = top.enter_context(tc.tile_pool(name="dxp", bufs=1))
        Dx_t = dxp.tile([P, IT, 2, DIN], F8, name="Dx_t")   # slot0=lo, slot1=hi
        for io in range(IT):
            nc.sync.dma_start(Dx_t[:, io, :, :], Dx8_d[ts(io, P), :, :])
        v0p = top.enter_context(tc.tile_pool(name="v0p", bufs=2))
        up = top.enter_context(tc.tile_pool(name="up", bufs=3))
        psp = top.enter_context(tc.tile_pool(name="psp", bufs=2, space="PSUM"))
        psap = top.enter_context(tc.tile_pool(name="psap", bufs=2, space="PSUM"))
        pscp = top.enter_context(tc.tile_pool(name="pscp", bufs=2, space="PSUM"))
        vp = top.enter_context(tc.tile_pool(name="vp", bufs=4))
        cp = top.enter_context(tc.tile_pool(name="cp", bufs=2))
        u16p = top.enter_context(tc.tile_pool(name="u16p", bufs=4))
        # NOTE: SBUF is within ~1KB of full; keep pool sizes in sync with budget
        pqp = top.enter_context(tc.tile_pool(name="pqp", bufs=4))
        u32p = top.enter_context(tc.tile_pool(name="u32p", bufs=4))
        stp = top.enter_context(tc.tile_pool(name="stp", bufs=2))

        def shrink_f8(vsrc_quad, q, u8_t):
            """vsrc_quad: [P,4,CH] f16 AP (scaled-32 v). Writes uh into slot 0."""
            c_t = cp.tile([P, GR, CH], F16, tag="c")
            for s in range(GR):
                i = GR * q + s
                nc.gpsimd.tensor_scalar(
                    c_t[:, s, :], vsrc_quad[:, s, :],
                    pth_t[:, i:i + 1], nth_t[:, i:i + 1], MIN, op1=MAX)
            u16_t = u16p.tile([P, GR, CH], F16, tag="u16")
            nc.vector.tensor_tensor(u16_t[:], vsrc_quad, c_t[:], SUB)
            nc.scalar.activation(u8_t[:, GR * q:GR * q + GR, 0, :], u16_t[:],
                                 COPY, scale=1.0 / SC)

        def shrink_sf8(vsrc_quad, q, u8_t):
            """Split-fp8 shrink: uh -> slot0, ul -> slot1 (u8 slots REVERSED vs S8)."""
            for s in range(GR):
                i = GR * q + s
                bias = nthu_t[:, i:i + 1]
                p_t = pqp.tile([P, CH], F32, tag="p")
                q_t = pqp.tile([P, CH], F32, tag="q")
                nc.scalar.activation(p_t[:], vsrc_quad[:, s, :], RELU,
                                     bias=bias, scale=1.0 / SC)
                nc.scalar.activation(q_t[:], vsrc_quad[:, s, :], RELU,
                                     bias=bias, scale=-1.0 / SC)
                u32 = u32p.tile([P, CH], F32, tag="u32")
                nc.vector.tensor_tensor(u32[:], p_t[:], q_t[:], SUB)
                nc.vector.tensor_copy(u8_t[:, i, 0, :], u32[:])
                nc.gpsimd.tensor_tensor(u8_t[:, i, 1, :], u32[:], u8_t[:, i, 0, :], SUB)

        def shrink_af8(vsrc_pair, q, a_t):
            """Final shrink -> SCALED-32 split-fp8 a (ah8 slot0, al8 slot1).
            e4m3(32*a) == 32*e4m3(a) exactly, so phase C just descales by 2^-10."""
            c_t = cp.tile([P, GR, CH], F16, tag="c")
            for s in range(GR):
                i = GR * q + s
                nc.gpsimd.tensor_scalar(
                    c_t[:, s, :], vsrc_pair[:, s, :],
                    pth_t[:, i:i + 1], nth_t[:, i:i + 1], MIN, op1=MAX)
            u16_t = u16p.tile([P, GR, CH], F16, tag="u16")
            nc.vector.tensor_tensor(u16_t[:], vsrc_pair, c_t[:], SUB)
            nc.scalar.activation(a_t[:, GR * q:GR * q + GR, 0, :], u16_t[:],
                                 COPY, scale=1.0)
            nc.gpsimd.tensor_tensor(a_t[:, GR * q:GR * q + GR, 1, :], u16_t[:],
                                    a_t[:, GR * q:GR * q + GR, 0, :], SUB)

        NQ = IT // GR   # shrink groups per step

        def phase_a(c):
            cs = ds(c * CH, CH)
            if c in y_pre:
                y_t = y_pre.pop(c)
            else:
                y_t = yp.tile([P, KW, CH], F16, tag="y")
                for k in range(KW):
                    nc.sync.dma_start(y_t[:, k, :], yT16[ts(k, P), cs])
            v0_t = v0p.tile([P, IT, CH], F16, tag="v0")
            for q in range(NQ):
                ps = psap.tile([P, GR, CH], F32, tag="psA")
                for s in range(GR):
                    i = GR * q + s
                    for k in range(KW):
                        nc.tensor.matmul(ps[:, s, :], W_t[:, k, ts(i, P)],
                                         y_t[:, k, :],
                                         start=(k == 0), stop=(k == KW - 1))
                nc.scalar.activation(v0_t[:, GR * q:GR * q + GR, :], ps[:],
                                     COPY, scale=1.0)
            return v0_t

        def first_shrink(v0_t, u8_t, fmt):
            for q in range(NQ):
                if fmt == "f8":
                    shrink_f8(v0_t[:, GR * q:GR * q + GR, :], q, u8_t)
                else:
                    shrink_sf8(v0_t[:, GR * q:GR * q + GR, :], q, u8_t)

        def step_pair(t, pair, v0s, u8s, u8ns, ats):
            """One B step for both chunks of the pair (chunk-sequential)."""
            m = mode[t]
            last = t == steps - 1
            nxt = None if last else mode[t + 1]
            for c in pair:
              for qq in range(IT // GRP):
                v0_t, u8_t = v0s[c], u8s[c]
                u8_n = None if last else u8ns[c]
                a_t = ats[c] if last else None
                ps = psp.tile([P, GRP, CH], F32, tag="ps")
                for s in range(GRP):
                    i = GRP * qq + s
                    if m == "f8":
                        n_mm = JP
                        for jp in range(JP):
                            nc.tensor.matmul(
                                ps[:, s, :],
                                S_t[:, 2 * jp:2 * jp + 2, 1, ts(i, P)],
                                u8_t[:, 2 * jp:2 * jp + 2, 0, :],
                                start=(jp == 0), stop=(jp == n_mm - 1),
                                perf_mode=DR)
                    else:
                        n_mm = JP + IT
                        k = 0
                        for jp in range(JP):
                            nc.tensor.matmul(
                                ps[:, s, :],
                                S_t[:, 2 * jp:2 * jp + 2, 1, ts(i, P)],
                                u8_t[:, 2 * jp:2 * jp + 2, 0, :],
                                start=(k == 0), stop=(k == n_mm - 1),
                                perf_mode=DR)
                            k += 1
                        for j in range(IT):
                            nc.tensor.matmul(
                                ps[:, s, :],
                                S_t[:, j, :, ts(i, P)],
                                u8_t[:, j, :, :],
                                start=False, stop=(k == n_mm - 1),
                                perf_mode=DR)
                            k += 1
                v_t = vp.tile([P, GRP, CH], F16, tag="v")
                nc.vector.tensor_tensor(v_t[:], ps[:],
                                        v0_t[:, GRP * qq:GRP * qq + GRP, :], ADD)
                for h in range(GRP // GR):
                    q = (GRP // GR) * qq + h
                    v_pair = v_t[:, GR * h:GR * h + GR, :]
                    if last:
                        shrink_af8(v_pair, q, a_t)
                    elif nxt == "f8":
                        shrink_f8(v_pair, q, u8_n)
                    else:
                        shrink_sf8(v_pair, q, u8_n)

        def phase_c(c, a_t):
            # out = (ah @ (DxH + DxL) + al @ DxH) / 32, all DoubleRow fp8
            cs_base = c * CH
            for bt in range(CH // P):
                for dn in range(DIN // CN):
                    ps = pscp.tile([P, CN], F32, tag="psC")
                    n_mm = JP + IT
                    k = 0
                    for jp in range(JP):
                        nc.tensor.matmul(
                            ps[:], a_t[:, 2 * jp:2 * jp + 2, 0, ts(bt, P)],
                            Dx_t[:, 2 * jp:2 * jp + 2, 1, ts(dn, CN)],
                            start=(k == 0), stop=(k == n_mm - 1), perf_mode=DR)
                        k += 1
                    for j in range(IT):
                        nc.tensor.matmul(
                            ps[:], a_t[:, j, :, ts(bt, P)],
                            Dx_t[:, j, :, ts(dn, CN)],
                            start=False, stop=(k == n_mm - 1), perf_mode=DR)
                        k += 1
                    st = stp.tile([P, CN], F32, tag="st")
                    nc.scalar.activation(st[:], ps[:], COPY, scale=1.0 / (SC * SC))
                    nc.sync.dma_start(out_d[ds(cs_base + bt * P, P), ts(dn, CN)],
                                      st[:])

        # ---- main schedule: chunk pairs; previous pair's C is emitted after
        # the next pair's A so its matmuls fill the shrink-chain latency ----
        pending_c = []
        for cp0 in range(0, NCH, 2):
            pair = (cp0, cp0 + 1)
            v0s, u8s, ats = {}, {}, {}
            for c in pair:
                v0s[c] = phase_a(c)
                u8s[c] = up.tile([P, IT, 2, CH], F8, tag="u8", name="u8_t")
                first_shrink(v0s[c], u8s[c], mode[0])
                ats[c] = None
            for nxt_c in (cp0 + 2, cp0 + 3):
                if nxt_c < NCH and nxt_c not in y_pre:
                    y_t = yp.tile([P, KW, CH], F16, tag="y", name="y_nxt")
                    for k in range(KW):
                        nc.sync.dma_start(y_t[:, k, :],
                                          yT16[ts(k, P), ds(nxt_c * CH, CH)])
                    y_pre[nxt_c] = y_t
            for c, a_t in pending_c:
                phase_c(c, a_t)
            pending_c = []
            for t in range(steps):
                last = t == steps - 1
                u8ns = {}
                for c in pair:
                    if last:
                        u8ns[c] = None
                        ats[c] = up.tile([P, IT, 2, CH], F8, tag="u8", name="a_t")
                    else:
                        u8ns[c] = up.tile([P, IT, 2, CH], F8, tag="u8", name="u8_n")
                step_pair(t, pair, v0s, u8s, u8ns, ats)
                for c in pair:
                    u8s[c] = u8ns[c]
            pending_c = [(c, ats[c]) for c in pair]
        for c, a_t in pending_c:
            phase_c(c, a_t)

    nc.compile()
    return nc


def _prep_in_maps(y, W, Theta, S, Dx):
    import ml_dtypes
    E4 = ml_dtypes.float8_e4m3  # TRN flavor (max normal 240)

    y = np.asarray(y, dtype=np.float32)
    W = np.asarray(W, dtype=np.float32)
    Theta = np.asarray(Theta, dtype=np.float32)
    S = np.asarray(S, dtype=np.float32)
    Dx = np.asarray(Dx, dtype=np.float32)
    assert y.shape == (B_FULL, DIN) and W.shape == (DIN, DD)
    assert S.shape == (DD, DD) and Dx.shape == (DD, DIN)

    th = np.maximum(Theta, 0.0) + np.float32(1e-7)
    W16 = (SC * W).astype(np.float16)
    Ss = np.float32(SC) * S
    S8H = np.clip(Ss, -240, 240).astype(E4)
    S8L = np.clip(Ss - S8H.astype(np.float32), -240, 240).astype(E4)
    S8 = np.stack([S8L, S8H], axis=1)         # [DD, 2, DD], slot0=lo slot1=hi
    Dxs = np.float32(SC) * Dx
    Dx8H = np.clip(Dxs, -240, 240).astype(E4)
    Dx8L = np.clip(Dxs - Dx8H.astype(np.float32), -240, 240).astype(E4)
    Dx8 = np.ascontiguousarray(np.stack([Dx8L, Dx8H], axis=1))
    yT16 = np.ascontiguousarray(y.T).astype(np.float16)

    shared = dict(
        W16=W16, S8=np.ascontiguousarray(S8), Dx8=Dx8,
        pth=(SC * th).astype(np.float32),
        nth=(-SC * th).astype(np.float32),
        nthu=(-th).astype(np.float32),
    )
    in_maps = []
    for c in range(NCORES):
        sl = slice(c * BSH, (c + 1) * BSH)
        in_maps.append(dict(shared, yT16=np.ascontiguousarray(yT16[:, sl])))
    return in_maps


_sharded_cache = {}


def _get_sharded(steps: int):
    """Build (once) the jitted shard_map executable for the compiled NEFF."""
    if steps in _sharded_cache:
        return _sharded_cache[steps]
    import jax
    from jax.experimental.shard_map import shard_map
    from jax.sharding import Mesh, PartitionSpec
    from concourse import bass2jax

    if steps not in _built:
        _built[steps] = _build(steps)
    nc = _built[steps]
    bass2jax.install_neuronx_cc_hook()
    assert nc.dbg_addr is None
    partition_name = nc.partition_id_tensor.name if nc.partition_id_tensor else None

    in_names, out_names, out_avals, zero_shapes = [], [], [], []
    for alloc in nc.m.functions[0].allocations:
        if not isinstance(alloc, mybir.MemoryLocationSet):
            continue
        name = alloc.memorylocations[0].name
        if alloc.kind == "ExternalInput":
            if name != partition_name:
                in_names.append(name)
        elif alloc.kind == "ExternalOutput":
            out_names.append(name)
            shape = tuple(alloc.tensor_shape)
            dtype = mybir.dt.np(alloc.dtype)
            out_avals.append(jax.core.ShapedArray(shape, dtype))
            zero_shapes.append((shape, dtype))
    n_params = len(in_names)
    n_outs = len(out_names)
    all_in_names = in_names + out_names
    if partition_name is not None:
        all_in_names.append(partition_name)

    def _body(*args):
        operands = list(args)
        if partition_name is not None:
            operands.append(bass2jax.partition_id_tensor())
        outs = bass2jax._bass_exec_p.bind(
            *operands,
            out_avals=tuple(out_avals),
            in_names=tuple(all_in_names),
            out_names=tuple(out_names),
            lowering_input_output_aliases=(),
            sim_require_finite=True,
            sim_require_nnan=True,
            nc=nc,
        )
        return tuple(outs)

    devices = jax.devices()[:NCORES]
    mesh = Mesh(np.asarray(devices), ("core",))
    donate = tuple(range(n_params, n_params + n_outs))
    sharded = jax.jit(
        shard_map(_body, mesh=mesh,
                  in_specs=(PartitionSpec("core"),) * (n_params + n_outs),
                  out_specs=(PartitionSpec("core"),) * n_outs,
                  check_rep=False),
        donate_argnums=donate, keep_unused=True)
    entry = dict(sharded=sharded, in_names=in_names, out_names=out_names,
                 zero_shapes=zero_shapes, mesh=mesh, n_params=n_params)
    _sharded_cache[steps] = entry
    return entry


def _concat_inputs(entry, in_maps):
    return [np.concatenate([np.asarray(in_maps[c][n]) for c in range(NCORES)], axis=0)
            for n in entry["in_names"]]


def _run(entry, concat_in):
    zeros = [np.zeros((NCORES * s[0], *s[1:]), d) for s, d in entry["zero_shapes"]]
    out_arrs = entry["sharded"](*concat_in, *zeros)
    return out_arrs


def kernel(y, W, Theta, S, Dx, unroll_steps):
    steps = int(unroll_steps)
    entry = _get_sharded(steps)
    in_maps = _prep_in_maps(y, W, Theta, S, Dx)
    out_arrs = _run(entry, _concat_inputs(entry, in_maps))
    idx = entry["out_names"].index("out")
    return np.ascontiguousarray(np.asarray(out_arrs[idx]))  # [NCORES*BSH, DIN]


def time_kernel(np_inputs, iters=6):
    """Steady-state wall time per NEFF execution (ns), device-resident inputs."""
    import jax
    from jax.sharding import NamedSharding, PartitionSpec
    steps = int(np_inputs["unroll_steps"])
    entry = _get_sharded(steps)
    in_maps = _prep_in_maps(np_inputs["y"], np_inputs["W"], np_inputs["Theta"],
                            np_inputs["S"], np_inputs["Dx"])
    concat_in = _concat_inputs(entry, in_maps)
    sh = NamedSharding(entry["mesh"], PartitionSpec("core"))
    dev_in = [jax.device_put(a, sh) for a in concat_in]
    import time as _time
    times = []
    for it in range(iters):
        zeros = [jax.device_put(np.zeros((NCORES * s[0], *s[1:]), d), sh)
                 for s, d in entry["zero_shapes"]]
        for z in zeros:
            z.block_until_ready()
        t0 = _time.perf_counter()
        outs = entry["sharded"](*dev_in, *zeros)
        for o in outs:
            o.block_until_ready()
        times.append(_time.perf_counter() - t0)
    best = min(times[1:]) if len(times) > 1 else times[0]
    print("  per-iter times (ms):", [f"{t*1e3:.1f}" for t in times])
    return best * 1e9


if __name__ == "__main__":
    rng = np.random.default_rng(0)
    inputs = dict(
        y=rng.standard_normal((B_FULL, DIN), dtype=np.float32),
        W=(rng.standard_normal((DIN, DD)) * 0.02).astype(np.float32),
        Theta=rng.random(DD, dtype=np.float32),
        S=(rng.standard_normal((DD, DD)) * 0.02).astype(np.float32),
        Dx=(rng.standard_normal((DD, DIN)) * 0.02).astype(np.float32),
        unroll_steps=16,
    )
    out = kernel(**inputs)
    print("out", out.shape, out.dtype, np.abs(out).max())
